# revision 53
# baseline (speedup 1.0000x reference)
"""Trainium2 Bass kernel for nn_BiVision_VQA2 (B=64,T=32,D=768,N=901).

Data-parallel over batch: 8 batch elems per core x 8 cores.
Key math simplifications (validated vs reference, rel err ~1e-4):
  - ga/go attention use a single key token -> softmax==1 -> those paths are
    linear in cls; question_embeds is mathematically unused.
  - GRU input `a` is constant over time; wx computed once.
  - local attention: row-constant score terms drop out of softmax; query
    pooling applied to the attention matrix before the @X contraction.
Performance structure:
  - GRU computed in TRANSPOSED gate layout [128(gate row), batch] via
    weight-stationary matmuls (moving N=8), elementwise on [128, 48].
  - bf16 everywhere outside the GRU recurrence (DMA casts on load).
  - phase D: transpose-free paT/ctxT via natural-operand-stationary matmuls.
"""

import os
import numpy as np
from contextlib import ExitStack

import concourse.bass as bass
import concourse.tile as tile
from concourse import bacc, mybir
from concourse.bass_utils import run_bass_kernel_spmd
from concourse.masks import make_identity

FP = mybir.dt.float32
FPR = mybir.dt.float32r
OP = mybir.AluOpType
AF = mybir.ActivationFunctionType
BF = mybir.dt.bfloat16

NCORES = 8
BL = 8
D = 768
T = 32
G = 3 * D
NK = 900
NH = 2
DK = 384
ET = D // 128
RQ = BL * T

CH_G = [(0, 512), (512, 512), (1024, 512), (1536, 512), (2048, 256)]
CH_NK = [(0, 512), (512, 388)]

GBF = os.environ.get("KGRUBF", "1") == "1"
GDT = BF if GBF else FP


def _r(ap):
    return ap if GBF else ap.bitcast(FPR)


def kchunks(n):
    out, o = [], 0
    while o < n:
        out.append((o, min(128, n - o)))
        o += 128
    return out


def build():
    nc = bacc.Bacc("TRN2", target_bir_lowering=False, debug=False,
                   enable_asserts=False)

    img = nc.dram_tensor("img", [BL, 901, D], FP, kind="ExternalInput").ap()
    h0 = nc.dram_tensor("h0", [BL, D], FP, kind="ExternalInput").ap()
    w_ih = nc.dram_tensor("gru_w_ih", [G, D], FP, kind="ExternalInput").ap()
    w_hh = nc.dram_tensor("gru_w_hh", [G, D], FP, kind="ExternalInput").ap()
    b_ih = nc.dram_tensor("gru_b_ih", [G], FP, kind="ExternalInput").ap()
    b_hh = nc.dram_tensor("gru_b_hh", [G], FP, kind="ExternalInput").ap()
    ga_w = nc.dram_tensor("ga_w", [4, D, D], FP, kind="ExternalInput").ap()
    ga_b = nc.dram_tensor("ga_b", [4, D], FP, kind="ExternalInput").ap()
    ga_pool = nc.dram_tensor("ga_pool", [1], FP, kind="ExternalInput").ap()
    la_w = nc.dram_tensor("la_w", [4, D, D], FP, kind="ExternalInput").ap()
    la_b = nc.dram_tensor("la_b", [4, D], FP, kind="ExternalInput").ap()
    la_pool = nc.dram_tensor("la_pool", [T], FP, kind="ExternalInput").ap()
    go_w = nc.dram_tensor("go_w", [4, D, D], FP, kind="ExternalInput").ap()
    go_b = nc.dram_tensor("go_b", [4, D], FP, kind="ExternalInput").ap()
    go_pool = nc.dram_tensor("go_pool", [T], FP, kind="ExternalInput").ap()
    f1_w = nc.dram_tensor("f1_w", [2 * D, 1024], FP, kind="ExternalInput").ap()
    f1_b = nc.dram_tensor("f1_b", [1024], FP, kind="ExternalInput").ap()
    f2_w = nc.dram_tensor("f2_w", [1024, 512], FP, kind="ExternalInput").ap()
    f2_b = nc.dram_tensor("f2_b", [512], FP, kind="ExternalInput").ap()
    f3_w = nc.dram_tensor("f3_w", [512, 1024], FP, kind="ExternalInput").ap()
    f3_b = nc.dram_tensor("f3_b", [1024], FP, kind="ExternalInput").ap()
    out_d = nc.dram_tensor("out", [BL, 1024], FP, kind="ExternalOutput").ap()

    def chunked(dram2d, nc_, cw=D):
        # [R, cw] dram viewed as [128, R//128, cw]
        return dram2d.rearrange("(c p) d -> p c d", p=128)

    with tile.TileContext(nc) as tc, ExitStack() as ctx:
        cpool = ctx.enter_context(tc.tile_pool(name="const", bufs=1))
        persist = ctx.enter_context(tc.tile_pool(name="persist", bufs=1))
        xb = ctx.enter_context(tc.tile_pool(name="xb", bufs=2))
        psA = ctx.enter_context(tc.tile_pool(name="psA", bufs=1, space="PSUM"))
        psB = ctx.enter_context(tc.tile_pool(name="psB", bufs=2, space="PSUM"))
        psC = ctx.enter_context(tc.tile_pool(name="psC", bufs=1, space="PSUM"))
        psD = ctx.enter_context(tc.tile_pool(name="psD", bufs=1, space="PSUM"))

        ident = cpool.tile([128, 128], FP, tag="ident")
        make_identity(nc, ident[:])
        identb = cpool.tile([128, 128], BF, tag="identb")
        nc.vector.tensor_copy(identb[:], ident[:])
        ones1 = cpool.tile([1, 128], FP, tag="ones1")
        nc.vector.memset(ones1[:], 1.0)
        ones1b = cpool.tile([1, 128], BF, tag="ones1b")
        nc.vector.memset(ones1b[:], 1.0)
        onesT = cpool.tile([T, 128], FP, tag="onesT")
        nc.vector.memset(onesT[:], 1.0)

        # ---- small bias vectors -> column layout via K=1 matmuls ---------
        def colvec_batch(specs):
            # pipelined: all row-loads first (3 rotating staging slots),
            # then K=1 matmuls into one psum tile, then copies out.
            pdvl = psC.tile([128, 64], FP, tag="pd")
            off = 0
            outs = []
            for idx, (src, n) in enumerate(specs):
                nt = n // 128
                vr = cpool.tile([1, 1024], FP, tag=f"vrow{idx % 2}")
                nc.sync.dma_start(vr[:, :n], src[:][None, :])
                for c in range(nt):
                    nc.tensor.matmul(pdvl[:, off + c:off + c + 1],
                                     vr[0:1, 128 * c:128 * (c + 1)],
                                     ones1[:1, :1], start=True, stop=True,
                                     skip_group_check=True)
                outs.append((off, nt))
                off += nt
            return pdvl, outs

        def colvec_out(pdvl, o_nt, tag):
            o, nt = o_nt
            t_ = cpool.tile([128, nt], FP, tag=tag)
            nc.vector.tensor_copy(t_[:], pdvl[:, o:o + nt])
            return t_

        pdv1, offs1 = colvec_batch([(ga_b[2], D), (ga_b[3], D),
                                    (go_b[2], D), (go_b[3], D),
                                    (la_b[0], D), (la_b[2], D),
                                    (la_b[3], D), (f1_b, 1024),
                                    (f2_b, 512), (f3_b, 1024)])
        b2gaT = colvec_out(pdv1, offs1[0], "b2gaT")
        b3gaT = colvec_out(pdv1, offs1[1], "b3gaT")
        b2goT = colvec_out(pdv1, offs1[2], "b2goT")
        b3goT = colvec_out(pdv1, offs1[3], "b3goT")
        b0laT = colvec_out(pdv1, offs1[4], "b0laT")
        b2laT = colvec_out(pdv1, offs1[5], "b2laT")
        b3laT = colvec_out(pdv1, offs1[6], "b3laT")
        b1fT = colvec_out(pdv1, offs1[7], "b1fT")
        b2fT = colvec_out(pdv1, offs1[8], "b2fT")
        b3fT = colvec_out(pdv1, offs1[9], "b3fT")
        b2laT_bf = cpool.tile([128, ET], BF, tag="b2laT_bf")
        nc.vector.tensor_copy(b2laT_bf[:], b2laT[:])

        lapool_c = cpool.tile([T, 1], FP, tag="lapool_c")
        nc.sync.dma_start(lapool_c[:], la_pool[:][:, None])
        gopool_c = cpool.tile([T, 1], FP, tag="gopool_c")
        nc.sync.dma_start(gopool_c[:], go_pool[:][:, None])
        gapool_c = cpool.tile([1, 1], FP, tag="gapool_c")
        nc.sync.dma_start(gapool_c[:], ga_pool[:][:, None])

        def sum_bcast(vcol, k, tag):
            p = psC.tile([128, 64], FP, tag="pd")
            lhs = onesT if k == T else ones1
            nc.tensor.matmul(p[:, 0:1], lhs[:k, :], vcol[:k, :], start=True,
                             stop=True, skip_group_check=True)
            s = cpool.tile([128, 1], FP, tag=tag)
            nc.vector.tensor_copy(s[:], p[:, 0:1])
            return s

        Sla = sum_bcast(lapool_c, T, "Sla")
        Sgo = sum_bcast(gopool_c, T, "Sgo")
        Sga = sum_bcast(gapool_c, 1, "Sga")

        pmask = cpool.tile([64, 2], FP, tag="pmask")
        nc.vector.memset(pmask[:], 0.0)
        nc.sync.dma_start(pmask[0:T, 0:1], la_pool[:][:, None])
        nc.sync.dma_start(pmask[T:2 * T, 1:2], la_pool[:][:, None])

        # img patch loads (streamed; b0/b1 prefetched early)
        KC = kchunks(NK)
        NKC = len(KC)
        XnMap = {}

        def load_Xn(b):
            Xn = xb.tile([128, NKC, D], BF, tag="Xn")
            if b < 2:
                # zero the pad rows once per buffer; later b's reuse the
                # buffer and only ever rewrite rows 0..kwl of the last chunk
                nc.vector.memset(Xn[:, NKC - 1, :], 0.0)
            nc.gpsimd.dma_start(
                Xn[:, 0:NKC - 1, :],
                img[b, 1:1 + 128 * (NKC - 1), :].rearrange(
                    "(c p) d -> p c d", p=128))
            k0l, kwl = KC[-1]
            nc.gpsimd.dma_start(Xn[:kwl, NKC - 1, :],
                                img[b, 1 + k0l:1 + k0l + kwl, :])
            XnMap[b] = Xn

        def pgroup(i, ncols=RQ):
            pl, tg = [(psC, "pd"), (psD, "gr"), (psD, "gz")][i % 3]
            pg_t = pl.tile([128, ncols], FP, tag=tg)
            return pg_t

        # persistent outputs of the phases
        qembT = cpool.tile([128, ET, BL, T], BF, tag="qembT")
        wxb = cpool.tile([BL, G], BF, tag="wxb")
        QtT = persist.tile([128, ET, NH * RQ], BF, tag="QtT")
        goutT = cpool.tile([128, ET, BL], BF, tag="goutT")
        aT = cpool.tile([128, ET, BL], GDT, tag="aT")
        pcxT2 = persist.tile([128, ET, NH, BL], BF, tag="pcxT2")

        # ================= phase B: GRU ===================================
        with tc.tile_pool(name="wbig", bufs=1) as wbig, \
             tc.tile_pool(name="wnat", bufs=2) as wnat, \
             tc.tile_pool(name="wst", bufs=3) as wst, \
             tc.tile_pool(name="g1", bufs=2) as g1:
            combr = wbig.tile([1, G], BF, tag="combr")
            nc.gpsimd.dma_start(combr[:], b_ih[:][None, :])
            bhhrow = wbig.tile([1, G], BF, tag="bhhrow")
            nc.gpsimd.dma_start(bhhrow[:], b_hh[:][None, :])
            nc.vector.tensor_add(combr[:, 0:2 * D], combr[:, 0:2 * D],
                                 bhhrow[:, 0:2 * D])
            bhhr_bf = bhhrow[:, 2 * D:3 * D]

            WT = wbig.tile([128, ET, G], GDT, tag="WT")
            tident = identb if GBF else ident

            def build_WT(w_dram):
                jts = kchunks(G)
                for g0 in range(0, len(jts), 5):
                    grp = jts[g0:g0 + 5]
                    ng = len(grp)
                    wn = wst.tile([128, 5, D], GDT, tag="wn")
                    src = w_dram[grp[0][0]:grp[-1][0] + grp[-1][1], :]
                    src = src.rearrange("(c p) d -> p c d", p=128)
                    if GBF:
                        nc.gpsimd.dma_start(wn[:, :ng, :], src)
                    else:
                        nc.sync.dma_start(wn[:, :ng, :], src)
                    sub = 5 if GBF else 3
                    for et in range(ET):
                        for s0 in range(0, ng, sub):
                            sg = min(sub, ng - s0)
                            pt = psB.tile([128, 128 * sub], GDT, tag="ptw")
                            for i in range(sg):
                                nc.tensor.matmul(pt[:, 128 * i:128 * (i + 1)],
                                                 wn[:, s0 + i, 128 * et:128 * (et + 1)],
                                                 tident[:], is_transpose=True,
                                                 skip_group_check=True)
                            w0 = grp[0][0] + 128 * s0
                            wlen = 128 * sg
                            if (et + s0) % 2 == 0:
                                nc.vector.tensor_copy(_r(WT[:, et, w0:w0 + wlen]),
                                                      pt[:, :wlen])
                            else:
                                nc.scalar.copy(_r(WT[:, et, w0:w0 + wlen]),
                                               pt[:, :wlen])

            build_WT(w_ih)

            # ---- phase A part 1 (cls -> a), interleaved after W_ih ------
            clsn = wbig.tile([BL, D], BF, tag="clsn")
            nc.gpsimd.dma_start(clsn[:], img[0:BL, 0, :])
            wA2 = wnat.tile([128, ET, D], BF, tag="wa")
            nc.gpsimd.dma_start(wA2[:], chunked(ga_w[2], nc))
            wA3 = wnat.tile([128, ET, D], BF, tag="wa")
            nc.gpsimd.dma_start(wA3[:], chunked(ga_w[3], nc))
            ptr = psB.tile([128, 512], BF, tag="ptw")
            for kt in range(ET):
                nc.tensor.matmul(ptr[:, 8 * kt:8 * kt + 8],
                                 clsn[:, 128 * kt:128 * (kt + 1)],
                                 identb[:BL, :BL], is_transpose=True,
                                 skip_group_check=True)
            clsT = wbig.tile([128, ET, BL], BF, tag="clsT")
            nc.vector.tensor_copy(clsT[:].rearrange("p a b -> p (a b)"),
                                  ptr[:, :8 * ET])

            def dense_T(wsb, rhsT, biasT, scaleT, otile, out_r=False):
                for mt in range(ET):
                    p = psC.tile([128, BL], FP, tag="pd")
                    for kt in range(ET):
                        nc.tensor.matmul(p[:], wsb[:, kt, 128 * mt:128 * (mt + 1)],
                                         rhsT[:, kt, :], start=(kt == 0),
                                         stop=(kt == ET - 1))
                    dst = otile[:, mt, :]
                    if out_r:
                        dst = _r(dst)
                    if scaleT is None:
                        nc.vector.tensor_scalar(dst, p[:], biasT[:, mt:mt + 1],
                                                None, OP.add)
                    else:
                        nc.vector.tensor_scalar(dst, p[:], biasT[:, mt:mt + 1],
                                                scaleT[:, 0:1], OP.add, OP.mult)

            A2T = wbig.tile([128, ET, BL], BF, tag="A2T")
            dense_T(wA2, clsT, b2gaT, None, A2T)
            dense_T(wA3, A2T, b3gaT, Sga, aT, out_r=not GBF)

            for (j0, jw) in CH_G:
                p = psA.tile([BL, 512], FP, tag="wh0")
                for kt in range(ET):
                    nc.tensor.matmul(p[:, :jw], aT[:, kt, :] if GBF else _r(aT[:, kt, :]),
                                     _r(WT[:, kt, j0:j0 + jw]),
                                     start=(kt == 0), stop=False)
                nc.tensor.matmul(p[:, :jw], ones1b[:1, :BL],
                                 combr[:, j0:j0 + jw], start=False, stop=True)
                nc.vector.tensor_copy(wxb[:, j0:j0 + jw], p[:, :jw])

            build_WT(w_hh)

            # ---- phase A part 2 (gout path, off critical path) ----------
            wG2 = wnat.tile([128, ET, D], BF, tag="wa")
            nc.gpsimd.dma_start(wG2[:], chunked(go_w[2], nc))
            wG3 = wnat.tile([128, ET, D], BF, tag="wa")
            nc.gpsimd.dma_start(wG3[:], chunked(go_w[3], nc))
            G2T = wbig.tile([128, ET, BL], BF, tag="G2T")
            dense_T(wG2, clsT, b2goT, None, G2T)
            dense_T(wG3, G2T, b3goT, Sgo, goutT)

            # transposed constant wx for the n-gate: [128, ET, BL]
            ptx = psC.tile([128, 64], BF, tag="pd")
            for kt in range(ET):
                nc.tensor.matmul(ptx[:, 8 * kt:8 * kt + 8],
                                 wxb[:, 2 * D + 128 * kt:2 * D + 128 * (kt + 1)],
                                 identb[:BL, :BL], is_transpose=True,
                                 skip_group_check=True)
            wxTn = wbig.tile([128, ET, BL], FP, tag="wxTn")
            nc.vector.tensor_copy(wxTn[:].rearrange("p a b -> p (a b)"),
                                  ptx[:, :8 * ET])

            # initial h0 transposed
            hnat0 = wbig.tile([BL, D], BF, tag="hnat0")
            nc.gpsimd.dma_start(hnat0[:], h0[:, :])
            ptr0 = psC.tile([128, 64], BF, tag="pd")
            for kt in range(ET):
                nc.tensor.matmul(ptr0[:, 8 * kt:8 * kt + 8],
                                 hnat0[:, 128 * kt:128 * (kt + 1)],
                                 identb[:BL, :BL], is_transpose=True,
                                 skip_group_check=True)
            hT = wbig.tile([128, ET, BL], GDT, tag="h0T")
            nc.vector.tensor_copy(_r(hT[:].rearrange("p a b -> p (a b)")),
                                  ptr0[:, :8 * ET])

            # prefetch DMAs for phases C/D/E: deprioritized so they only
            # fill DMA slots the W/A loads are not using
            with tc.high_priority(offset=-100000):
                W0 = persist.tile([128, ET, D], BF, tag="W0")
                nc.gpsimd.dma_start(W0[:], chunked(la_w[0], nc))
                W1 = persist.tile([128, ET, D], BF, tag="W1")
                nc.gpsimd.dma_start(W1[:], chunked(la_w[1], nc))
                W2 = persist.tile([128, ET, D], BF, tag="W2")
                nc.gpsimd.dma_start(W2[:], chunked(la_w[2], nc))
                W3 = persist.tile([128, ET, D], BF, tag="W3")
                nc.gpsimd.dma_start(W3[:], chunked(la_w[3], nc))
                f1 = persist.tile([128, 12, 1024], BF, tag="f1")
                nc.gpsimd.dma_start(f1[:], f1_w.rearrange("(c p) n -> p c n", p=128))
                f2 = persist.tile([128, 8, 512], BF, tag="f2")
                nc.gpsimd.dma_start(f2[:], f2_w.rearrange("(c p) n -> p c n", p=128))
                f3 = persist.tile([128, 4, 1024], BF, tag="f3")
                nc.gpsimd.dma_start(f3[:], f3_w.rearrange("(c p) n -> p c n", p=128))
                load_Xn(0)

            KSTEPS = int(os.environ.get("KSTEPS", str(T)))
            KHALF = os.environ.get("KHALF", "1") == "1"
            HB = BL // 2
            wxTn3 = wxTn[:]
            if not KHALF:
                for t in range(KSTEPS):
                    psR = psD.tile([128, ET * BL], FP, tag="gr")
                    psZ = psD.tile([128, ET * BL], FP, tag="gz")
                    psN = psD.tile([128, ET * BL], FP, tag="gn")

                    def gate_chunk(ps, mi, m):
                        j0 = 128 * m
                        for kt in range(ET):
                            nc.tensor.matmul(ps[:, BL * mi:BL * (mi + 1)],
                                             _r(WT[:, kt, j0:j0 + 128]),
                                             _r(hT[:, kt, :]),
                                             start=(kt == 0), stop=False,
                                             skip_group_check=True)
                        if m < 12:
                            nc.tensor.matmul(ps[:, BL * mi:BL * (mi + 1)],
                                             wxb[:, j0:j0 + 128],
                                             identb[:BL, :BL], start=False,
                                             stop=True, skip_group_check=True)
                        else:
                            nc.tensor.matmul(ps[:, BL * mi:BL * (mi + 1)],
                                             bhhr_bf[:, j0 - 2 * D:j0 - 2 * D + 128],
                                             ones1b[:1, :BL],
                                             start=False, stop=True,
                                             skip_group_check=True)

                    for mi in range(ET):
                        gate_chunk(psR, mi, mi)
                    for mi in range(ET):
                        gate_chunk(psN, mi, 12 + mi)
                    for mi in range(ET):
                        gate_chunk(psZ, mi, 6 + mi)

                    # h_new = (1-z)*n + z*h ; z-products run in tanh's shadow
                    rsig = g1.tile([128, ET * BL], FP, tag="rsig")
                    nc.scalar.activation(rsig[:], psR[:], AF.Sigmoid)
                    zsig = g1.tile([128, ET * BL], FP, tag="zsig")
                    nc.scalar.activation(zsig[:], psZ[:], AF.Sigmoid)
                    rwn = g1.tile([128, ET * BL], FP, tag="rwn")
                    nc.vector.tensor_mul(rwn[:], rsig[:], psN[:])
                    npre = g1.tile([128, ET * BL], FP, tag="npre")
                    nc.vector.tensor_add(npre[:], rwn[:],
                                         wxTn[:].rearrange("p a b -> p (a b)"))
                    nt_ = g1.tile([128, ET * BL], FP, tag="nt")
                    nc.scalar.activation(nt_[:], npre[:], AF.Tanh)
                    zh = g1.tile([128, ET * BL], FP, tag="zh")
                    nc.vector.tensor_mul(zh[:], zsig[:],
                                         hT[:].rearrange("p a b -> p (a b)"))
                    omz = g1.tile([128, ET * BL], FP, tag="omz")
                    nc.vector.tensor_scalar(omz[:], zsig[:], -1.0, 1.0,
                                            OP.mult, OP.add)
                    ozn = g1.tile([128, ET * BL], FP, tag="ozn")
                    nc.vector.tensor_mul(ozn[:], omz[:], nt_[:])
                    hT = g1.tile([128, ET, BL], GDT, tag="hT")
                    nc.vector.tensor_add(_r(hT[:].rearrange("p a b -> p (a b)")),
                                         ozn[:], zh[:])
                    nc.scalar.copy(qembT[:, :, :, t].rearrange("p a b -> p (a b)"),
                                   hT[:].rearrange("p a b -> p (a b)"))
            else:
                # two independent half-batch chains, interleaved so each
                # half's elementwise hides in the other's latency
                hTs = [None, None]
                psmap = [(psD, "gr"), (psD, "gz"), (psA, "wh0"), (psA, "wh1"),
                         (psD, "gn"), (psC, "pd")]
                for t in range(KSTEPS):
                    def hprev(g, kt):
                        if t == 0:
                            return _r(hT[:, kt, HB * g:HB * (g + 1)])
                        return _r(hTs[g][:, kt, :])

                    def ps_half(i):
                        pl, tg = psmap[i]
                        ph_t = pl.tile([128, ET * HB], FP, tag=tg)
                        return ph_t

                    psRs = [ps_half(0), ps_half(1)]
                    psZs = [ps_half(2), ps_half(3)]
                    psNs = [ps_half(4), ps_half(5)]

                    def gate_chunk2(ps, mi, m, g):
                        j0 = 128 * m
                        for kt in range(ET):
                            nc.tensor.matmul(ps[:, HB * mi:HB * (mi + 1)],
                                             _r(WT[:, kt, j0:j0 + 128]),
                                             hprev(g, kt),
                                             start=(kt == 0), stop=False,
                                             skip_group_check=True)
                        if m < 12:
                            nc.tensor.matmul(ps[:, HB * mi:HB * (mi + 1)],
                                             wxb[:, j0:j0 + 128],
                                             identb[:BL, HB * g:HB * (g + 1)],
                                             start=False, stop=True,
                                             skip_group_check=True)
                        else:
                            nc.tensor.matmul(ps[:, HB * mi:HB * (mi + 1)],
                                             bhhr_bf[:, j0 - 2 * D:j0 - 2 * D + 128],
                                             ones1b[:1, :HB],
                                             start=False, stop=True,
                                             skip_group_check=True)

                    for g in (0, 1):
                        for mi in range(ET):
                            gate_chunk2(psRs[g], mi, mi, g)
                    for g in (0, 1):
                        for mi in range(ET):
                            gate_chunk2(psNs[g], mi, 12 + mi, g)
                    for g in (0, 1):
                        for mi in range(ET):
                            gate_chunk2(psZs[g], mi, 6 + mi, g)

                    def tile3(tag):
                        t3 = g1.tile([128, ET, HB], FP, tag=tag)
                        return t3

                    rsig = [tile3("rsig0"), tile3("rsig1")]
                    zsig = [tile3("zsig0"), tile3("zsig1")]
                    rwn = [tile3("rwn0"), tile3("rwn1")]
                    npre = [tile3("npre0"), tile3("npre1")]
                    nt_ = [tile3("nt0"), tile3("nt1")]
                    zh = [tile3("zh0"), tile3("zh1")]
                    omz = [tile3("omz0"), tile3("omz1")]
                    ozn = [tile3("ozn0"), tile3("ozn1")]
                    def tile3g(tag):
                        t3g = g1.tile([128, ET, HB], GDT, tag=tag)
                        return t3g

                    hnew = [tile3g("hTn0"), tile3g("hTn1")]
                    for g in (0, 1):
                        nc.scalar.activation(rsig[g][:].rearrange("p a b -> p (a b)"),
                                             psRs[g][:], AF.Sigmoid)
                    for g in (0, 1):
                        nc.scalar.activation(zsig[g][:].rearrange("p a b -> p (a b)"),
                                             psZs[g][:], AF.Sigmoid)
                    for g in (0, 1):
                        nc.vector.tensor_mul(rwn[g][:].rearrange("p a b -> p (a b)"),
                                             rsig[g][:].rearrange("p a b -> p (a b)"),
                                             psNs[g][:])
                    for g in (0, 1):
                        nc.vector.tensor_add(npre[g][:], rwn[g][:],
                                             wxTn3[:, :, HB * g:HB * (g + 1)])
                    for g in (0, 1):
                        nc.scalar.activation(nt_[g][:].rearrange("p a b -> p (a b)"),
                                             npre[g][:].rearrange("p a b -> p (a b)"),
                                             AF.Tanh)
                    for g in (0, 1):
                        hp = (hT[:, :, HB * g:HB * (g + 1)] if t == 0
                              else hTs[g][:])
                        nc.vector.tensor_mul(zh[g][:], zsig[g][:], hp)
                    for g in (0, 1):
                        nc.vector.tensor_scalar(omz[g][:].rearrange("p a b -> p (a b)"),
                                                zsig[g][:].rearrange("p a b -> p (a b)"),
                                                -1.0, 1.0, OP.mult, OP.add)
                    for g in (0, 1):
                        nc.vector.tensor_mul(ozn[g][:], omz[g][:], nt_[g][:])
                    for g in (0, 1):
                        nc.vector.tensor_add(_r(hnew[g][:]), ozn[g][:], zh[g][:])
                    for g in (0, 1):
                        nc.scalar.copy(qembT[:, :, HB * g:HB * (g + 1), t],
                                       hnew[g][:])
                    hTs = hnew
            load_Xn(1)

        # ================= phase C: Q^T, W1^T, Qt^T =======================
        with tc.tile_pool(name="prep", bufs=1) as prep:
            QT = prep.tile([128, ET, RQ], BF, tag="QT")
            qflat = qembT[:].rearrange("p a b t -> p a (b t)")
            for mt in range(ET):
                p = pgroup(mt)
                for kt in range(ET):
                    nc.tensor.matmul(p[:], W0[:, kt, 128 * mt:128 * (mt + 1)],
                                     qflat[:, kt, :], start=(kt == 0),
                                     stop=(kt == ET - 1))
                nc.vector.tensor_scalar(QT[:, mt, :], p[:], b0laT[:, mt:mt + 1],
                                        None, OP.add)
            W1T = prep.tile([128, ET, D], BF, tag="W1T")
            for hd in range(ET):
                for grp in range(2):
                    pt2 = psB.tile([128, 512], BF, tag="ptw")
                    for i in range(3):
                        e2 = grp * 3 + i
                        nc.tensor.matmul(pt2[:, 128 * i:128 * (i + 1)],
                                         W1[:, e2, 128 * hd:128 * (hd + 1)],
                                         identb[:], is_transpose=True,
                                         skip_group_check=True)
                    if grp == 0:
                        nc.vector.tensor_copy(W1T[:, hd, 0:384], pt2[:, 0:384])
                    else:
                        nc.scalar.copy(W1T[:, hd, 384:768], pt2[:, 0:384])
            scl = 1.0 / float(np.sqrt(DK))
            for h in range(NH):
                for mt in range(ET):
                    p = pgroup(h * ET + mt)
                    for i in range(3):
                        kt = h * 3 + i
                        nc.tensor.matmul(p[:], W1T[:, kt, 128 * mt:128 * (mt + 1)],
                                         QT[:, kt, :], start=(i == 0), stop=(i == 2))
                    dst = QtT[:, mt, :].rearrange("p (b h2 t) -> p b h2 t",
                                                  h2=NH, t=T)[:, :, h, :]
                    if (h * ET + mt) % 3 != 2:
                        nc.vector.tensor_scalar(dst, p[:], scl, None, OP.mult)
                    else:
                        nc.scalar.activation(dst, p[:], AF.Copy, scale=scl)

        # ================= phase D: per-b attention =======================
        with tc.tile_pool(name="ab", bufs=2) as ab:
            for b in range(BL):
                if b + 1 < BL and b + 1 not in XnMap:
                    load_Xn(b + 1)
                Xn = XnMap.pop(b)
                XT = ab.tile([128, ET, NKC * 128], BF, tag="XT")
                for et in range(ET):
                    pt = psB.tile([128, 1024], BF, tag="ptw")
                    for c in range(NKC):
                        nc.tensor.matmul(pt[:, 128 * c:128 * (c + 1)],
                                         Xn[:, c, 128 * et:128 * (et + 1)],
                                         identb[:], is_transpose=True,
                                         skip_group_check=True)
                    if et % 3 != 2:
                        nc.vector.tensor_copy(XT[:, et, :], pt[:])
                    else:
                        nc.scalar.copy(XT[:, et, :], pt[:])
                att = ab.tile([64, NKC * 128], BF, tag="att")
                nc.vector.memset(att[:, NK:], 0.0)
                zacc = ab.tile([64, 2], FP, tag="zacc")
                for ci, (n0, nw) in enumerate(CH_NK):
                    p = psA.tile([64, 512], FP, tag=f"wh{ci}")
                    for kt in range(ET):
                        nc.tensor.matmul(p[:, :nw],
                                         QtT[:, kt, b * 2 * T:(b + 1) * 2 * T],
                                         XT[:, kt, n0:n0 + nw],
                                         start=(kt == 0), stop=(kt == ET - 1))
                    nc.scalar.activation(att[:, n0:n0 + nw], p[:, :nw], AF.Exp,
                                         accum_out=zacc[:, ci:ci + 1])
                zs = ab.tile([64, 1], FP, tag="zs")
                nc.vector.tensor_add(zs[:], zacc[:, 0:1], zacc[:, 1:2])
                rz = ab.tile([64, 1], FP, tag="rz1")
                nc.vector.reciprocal(rz[:], zs[:])
                wm = ab.tile([64, 2], BF, tag="wm")
                nc.vector.tensor_scalar(wm[:], pmask[:], rz[:, 0:1], None, OP.mult)
                # paT[k, i] = sum_q att[q, k] * wm[q, i]  (no transposes!)
                pp = psD.tile([128, 2 * NKC], FP, tag="gr")
                for c in range(NKC):
                    nc.tensor.matmul(pp[:, 2 * c:2 * c + 2],
                                     att[:, 128 * c:128 * (c + 1)], wm[:],
                                     start=True, stop=True,
                                     skip_group_check=True)
                paT = ab.tile([128, NKC, 2], BF, tag="paT")
                nc.vector.tensor_copy(paT[:].rearrange("p a b -> p (a b)"), pp[:])
                # ctxT[d, i] = sum_k Xn[k, d] * paT[k, i]
                pc = psD.tile([128, 2 * ET], FP, tag="gz")
                for dc in range(ET):
                    for c in range(NKC):
                        nc.tensor.matmul(pc[:, 2 * dc:2 * dc + 2],
                                         Xn[:, c, 128 * dc:128 * (dc + 1)],
                                         paT[:, c, :], start=(c == 0),
                                         stop=(c == NKC - 1),
                                         skip_group_check=True)
                nc.vector.tensor_copy(
                    pcxT2[:, :, :, b].rearrange("p a b -> p (a b)"), pc[:])

        # ================= phase E: projections + MLP =====================
        with tc.tile_pool(name="tail", bufs=1) as tail:
            vconT = tail.tile([128, ET], FP, tag="vconT")
            for mt in range(ET):
                p = pgroup(mt)
                for kt in range(ET):
                    nc.tensor.matmul(p[:, 0:1], W3[:, kt, 128 * mt:128 * (mt + 1)],
                                     b2laT_bf[:, kt:kt + 1], start=(kt == 0),
                                     stop=(kt == ET - 1), skip_group_check=True)
                nc.vector.tensor_scalar(vconT[:, mt:mt + 1], p[:, 0:1],
                                        b3laT[:, mt:mt + 1], Sla[:, 0:1],
                                        OP.add, OP.mult)
            pctxT = tail.tile([128, ET, BL], BF, tag="pctxT")
            for h in range(NH):
                for mi in range(3):
                    mt = h * 3 + mi
                    p = pgroup(mt)
                    for kt in range(ET):
                        nc.tensor.matmul(p[:, 0:BL],
                                         W2[:, kt, 128 * mt:128 * (mt + 1)],
                                         pcxT2[:, kt, h, :], start=(kt == 0),
                                         stop=(kt == ET - 1),
                                         skip_group_check=True)
                    nc.vector.tensor_copy(pctxT[:, mt, :], p[:, 0:BL])
            loT = tail.tile([128, ET, BL], BF, tag="loT")
            for mt in range(ET):
                p = pgroup(mt)
                for kt in range(ET):
                    nc.tensor.matmul(p[:, 0:BL], W3[:, kt, 128 * mt:128 * (mt + 1)],
                                     pctxT[:, kt, :], start=(kt == 0),
                                     stop=(kt == ET - 1), skip_group_check=True)
                nc.vector.tensor_scalar(loT[:, mt, :], p[:, 0:BL],
                                        vconT[:, mt:mt + 1], None, OP.add)

            y1T = tail.tile([128, 8, BL], BF, tag="y1T")
            for mt in range(8):
                p = pgroup(mt)
                for kt in range(12):
                    r_ = loT[:, kt, :] if kt < ET else goutT[:, kt - ET, :]
                    nc.tensor.matmul(p[:, 0:BL], f1[:, kt, 128 * mt:128 * (mt + 1)],
                                     r_, start=(kt == 0), stop=(kt == 11),
                                     skip_group_check=True)
                nc.vector.tensor_scalar(y1T[:, mt, :], p[:, 0:BL],
                                        b1fT[:, mt:mt + 1], None, OP.add)
            y2T = tail.tile([128, 4, BL], BF, tag="y2T")
            for mt in range(4):
                p = pgroup(mt)
                for kt in range(8):
                    nc.tensor.matmul(p[:, 0:BL], f2[:, kt, 128 * mt:128 * (mt + 1)],
                                     y1T[:, kt, :], start=(kt == 0), stop=(kt == 7),
                                     skip_group_check=True)
                nc.scalar.activation(y2T[:, mt, :], p[:, 0:BL], AF.Relu,
                                     bias=b2fT[:, mt:mt + 1])
            yT = tail.tile([128, 8, BL], FP, tag="yT")
            for mt in range(8):
                p = pgroup(mt)
                for kt in range(4):
                    nc.tensor.matmul(p[:, 0:BL], f3[:, kt, 128 * mt:128 * (mt + 1)],
                                     y2T[:, kt, :], start=(kt == 0), stop=(kt == 3),
                                     skip_group_check=True)
                nc.vector.tensor_scalar(yT[:, mt, :], p[:, 0:BL],
                                        b3fT[:, mt:mt + 1], None, OP.add)
            ynat = tail.tile([BL, 1024], FP, tag="ynat")
            for g in range(2):
                po = psB.tile([128, 512], FP, tag="ptw")
                for i in range(4):
                    mt = g * 4 + i
                    nc.tensor.matmul(po[:BL, 128 * i:128 * (i + 1)], yT[:, mt, :],
                                     ident[:128, :128], is_transpose=True,
                                     skip_group_check=True)
                nc.vector.tensor_copy(ynat[:, 512 * g:512 * (g + 1)], po[:BL, :])
            nc.sync.dma_start(out_d[:, :], ynat[:])

    nc.compile()
    return nc


_NC = None


def kernel(**inputs):
    global _NC
    if _NC is None:
        _NC = build()
    B = inputs["image_local_embeds"].shape[0]
    per = B // NCORES
    in_maps = []
    for c in range(NCORES):
        sl = slice(c * per, (c + 1) * per)
        m = {
            "img": np.ascontiguousarray(np.asarray(inputs["image_local_embeds"])[sl], dtype=np.float32),
            "h0": np.ascontiguousarray(np.asarray(inputs["h0"])[sl], dtype=np.float32),
        }
        for k in ["gru_w_ih", "gru_w_hh", "gru_b_ih", "gru_b_hh", "ga_w", "ga_b",
                  "ga_pool", "la_w", "la_b", "la_pool", "go_w", "go_b", "go_pool",
                  "f1_w", "f1_b", "f2_w", "f2_b", "f3_w", "f3_b"]:
            m[k] = np.ascontiguousarray(np.asarray(inputs[k], dtype=np.float32))
        in_maps.append(m)
    res = run_bass_kernel_spmd(_NC, in_maps, core_ids=list(range(NCORES)))
    return np.concatenate([res.results[c]["out"] for c in range(NCORES)], axis=0)


# revision 65
# speedup vs baseline: 1.0666x; 1.0666x over previous
"""Trainium2 Bass kernel for nn_BiVision_VQA2 (B=64,T=32,D=768,N=901).

Data-parallel over batch: 8 batch elems per core x 8 cores.
Key math simplifications (validated vs reference, rel err ~1e-4):
  - ga/go attention use a single key token -> softmax==1 -> those paths are
    linear in cls; question_embeds is mathematically unused.
  - GRU input `a` is constant over time; wx computed once.
  - local attention: row-constant score terms drop out of softmax; query
    pooling applied to the attention matrix before the @X contraction.
Performance structure:
  - GRU computed in TRANSPOSED gate layout [128(gate row), batch] via
    weight-stationary matmuls (moving N=8), elementwise on [128, 48].
  - bf16 everywhere outside the GRU recurrence (DMA casts on load).
  - phase D: transpose-free paT/ctxT via natural-operand-stationary matmuls.
"""

import os
import numpy as np
from contextlib import ExitStack

import concourse.bass as bass
import concourse.tile as tile
from concourse import bacc, mybir
from concourse.bass_utils import run_bass_kernel_spmd
from concourse.masks import make_identity

FP = mybir.dt.float32
FPR = mybir.dt.float32r
OP = mybir.AluOpType
AF = mybir.ActivationFunctionType
BF = mybir.dt.bfloat16

NCORES = 8
BL = 8
D = 768
T = 32
G = 3 * D
NK = 900
NH = 2
DK = 384
ET = D // 128
RQ = BL * T

CH_G = [(0, 512), (512, 512), (1024, 512), (1536, 512), (2048, 256)]
CH_NK = [(0, 512), (512, 388)]

GBF = os.environ.get("KGRUBF", "1") == "1"
GDT = BF if GBF else FP


def _r(ap):
    return ap if GBF else ap.bitcast(FPR)


def kchunks(n):
    out, o = [], 0
    while o < n:
        out.append((o, min(128, n - o)))
        o += 128
    return out


def build():
    nc = bacc.Bacc("TRN2", target_bir_lowering=False, debug=False,
                   enable_asserts=False)

    img = nc.dram_tensor("img", [BL, 901, D], FP, kind="ExternalInput").ap()
    h0 = nc.dram_tensor("h0", [BL, D], FP, kind="ExternalInput").ap()
    w_ih = nc.dram_tensor("gru_w_ih", [G, D], FP, kind="ExternalInput").ap()
    w_hh = nc.dram_tensor("gru_w_hh", [G, D], FP, kind="ExternalInput").ap()
    b_ih = nc.dram_tensor("gru_b_ih", [G], FP, kind="ExternalInput").ap()
    b_hh = nc.dram_tensor("gru_b_hh", [G], FP, kind="ExternalInput").ap()
    ga_w = nc.dram_tensor("ga_w", [4, D, D], FP, kind="ExternalInput").ap()
    ga_b = nc.dram_tensor("ga_b", [4, D], FP, kind="ExternalInput").ap()
    ga_pool = nc.dram_tensor("ga_pool", [1], FP, kind="ExternalInput").ap()
    la_w = nc.dram_tensor("la_w", [4, D, D], FP, kind="ExternalInput").ap()
    la_b = nc.dram_tensor("la_b", [4, D], FP, kind="ExternalInput").ap()
    la_pool = nc.dram_tensor("la_pool", [T], FP, kind="ExternalInput").ap()
    go_w = nc.dram_tensor("go_w", [4, D, D], FP, kind="ExternalInput").ap()
    go_b = nc.dram_tensor("go_b", [4, D], FP, kind="ExternalInput").ap()
    go_pool = nc.dram_tensor("go_pool", [T], FP, kind="ExternalInput").ap()
    f1_w = nc.dram_tensor("f1_w", [2 * D, 1024], FP, kind="ExternalInput").ap()
    f1_b = nc.dram_tensor("f1_b", [1024], FP, kind="ExternalInput").ap()
    f2_w = nc.dram_tensor("f2_w", [1024, 512], FP, kind="ExternalInput").ap()
    f2_b = nc.dram_tensor("f2_b", [512], FP, kind="ExternalInput").ap()
    f3_w = nc.dram_tensor("f3_w", [512, 1024], FP, kind="ExternalInput").ap()
    f3_b = nc.dram_tensor("f3_b", [1024], FP, kind="ExternalInput").ap()
    out_d = nc.dram_tensor("out", [BL, 1024], FP, kind="ExternalOutput").ap()

    def chunked(dram2d, nc_, cw=D):
        # [R, cw] dram viewed as [128, R//128, cw]
        return dram2d.rearrange("(c p) d -> p c d", p=128)

    with tile.TileContext(nc) as tc, ExitStack() as ctx:
        cpool = ctx.enter_context(tc.tile_pool(name="const", bufs=1))
        persist = ctx.enter_context(tc.tile_pool(name="persist", bufs=1))
        xb = ctx.enter_context(tc.tile_pool(name="xb", bufs=2))
        psA = ctx.enter_context(tc.tile_pool(name="psA", bufs=1, space="PSUM"))
        psB = ctx.enter_context(tc.tile_pool(name="psB", bufs=2, space="PSUM"))
        psC = ctx.enter_context(tc.tile_pool(name="psC", bufs=1, space="PSUM"))
        psD = ctx.enter_context(tc.tile_pool(name="psD", bufs=1, space="PSUM"))

        ident = cpool.tile([128, 128], FP, tag="ident")
        make_identity(nc, ident[:])
        identb = cpool.tile([128, 128], BF, tag="identb")
        nc.vector.tensor_copy(identb[:], ident[:])
        ones1 = cpool.tile([1, 128], FP, tag="ones1")
        nc.vector.memset(ones1[:], 1.0)
        ones1b = cpool.tile([1, 128], BF, tag="ones1b")
        nc.vector.memset(ones1b[:], 1.0)
        onesT = cpool.tile([T, 128], FP, tag="onesT")
        nc.vector.memset(onesT[:], 1.0)

        # ---- small bias vectors -> column layout via K=1 matmuls ---------
        def colvec_batch(specs):
            # pipelined: all row-loads first (3 rotating staging slots),
            # then K=1 matmuls into one psum tile, then copies out.
            pdvl = psC.tile([128, 64], FP, tag="pd")
            off = 0
            outs = []
            for idx, (src, n) in enumerate(specs):
                nt = n // 128
                vr = cpool.tile([1, 1024], FP, tag=f"vrow{idx % 2}")
                nc.sync.dma_start(vr[:, :n], src[:][None, :])
                for c in range(nt):
                    nc.tensor.matmul(pdvl[:, off + c:off + c + 1],
                                     vr[0:1, 128 * c:128 * (c + 1)],
                                     ones1[:1, :1], start=True, stop=True,
                                     skip_group_check=True)
                outs.append((off, nt))
                off += nt
            return pdvl, outs

        def colvec_out(pdvl, o_nt, tag):
            o, nt = o_nt
            t_ = cpool.tile([128, nt], FP, tag=tag)
            nc.vector.tensor_copy(t_[:], pdvl[:, o:o + nt])
            return t_

        pdv1, offs1 = colvec_batch([(ga_b[2], D), (ga_b[3], D),
                                    (go_b[2], D), (go_b[3], D),
                                    (la_b[0], D), (la_b[2], D),
                                    (la_b[3], D), (f1_b, 1024),
                                    (f2_b, 512), (f3_b, 1024)])
        b2gaT = colvec_out(pdv1, offs1[0], "b2gaT")
        b3gaT = colvec_out(pdv1, offs1[1], "b3gaT")
        b2goT = colvec_out(pdv1, offs1[2], "b2goT")
        b3goT = colvec_out(pdv1, offs1[3], "b3goT")
        b0laT = colvec_out(pdv1, offs1[4], "b0laT")
        b2laT = colvec_out(pdv1, offs1[5], "b2laT")
        b3laT = colvec_out(pdv1, offs1[6], "b3laT")
        b1fT = colvec_out(pdv1, offs1[7], "b1fT")
        b2fT = colvec_out(pdv1, offs1[8], "b2fT")
        b3fT = colvec_out(pdv1, offs1[9], "b3fT")
        b2laT_bf = cpool.tile([128, ET], BF, tag="b2laT_bf")
        nc.vector.tensor_copy(b2laT_bf[:], b2laT[:])

        lapool_c = cpool.tile([T, 1], FP, tag="lapool_c")
        nc.sync.dma_start(lapool_c[:], la_pool[:][:, None])
        gopool_c = cpool.tile([T, 1], FP, tag="gopool_c")
        nc.sync.dma_start(gopool_c[:], go_pool[:][:, None])
        gapool_c = cpool.tile([1, 1], FP, tag="gapool_c")
        nc.sync.dma_start(gapool_c[:], ga_pool[:][:, None])

        def sum_bcast(vcol, k, tag):
            p = psC.tile([128, 64], FP, tag="pd")
            lhs = onesT if k == T else ones1
            nc.tensor.matmul(p[:, 0:1], lhs[:k, :], vcol[:k, :], start=True,
                             stop=True, skip_group_check=True)
            s = cpool.tile([128, 1], FP, tag=tag)
            nc.vector.tensor_copy(s[:], p[:, 0:1])
            return s

        Sla = sum_bcast(lapool_c, T, "Sla")
        Sgo = sum_bcast(gopool_c, T, "Sgo")
        Sga = sum_bcast(gapool_c, 1, "Sga")

        pmask = cpool.tile([64, 2], FP, tag="pmask")
        nc.vector.memset(pmask[:], 0.0)
        nc.sync.dma_start(pmask[0:T, 0:1], la_pool[:][:, None])
        nc.sync.dma_start(pmask[T:2 * T, 1:2], la_pool[:][:, None])

        # img patch loads (streamed; b0/b1 prefetched early)
        KC = kchunks(NK)
        NKC = len(KC)
        XnMap = {}

        xb2 = [None]

        def load_Xn(b):
            # 3-way buffer rotation: xb holds 2, xb2 (opened for phase D,
            # reusing SBUF freed by the GRU pools) holds the third
            pool = xb2[0] if (b % 3 == 2 and xb2[0] is not None) else xb
            Xn = pool.tile([128, NKC, D], BF, tag="Xn")
            if b < 3:
                # zero the pad rows once per physical buffer (b0,b1 -> xb's
                # two buffers, b2 -> xb2); later b's reuse a buffer and only
                # ever rewrite rows 0..kwl of the last chunk
                nc.vector.memset(Xn[:, NKC - 1, :], 0.0)
            nc.gpsimd.dma_start(
                Xn[:, 0:NKC - 1, :],
                img[b, 1:1 + 128 * (NKC - 1), :].rearrange(
                    "(c p) d -> p c d", p=128))
            k0l, kwl = KC[-1]
            nc.gpsimd.dma_start(Xn[:kwl, NKC - 1, :],
                                img[b, 1 + k0l:1 + k0l + kwl, :])
            XnMap[b] = Xn

        def pgroup(i, ncols=RQ):
            pl, tg = [(psC, "pd"), (psD, "gr"), (psD, "gz")][i % 3]
            pg_t = pl.tile([128, ncols], FP, tag=tg)
            return pg_t

        # persistent outputs of the phases
        qembT = cpool.tile([128, ET, BL, T], BF, tag="qembT")
        wxb = cpool.tile([BL, G], BF, tag="wxb")
        QtT = persist.tile([128, ET, NH * RQ], BF, tag="QtT")
        goutT = cpool.tile([128, ET, BL], BF, tag="goutT")
        aT = cpool.tile([128, ET, BL], GDT, tag="aT")
        pcxT2 = persist.tile([128, ET, NH, BL], BF, tag="pcxT2")

        # ================= phase B: GRU ===================================
        with tc.tile_pool(name="wbig", bufs=1) as wbig, \
             tc.tile_pool(name="wnat", bufs=2) as wnat, \
             tc.tile_pool(name="wst", bufs=3) as wst, \
             tc.tile_pool(name="g1", bufs=2) as g1:
            combr = wbig.tile([1, G], BF, tag="combr")
            nc.gpsimd.dma_start(combr[:], b_ih[:][None, :])
            bhhrow = wbig.tile([1, G], BF, tag="bhhrow")
            nc.gpsimd.dma_start(bhhrow[:], b_hh[:][None, :])
            nc.vector.tensor_add(combr[:, 0:2 * D], combr[:, 0:2 * D],
                                 bhhrow[:, 0:2 * D])
            bhhr_bf = bhhrow[:, 2 * D:3 * D]

            WT = wbig.tile([128, ET, G], GDT, tag="WT")
            tident = identb if GBF else ident

            def build_WT(w_dram):
                jts = kchunks(G)
                for g0 in range(0, len(jts), 5):
                    grp = jts[g0:g0 + 5]
                    ng = len(grp)
                    wn = wst.tile([128, 5, D], GDT, tag="wn")
                    src = w_dram[grp[0][0]:grp[-1][0] + grp[-1][1], :]
                    src = src.rearrange("(c p) d -> p c d", p=128)
                    if GBF:
                        nc.gpsimd.dma_start(wn[:, :ng, :], src)
                    else:
                        nc.sync.dma_start(wn[:, :ng, :], src)
                    sub = 5 if GBF else 3
                    for et in range(ET):
                        for s0 in range(0, ng, sub):
                            sg = min(sub, ng - s0)
                            pt = psB.tile([128, 128 * sub], GDT, tag="ptw")
                            for i in range(sg):
                                nc.tensor.matmul(pt[:, 128 * i:128 * (i + 1)],
                                                 wn[:, s0 + i, 128 * et:128 * (et + 1)],
                                                 tident[:], is_transpose=True,
                                                 skip_group_check=True)
                            w0 = grp[0][0] + 128 * s0
                            wlen = 128 * sg
                            if (et + s0) % 2 == 0:
                                nc.vector.tensor_copy(_r(WT[:, et, w0:w0 + wlen]),
                                                      pt[:, :wlen])
                            else:
                                nc.scalar.copy(_r(WT[:, et, w0:w0 + wlen]),
                                               pt[:, :wlen])

            build_WT(w_ih)

            # ---- phase A part 1 (cls -> a), interleaved after W_ih ------
            clsn = wbig.tile([BL, D], BF, tag="clsn")
            nc.gpsimd.dma_start(clsn[:], img[0:BL, 0, :])
            wA2 = wnat.tile([128, ET, D], BF, tag="wa")
            nc.gpsimd.dma_start(wA2[:], chunked(ga_w[2], nc))
            wA3 = wnat.tile([128, ET, D], BF, tag="wa")
            nc.gpsimd.dma_start(wA3[:], chunked(ga_w[3], nc))
            ptr = psB.tile([128, 512], BF, tag="ptw")
            for kt in range(ET):
                nc.tensor.matmul(ptr[:, 8 * kt:8 * kt + 8],
                                 clsn[:, 128 * kt:128 * (kt + 1)],
                                 identb[:BL, :BL], is_transpose=True,
                                 skip_group_check=True)
            clsT = wbig.tile([128, ET, BL], BF, tag="clsT")
            nc.vector.tensor_copy(clsT[:].rearrange("p a b -> p (a b)"),
                                  ptr[:, :8 * ET])

            def dense_T(wsb, rhsT, biasT, scaleT, otile, out_r=False):
                for mt in range(ET):
                    p = psC.tile([128, BL], FP, tag="pd")
                    for kt in range(ET):
                        nc.tensor.matmul(p[:], wsb[:, kt, 128 * mt:128 * (mt + 1)],
                                         rhsT[:, kt, :], start=(kt == 0),
                                         stop=(kt == ET - 1))
                    dst = otile[:, mt, :]
                    if out_r:
                        dst = _r(dst)
                    if scaleT is None:
                        nc.vector.tensor_scalar(dst, p[:], biasT[:, mt:mt + 1],
                                                None, OP.add)
                    else:
                        nc.vector.tensor_scalar(dst, p[:], biasT[:, mt:mt + 1],
                                                scaleT[:, 0:1], OP.add, OP.mult)

            A2T = wbig.tile([128, ET, BL], BF, tag="A2T")
            dense_T(wA2, clsT, b2gaT, None, A2T)
            dense_T(wA3, A2T, b3gaT, Sga, aT, out_r=not GBF)

            for (j0, jw) in CH_G:
                p = psA.tile([BL, 512], FP, tag="wh0")
                for kt in range(ET):
                    nc.tensor.matmul(p[:, :jw], aT[:, kt, :] if GBF else _r(aT[:, kt, :]),
                                     _r(WT[:, kt, j0:j0 + jw]),
                                     start=(kt == 0), stop=False)
                nc.tensor.matmul(p[:, :jw], ones1b[:1, :BL],
                                 combr[:, j0:j0 + jw], start=False, stop=True)
                nc.vector.tensor_copy(wxb[:, j0:j0 + jw], p[:, :jw])

            build_WT(w_hh)

            # ---- phase A part 2 (gout path, off critical path) ----------
            wG2 = wnat.tile([128, ET, D], BF, tag="wa")
            nc.gpsimd.dma_start(wG2[:], chunked(go_w[2], nc))
            wG3 = wnat.tile([128, ET, D], BF, tag="wa")
            nc.gpsimd.dma_start(wG3[:], chunked(go_w[3], nc))
            G2T = wbig.tile([128, ET, BL], BF, tag="G2T")
            dense_T(wG2, clsT, b2goT, None, G2T)
            dense_T(wG3, G2T, b3goT, Sgo, goutT)

            # transposed constant wx for the n-gate: [128, ET, BL]
            ptx = psC.tile([128, 64], BF, tag="pd")
            for kt in range(ET):
                nc.tensor.matmul(ptx[:, 8 * kt:8 * kt + 8],
                                 wxb[:, 2 * D + 128 * kt:2 * D + 128 * (kt + 1)],
                                 identb[:BL, :BL], is_transpose=True,
                                 skip_group_check=True)
            wxTn = wbig.tile([128, ET, BL], FP, tag="wxTn")
            nc.vector.tensor_copy(wxTn[:].rearrange("p a b -> p (a b)"),
                                  ptx[:, :8 * ET])

            # initial h0 transposed
            hnat0 = wbig.tile([BL, D], BF, tag="hnat0")
            nc.gpsimd.dma_start(hnat0[:], h0[:, :])
            ptr0 = psC.tile([128, 64], BF, tag="pd")
            for kt in range(ET):
                nc.tensor.matmul(ptr0[:, 8 * kt:8 * kt + 8],
                                 hnat0[:, 128 * kt:128 * (kt + 1)],
                                 identb[:BL, :BL], is_transpose=True,
                                 skip_group_check=True)
            hT = wbig.tile([128, ET, BL], GDT, tag="h0T")
            nc.vector.tensor_copy(_r(hT[:].rearrange("p a b -> p (a b)")),
                                  ptr0[:, :8 * ET])

            # prefetch DMAs for phases C/D/E: deprioritized so they only
            # fill DMA slots the W/A loads are not using
            with tc.high_priority(offset=-100000):
                W0 = persist.tile([128, ET, D], BF, tag="W0")
                nc.gpsimd.dma_start(W0[:], chunked(la_w[0], nc))
                W1 = persist.tile([128, ET, D], BF, tag="W1")
                nc.gpsimd.dma_start(W1[:], chunked(la_w[1], nc))
                W2 = persist.tile([128, ET, D], BF, tag="W2")
                nc.gpsimd.dma_start(W2[:], chunked(la_w[2], nc))
                W3 = persist.tile([128, ET, D], BF, tag="W3")
                nc.gpsimd.dma_start(W3[:], chunked(la_w[3], nc))
                f1 = persist.tile([128, 12, 1024], BF, tag="f1")
                nc.gpsimd.dma_start(f1[:], f1_w.rearrange("(c p) n -> p c n", p=128))
                f2 = persist.tile([128, 8, 512], BF, tag="f2")
                nc.gpsimd.dma_start(f2[:], f2_w.rearrange("(c p) n -> p c n", p=128))
                f3 = persist.tile([128, 4, 1024], BF, tag="f3")
                nc.gpsimd.dma_start(f3[:], f3_w.rearrange("(c p) n -> p c n", p=128))
                load_Xn(0)

            KSTEPS = int(os.environ.get("KSTEPS", str(T)))
            KHALF = os.environ.get("KHALF", "1") == "1"
            HB = BL // 2
            wxTn3 = wxTn[:]
            if not KHALF:
                for t in range(KSTEPS):
                    psR = psD.tile([128, ET * BL], FP, tag="gr")
                    psZ = psD.tile([128, ET * BL], FP, tag="gz")
                    psN = psD.tile([128, ET * BL], FP, tag="gn")

                    def gate_chunk(ps, mi, m):
                        j0 = 128 * m
                        for kt in range(ET):
                            nc.tensor.matmul(ps[:, BL * mi:BL * (mi + 1)],
                                             _r(WT[:, kt, j0:j0 + 128]),
                                             _r(hT[:, kt, :]),
                                             start=(kt == 0), stop=False,
                                             skip_group_check=True)
                        if m < 12:
                            nc.tensor.matmul(ps[:, BL * mi:BL * (mi + 1)],
                                             wxb[:, j0:j0 + 128],
                                             identb[:BL, :BL], start=False,
                                             stop=True, skip_group_check=True)
                        else:
                            nc.tensor.matmul(ps[:, BL * mi:BL * (mi + 1)],
                                             bhhr_bf[:, j0 - 2 * D:j0 - 2 * D + 128],
                                             ones1b[:1, :BL],
                                             start=False, stop=True,
                                             skip_group_check=True)

                    for mi in range(ET):
                        gate_chunk(psR, mi, mi)
                    for mi in range(ET):
                        gate_chunk(psN, mi, 12 + mi)
                    for mi in range(ET):
                        gate_chunk(psZ, mi, 6 + mi)

                    # h_new = (1-z)*n + z*h ; z-products run in tanh's shadow
                    rsig = g1.tile([128, ET * BL], FP, tag="rsig")
                    nc.scalar.activation(rsig[:], psR[:], AF.Sigmoid)
                    zsig = g1.tile([128, ET * BL], FP, tag="zsig")
                    nc.scalar.activation(zsig[:], psZ[:], AF.Sigmoid)
                    rwn = g1.tile([128, ET * BL], FP, tag="rwn")
                    nc.vector.tensor_mul(rwn[:], rsig[:], psN[:])
                    npre = g1.tile([128, ET * BL], FP, tag="npre")
                    nc.vector.tensor_add(npre[:], rwn[:],
                                         wxTn[:].rearrange("p a b -> p (a b)"))
                    nt_ = g1.tile([128, ET * BL], FP, tag="nt")
                    nc.scalar.activation(nt_[:], npre[:], AF.Tanh)
                    zh = g1.tile([128, ET * BL], FP, tag="zh")
                    nc.vector.tensor_mul(zh[:], zsig[:],
                                         hT[:].rearrange("p a b -> p (a b)"))
                    omz = g1.tile([128, ET * BL], FP, tag="omz")
                    nc.vector.tensor_scalar(omz[:], zsig[:], -1.0, 1.0,
                                            OP.mult, OP.add)
                    ozn = g1.tile([128, ET * BL], FP, tag="ozn")
                    nc.vector.tensor_mul(ozn[:], omz[:], nt_[:])
                    hT = g1.tile([128, ET, BL], GDT, tag="hT")
                    nc.vector.tensor_add(_r(hT[:].rearrange("p a b -> p (a b)")),
                                         ozn[:], zh[:])
                    nc.scalar.copy(qembT[:, :, :, t].rearrange("p a b -> p (a b)"),
                                   hT[:].rearrange("p a b -> p (a b)"))
            else:
                # two independent half-batch chains, interleaved so each
                # half's elementwise hides in the other's latency
                hTs = [None, None]
                psmap = [(psD, "gr"), (psD, "gz"), (psA, "wh0"), (psA, "wh1"),
                         (psD, "gn"), (psC, "pd")]
                for t in range(KSTEPS):
                    def hprev(g, kt):
                        if t == 0:
                            return _r(hT[:, kt, HB * g:HB * (g + 1)])
                        return _r(hTs[g][:, kt, :])

                    def ps_half(i):
                        pl, tg = psmap[i]
                        ph_t = pl.tile([128, ET * HB], FP, tag=tg)
                        return ph_t

                    psRs = [ps_half(0), ps_half(1)]
                    psZs = [ps_half(2), ps_half(3)]
                    psNs = [ps_half(4), ps_half(5)]

                    def gate_chunk2(ps, mi, m, g):
                        j0 = 128 * m
                        for kt in range(ET):
                            nc.tensor.matmul(ps[:, HB * mi:HB * (mi + 1)],
                                             _r(WT[:, kt, j0:j0 + 128]),
                                             hprev(g, kt),
                                             start=(kt == 0), stop=False,
                                             skip_group_check=True)
                        if m < 12:
                            nc.tensor.matmul(ps[:, HB * mi:HB * (mi + 1)],
                                             wxb[:, j0:j0 + 128],
                                             identb[:BL, HB * g:HB * (g + 1)],
                                             start=False, stop=True,
                                             skip_group_check=True)
                        else:
                            nc.tensor.matmul(ps[:, HB * mi:HB * (mi + 1)],
                                             bhhr_bf[:, j0 - 2 * D:j0 - 2 * D + 128],
                                             ones1b[:1, :HB],
                                             start=False, stop=True,
                                             skip_group_check=True)

                    for g in (0, 1):
                        for mi in range(ET):
                            gate_chunk2(psRs[g], mi, mi, g)
                    for g in (0, 1):
                        for mi in range(ET):
                            gate_chunk2(psNs[g], mi, 12 + mi, g)
                    for g in (0, 1):
                        for mi in range(ET):
                            gate_chunk2(psZs[g], mi, 6 + mi, g)

                    def tile3(tag):
                        t3 = g1.tile([128, ET, HB], FP, tag=tag)
                        return t3

                    rsig = [tile3("rsig0"), tile3("rsig1")]
                    zsig = [tile3("zsig0"), tile3("zsig1")]
                    rwn = [tile3("rwn0"), tile3("rwn1")]
                    npre = [tile3("npre0"), tile3("npre1")]
                    nt_ = [tile3("nt0"), tile3("nt1")]
                    zh = [tile3("zh0"), tile3("zh1")]
                    omz = [tile3("omz0"), tile3("omz1")]
                    ozn = [tile3("ozn0"), tile3("ozn1")]
                    def tile3g(tag):
                        t3g = g1.tile([128, ET, HB], GDT, tag=tag)
                        return t3g

                    hnew = [tile3g("hTn0"), tile3g("hTn1")]
                    for g in (0, 1):
                        nc.scalar.activation(rsig[g][:].rearrange("p a b -> p (a b)"),
                                             psRs[g][:], AF.Sigmoid)
                    for g in (0, 1):
                        nc.scalar.activation(zsig[g][:].rearrange("p a b -> p (a b)"),
                                             psZs[g][:], AF.Sigmoid)
                    for g in (0, 1):
                        nc.vector.tensor_mul(rwn[g][:].rearrange("p a b -> p (a b)"),
                                             rsig[g][:].rearrange("p a b -> p (a b)"),
                                             psNs[g][:])
                    for g in (0, 1):
                        nc.vector.tensor_add(npre[g][:], rwn[g][:],
                                             wxTn3[:, :, HB * g:HB * (g + 1)])
                    for g in (0, 1):
                        nc.scalar.activation(nt_[g][:].rearrange("p a b -> p (a b)"),
                                             npre[g][:].rearrange("p a b -> p (a b)"),
                                             AF.Tanh)
                    for g in (0, 1):
                        hp = (hT[:, :, HB * g:HB * (g + 1)] if t == 0
                              else hTs[g][:])
                        nc.vector.tensor_mul(zh[g][:], zsig[g][:], hp)
                    for g in (0, 1):
                        nc.vector.tensor_scalar(omz[g][:].rearrange("p a b -> p (a b)"),
                                                zsig[g][:].rearrange("p a b -> p (a b)"),
                                                -1.0, 1.0, OP.mult, OP.add)
                    for g in (0, 1):
                        nc.vector.tensor_mul(ozn[g][:], omz[g][:], nt_[g][:])
                    for g in (0, 1):
                        nc.vector.tensor_add(_r(hnew[g][:]), ozn[g][:], zh[g][:])
                    for g in (0, 1):
                        nc.scalar.copy(qembT[:, :, HB * g:HB * (g + 1), t],
                                       hnew[g][:])
                    hTs = hnew
            load_Xn(1)

        # ================= phase C: Q^T, W1^T, Qt^T =======================
        with tc.tile_pool(name="prep", bufs=1) as prep:
            QT = prep.tile([128, ET, RQ], BF, tag="QT")
            qflat = qembT[:].rearrange("p a b t -> p a (b t)")
            for mt in range(ET):
                p = pgroup(mt)
                for kt in range(ET):
                    nc.tensor.matmul(p[:], W0[:, kt, 128 * mt:128 * (mt + 1)],
                                     qflat[:, kt, :], start=(kt == 0),
                                     stop=(kt == ET - 1))
                nc.vector.tensor_scalar(QT[:, mt, :], p[:], b0laT[:, mt:mt + 1],
                                        None, OP.add)
            W1T = prep.tile([128, ET, D], BF, tag="W1T")
            for hd in range(ET):
                for grp in range(2):
                    pt2 = psB.tile([128, 512], BF, tag="ptw")
                    for i in range(3):
                        e2 = grp * 3 + i
                        nc.tensor.matmul(pt2[:, 128 * i:128 * (i + 1)],
                                         W1[:, e2, 128 * hd:128 * (hd + 1)],
                                         identb[:], is_transpose=True,
                                         skip_group_check=True)
                    if grp == 0:
                        nc.vector.tensor_copy(W1T[:, hd, 0:384], pt2[:, 0:384])
                    else:
                        nc.scalar.copy(W1T[:, hd, 384:768], pt2[:, 0:384])
            scl = 1.0 / float(np.sqrt(DK))
            for h in range(NH):
                for mt in range(ET):
                    p = pgroup(h * ET + mt)
                    for i in range(3):
                        kt = h * 3 + i
                        nc.tensor.matmul(p[:], W1T[:, kt, 128 * mt:128 * (mt + 1)],
                                         QT[:, kt, :], start=(i == 0), stop=(i == 2))
                    dst = QtT[:, mt, :].rearrange("p (b h2 t) -> p b h2 t",
                                                  h2=NH, t=T)[:, :, h, :]
                    if (h * ET + mt) % 3 != 2:
                        nc.vector.tensor_scalar(dst, p[:], scl, None, OP.mult)
                    else:
                        nc.scalar.activation(dst, p[:], AF.Copy, scale=scl)

        # ================= phase D: per-b attention =======================
        with tc.tile_pool(name="ab", bufs=2) as ab, \
             tc.tile_pool(name="xbp2", bufs=1) as xb2_pool:
            xb2[0] = xb2_pool
            for b in range(BL):
                for bn in (b + 1, b + 2):
                    if bn < BL and bn not in XnMap:
                        load_Xn(bn)
                Xn = XnMap.pop(b)
                XT = ab.tile([128, ET, NKC * 128], BF, tag="XT")
                for et in range(ET):
                    pt = psB.tile([128, 1024], BF, tag="ptw")
                    for c in range(NKC):
                        nc.tensor.matmul(pt[:, 128 * c:128 * (c + 1)],
                                         Xn[:, c, 128 * et:128 * (et + 1)],
                                         identb[:], is_transpose=True,
                                         skip_group_check=True)
                    if et % 3 != 2:
                        nc.vector.tensor_copy(XT[:, et, :], pt[:])
                    else:
                        nc.scalar.copy(XT[:, et, :], pt[:])
                att = ab.tile([64, NKC * 128], BF, tag="att")
                nc.vector.memset(att[:, NK:], 0.0)
                zacc = ab.tile([64, 2], FP, tag="zacc")
                for ci, (n0, nw) in enumerate(CH_NK):
                    p = psA.tile([64, 512], FP, tag=f"wh{ci}")
                    for kt in range(ET):
                        nc.tensor.matmul(p[:, :nw],
                                         QtT[:, kt, b * 2 * T:(b + 1) * 2 * T],
                                         XT[:, kt, n0:n0 + nw],
                                         start=(kt == 0), stop=(kt == ET - 1))
                    nc.scalar.activation(att[:, n0:n0 + nw], p[:, :nw], AF.Exp,
                                         accum_out=zacc[:, ci:ci + 1])
                zs = ab.tile([64, 1], FP, tag="zs")
                nc.vector.tensor_add(zs[:], zacc[:, 0:1], zacc[:, 1:2])
                rz = ab.tile([64, 1], FP, tag="rz1")
                nc.vector.reciprocal(rz[:], zs[:])
                wm = ab.tile([64, 2], BF, tag="wm")
                nc.vector.tensor_scalar(wm[:], pmask[:], rz[:, 0:1], None, OP.mult)
                # paT[k, i] = sum_q att[q, k] * wm[q, i]  (no transposes!)
                pp = psD.tile([128, 2 * NKC], FP, tag="gr")
                for c in range(NKC):
                    nc.tensor.matmul(pp[:, 2 * c:2 * c + 2],
                                     att[:, 128 * c:128 * (c + 1)], wm[:],
                                     start=True, stop=True,
                                     skip_group_check=True)
                paT = ab.tile([128, NKC, 2], BF, tag="paT")
                nc.vector.tensor_copy(paT[:].rearrange("p a b -> p (a b)"), pp[:])
                # ctxT[d, i] = sum_k Xn[k, d] * paT[k, i]
                pc = psD.tile([128, 2 * ET], FP, tag="gz")
                for dc in range(ET):
                    for c in range(NKC):
                        nc.tensor.matmul(pc[:, 2 * dc:2 * dc + 2],
                                         Xn[:, c, 128 * dc:128 * (dc + 1)],
                                         paT[:, c, :], start=(c == 0),
                                         stop=(c == NKC - 1),
                                         skip_group_check=True)
                nc.vector.tensor_copy(
                    pcxT2[:, :, :, b].rearrange("p a b -> p (a b)"), pc[:])

        # ================= phase E: projections + MLP =====================
        with tc.tile_pool(name="tail", bufs=1) as tail:
            vconT = tail.tile([128, ET], FP, tag="vconT")
            for mt in range(ET):
                p = pgroup(mt)
                for kt in range(ET):
                    nc.tensor.matmul(p[:, 0:1], W3[:, kt, 128 * mt:128 * (mt + 1)],
                                     b2laT_bf[:, kt:kt + 1], start=(kt == 0),
                                     stop=(kt == ET - 1), skip_group_check=True)
                nc.vector.tensor_scalar(vconT[:, mt:mt + 1], p[:, 0:1],
                                        b3laT[:, mt:mt + 1], Sla[:, 0:1],
                                        OP.add, OP.mult)
            pctxT = tail.tile([128, ET, BL], BF, tag="pctxT")
            for h in range(NH):
                for mi in range(3):
                    mt = h * 3 + mi
                    p = pgroup(mt)
                    for kt in range(ET):
                        nc.tensor.matmul(p[:, 0:BL],
                                         W2[:, kt, 128 * mt:128 * (mt + 1)],
                                         pcxT2[:, kt, h, :], start=(kt == 0),
                                         stop=(kt == ET - 1),
                                         skip_group_check=True)
                    nc.vector.tensor_copy(pctxT[:, mt, :], p[:, 0:BL])
            loT = tail.tile([128, ET, BL], BF, tag="loT")
            for mt in range(ET):
                p = pgroup(mt)
                for kt in range(ET):
                    nc.tensor.matmul(p[:, 0:BL], W3[:, kt, 128 * mt:128 * (mt + 1)],
                                     pctxT[:, kt, :], start=(kt == 0),
                                     stop=(kt == ET - 1), skip_group_check=True)
                nc.vector.tensor_scalar(loT[:, mt, :], p[:, 0:BL],
                                        vconT[:, mt:mt + 1], None, OP.add)

            y1T = tail.tile([128, 8, BL], BF, tag="y1T")
            for mt in range(8):
                p = pgroup(mt)
                for kt in range(12):
                    r_ = loT[:, kt, :] if kt < ET else goutT[:, kt - ET, :]
                    nc.tensor.matmul(p[:, 0:BL], f1[:, kt, 128 * mt:128 * (mt + 1)],
                                     r_, start=(kt == 0), stop=(kt == 11),
                                     skip_group_check=True)
                nc.vector.tensor_scalar(y1T[:, mt, :], p[:, 0:BL],
                                        b1fT[:, mt:mt + 1], None, OP.add)
            y2T = tail.tile([128, 4, BL], BF, tag="y2T")
            for mt in range(4):
                p = pgroup(mt)
                for kt in range(8):
                    nc.tensor.matmul(p[:, 0:BL], f2[:, kt, 128 * mt:128 * (mt + 1)],
                                     y1T[:, kt, :], start=(kt == 0), stop=(kt == 7),
                                     skip_group_check=True)
                nc.scalar.activation(y2T[:, mt, :], p[:, 0:BL], AF.Relu,
                                     bias=b2fT[:, mt:mt + 1])
            yT = tail.tile([128, 8, BL], FP, tag="yT")
            for mt in range(8):
                p = pgroup(mt)
                for kt in range(4):
                    nc.tensor.matmul(p[:, 0:BL], f3[:, kt, 128 * mt:128 * (mt + 1)],
                                     y2T[:, kt, :], start=(kt == 0), stop=(kt == 3),
                                     skip_group_check=True)
                nc.vector.tensor_scalar(yT[:, mt, :], p[:, 0:BL],
                                        b3fT[:, mt:mt + 1], None, OP.add)
            ynat = tail.tile([BL, 1024], FP, tag="ynat")
            for g in range(2):
                po = psB.tile([128, 512], FP, tag="ptw")
                for i in range(4):
                    mt = g * 4 + i
                    nc.tensor.matmul(po[:BL, 128 * i:128 * (i + 1)], yT[:, mt, :],
                                     ident[:128, :128], is_transpose=True,
                                     skip_group_check=True)
                nc.vector.tensor_copy(ynat[:, 512 * g:512 * (g + 1)], po[:BL, :])
            nc.sync.dma_start(out_d[:, :], ynat[:])

    nc.compile()
    return nc


_NC = None


def kernel(**inputs):
    global _NC
    if _NC is None:
        _NC = build()
    B = inputs["image_local_embeds"].shape[0]
    per = B // NCORES
    in_maps = []
    for c in range(NCORES):
        sl = slice(c * per, (c + 1) * per)
        m = {
            "img": np.ascontiguousarray(np.asarray(inputs["image_local_embeds"])[sl], dtype=np.float32),
            "h0": np.ascontiguousarray(np.asarray(inputs["h0"])[sl], dtype=np.float32),
        }
        for k in ["gru_w_ih", "gru_w_hh", "gru_b_ih", "gru_b_hh", "ga_w", "ga_b",
                  "ga_pool", "la_w", "la_b", "la_pool", "go_w", "go_b", "go_pool",
                  "f1_w", "f1_b", "f2_w", "f2_b", "f3_w", "f3_b"]:
            m[k] = np.ascontiguousarray(np.asarray(inputs[k], dtype=np.float32))
        in_maps.append(m)
    res = run_bass_kernel_spmd(_NC, in_maps, core_ids=list(range(NCORES)))
    return np.concatenate([res.results[c]["out"] for c in range(NCORES)], axis=0)


# revision 72
# speedup vs baseline: 1.0962x; 1.0277x over previous
"""Trainium2 Bass kernel for nn_BiVision_VQA2 (B=64,T=32,D=768,N=901).

Data-parallel over batch: 8 batch elems per core x 8 cores.
Key math simplifications (validated vs reference, rel err ~1e-4):
  - ga/go attention use a single key token -> softmax==1 -> those paths are
    linear in cls; question_embeds is mathematically unused.
  - GRU input `a` is constant over time; wx computed once.
  - local attention: row-constant score terms drop out of softmax; query
    pooling applied to the attention matrix before the @X contraction.
Performance structure:
  - GRU computed in TRANSPOSED gate layout [128(gate row), batch] via
    weight-stationary matmuls (moving N=8), elementwise on [128, 48].
  - bf16 everywhere outside the GRU recurrence (DMA casts on load).
  - phase D: transpose-free paT/ctxT via natural-operand-stationary matmuls.
"""

import os
import numpy as np
from contextlib import ExitStack

import concourse.bass as bass
import concourse.tile as tile
from concourse import bacc, mybir
from concourse.bass_utils import run_bass_kernel_spmd
from concourse.masks import make_identity

FP = mybir.dt.float32
FPR = mybir.dt.float32r
OP = mybir.AluOpType
AF = mybir.ActivationFunctionType
BF = mybir.dt.bfloat16

NCORES = 8
BL = 8
D = 768
T = 32
G = 3 * D
NK = 900
NH = 2
DK = 384
ET = D // 128
RQ = BL * T

CH_G = [(0, 512), (512, 512), (1024, 512), (1536, 512), (2048, 256)]
CH_NK = [(0, 512), (512, 388)]

GBF = os.environ.get("KGRUBF", "1") == "1"
GDT = BF if GBF else FP


def _r(ap):
    return ap if GBF else ap.bitcast(FPR)


from contextlib import contextmanager


@contextmanager
def _nullcm():
    yield


def kchunks(n):
    out, o = [], 0
    while o < n:
        out.append((o, min(128, n - o)))
        o += 128
    return out


def build():
    nc = bacc.Bacc("TRN2", target_bir_lowering=False, debug=False,
                   enable_asserts=False)

    img = nc.dram_tensor("img", [BL, 901, D], FP, kind="ExternalInput").ap()
    h0 = nc.dram_tensor("h0", [BL, D], FP, kind="ExternalInput").ap()
    w_ih = nc.dram_tensor("gru_w_ih", [G, D], FP, kind="ExternalInput").ap()
    w_hh = nc.dram_tensor("gru_w_hh", [G, D], FP, kind="ExternalInput").ap()
    b_ih = nc.dram_tensor("gru_b_ih", [G], FP, kind="ExternalInput").ap()
    b_hh = nc.dram_tensor("gru_b_hh", [G], FP, kind="ExternalInput").ap()
    ga_w = nc.dram_tensor("ga_w", [4, D, D], FP, kind="ExternalInput").ap()
    ga_b = nc.dram_tensor("ga_b", [4, D], FP, kind="ExternalInput").ap()
    ga_pool = nc.dram_tensor("ga_pool", [1], FP, kind="ExternalInput").ap()
    la_w = nc.dram_tensor("la_w", [4, D, D], FP, kind="ExternalInput").ap()
    la_b = nc.dram_tensor("la_b", [4, D], FP, kind="ExternalInput").ap()
    la_pool = nc.dram_tensor("la_pool", [T], FP, kind="ExternalInput").ap()
    go_w = nc.dram_tensor("go_w", [4, D, D], FP, kind="ExternalInput").ap()
    go_b = nc.dram_tensor("go_b", [4, D], FP, kind="ExternalInput").ap()
    go_pool = nc.dram_tensor("go_pool", [T], FP, kind="ExternalInput").ap()
    f1_w = nc.dram_tensor("f1_w", [2 * D, 1024], FP, kind="ExternalInput").ap()
    f1_b = nc.dram_tensor("f1_b", [1024], FP, kind="ExternalInput").ap()
    f2_w = nc.dram_tensor("f2_w", [1024, 512], FP, kind="ExternalInput").ap()
    f2_b = nc.dram_tensor("f2_b", [512], FP, kind="ExternalInput").ap()
    f3_w = nc.dram_tensor("f3_w", [512, 1024], FP, kind="ExternalInput").ap()
    f3_b = nc.dram_tensor("f3_b", [1024], FP, kind="ExternalInput").ap()
    out_d = nc.dram_tensor("out", [BL, 1024], FP, kind="ExternalOutput").ap()

    def chunked(dram2d, nc_, cw=D):
        # [R, cw] dram viewed as [128, R//128, cw]
        return dram2d.rearrange("(c p) d -> p c d", p=128)

    with tile.TileContext(nc) as tc, ExitStack() as ctx:
        cpool = ctx.enter_context(tc.tile_pool(name="const", bufs=1))
        persist = ctx.enter_context(tc.tile_pool(name="persist", bufs=1))
        xb = ctx.enter_context(tc.tile_pool(name="xb", bufs=2))
        psA = ctx.enter_context(tc.tile_pool(name="psA", bufs=1, space="PSUM"))
        psB = ctx.enter_context(tc.tile_pool(name="psB", bufs=2, space="PSUM"))
        psC = ctx.enter_context(tc.tile_pool(name="psC", bufs=1, space="PSUM"))
        psD = ctx.enter_context(tc.tile_pool(name="psD", bufs=1, space="PSUM"))

        ident = cpool.tile([128, 128], FP, tag="ident")
        make_identity(nc, ident[:])
        identb = cpool.tile([128, 128], BF, tag="identb")
        nc.vector.tensor_copy(identb[:], ident[:])
        ones1 = cpool.tile([1, 128], FP, tag="ones1")
        nc.vector.memset(ones1[:], 1.0)
        ones1b = cpool.tile([1, 128], BF, tag="ones1b")
        nc.vector.memset(ones1b[:], 1.0)
        onesT = cpool.tile([T, 128], FP, tag="onesT")
        nc.vector.memset(onesT[:], 1.0)

        # ---- small bias vectors -> column layout via K=1 matmuls ---------
        def colvec_batch(specs):
            # pipelined: all row-loads first (3 rotating staging slots),
            # then K=1 matmuls into one psum tile, then copies out.
            pdvl = psC.tile([128, 64], FP, tag="pd")
            off = 0
            outs = []
            for idx, (src, n) in enumerate(specs):
                nt = n // 128
                vr = cpool.tile([1, 1024], FP, tag=f"vrow{idx % 2}")
                nc.sync.dma_start(vr[:, :n], src[:][None, :])
                for c in range(nt):
                    nc.tensor.matmul(pdvl[:, off + c:off + c + 1],
                                     vr[0:1, 128 * c:128 * (c + 1)],
                                     ones1[:1, :1], start=True, stop=True,
                                     skip_group_check=True)
                outs.append((off, nt))
                off += nt
            return pdvl, outs

        def colvec_out(pdvl, o_nt, tag):
            o, nt = o_nt
            t_ = cpool.tile([128, nt], FP, tag=tag)
            nc.vector.tensor_copy(t_[:], pdvl[:, o:o + nt])
            return t_

        pdv1, offs1 = colvec_batch([(ga_b[2], D), (ga_b[3], D),
                                    (go_b[2], D), (go_b[3], D),
                                    (la_b[0], D), (la_b[2], D),
                                    (la_b[3], D), (f1_b, 1024),
                                    (f2_b, 512), (f3_b, 1024)])
        b2gaT = colvec_out(pdv1, offs1[0], "b2gaT")
        b3gaT = colvec_out(pdv1, offs1[1], "b3gaT")
        b2goT = colvec_out(pdv1, offs1[2], "b2goT")
        b3goT = colvec_out(pdv1, offs1[3], "b3goT")
        b0laT = colvec_out(pdv1, offs1[4], "b0laT")
        b2laT = colvec_out(pdv1, offs1[5], "b2laT")
        b3laT = colvec_out(pdv1, offs1[6], "b3laT")
        b1fT = colvec_out(pdv1, offs1[7], "b1fT")
        b2fT = colvec_out(pdv1, offs1[8], "b2fT")
        b3fT = colvec_out(pdv1, offs1[9], "b3fT")
        b2laT_bf = cpool.tile([128, ET], BF, tag="b2laT_bf")
        nc.vector.tensor_copy(b2laT_bf[:], b2laT[:])

        lapool_c = cpool.tile([T, 1], FP, tag="lapool_c")
        nc.sync.dma_start(lapool_c[:], la_pool[:][:, None])
        gopool_c = cpool.tile([T, 1], FP, tag="gopool_c")
        nc.sync.dma_start(gopool_c[:], go_pool[:][:, None])
        gapool_c = cpool.tile([1, 1], FP, tag="gapool_c")
        nc.sync.dma_start(gapool_c[:], ga_pool[:][:, None])

        def sum_bcast(vcol, k, tag):
            p = psC.tile([128, 64], FP, tag="pd")
            lhs = onesT if k == T else ones1
            nc.tensor.matmul(p[:, 0:1], lhs[:k, :], vcol[:k, :], start=True,
                             stop=True, skip_group_check=True)
            s = cpool.tile([128, 1], FP, tag=tag)
            nc.vector.tensor_copy(s[:], p[:, 0:1])
            return s

        Sla = sum_bcast(lapool_c, T, "Sla")
        Sgo = sum_bcast(gopool_c, T, "Sgo")
        Sga = sum_bcast(gapool_c, 1, "Sga")

        pmask = cpool.tile([64, 2], FP, tag="pmask")
        nc.vector.memset(pmask[:], 0.0)
        nc.sync.dma_start(pmask[0:T, 0:1], la_pool[:][:, None])
        nc.sync.dma_start(pmask[T:2 * T, 1:2], la_pool[:][:, None])

        # img patch loads (streamed; b0/b1 prefetched early)
        KC = kchunks(NK)
        NKC = len(KC)
        XnMap = {}

        xb2 = [None]

        def load_Xn(b):
            # 3-way buffer rotation: xb holds 2, xb2 (opened for phase D,
            # reusing SBUF freed by the GRU pools) holds the third
            pool = xb2[0] if (b % 3 == 2 and xb2[0] is not None) else xb
            Xn = pool.tile([128, NKC, D], BF, tag="Xn")
            if b < 3:
                # zero the pad rows once per physical buffer (b0,b1 -> xb's
                # two buffers, b2 -> xb2); later b's reuse a buffer and only
                # ever rewrite rows 0..kwl of the last chunk
                nc.vector.memset(Xn[:, NKC - 1, :], 0.0)
            nc.gpsimd.dma_start(
                Xn[:, 0:NKC - 1, :],
                img[b, 1:1 + 128 * (NKC - 1), :].rearrange(
                    "(c p) d -> p c d", p=128))
            k0l, kwl = KC[-1]
            nc.gpsimd.dma_start(Xn[:kwl, NKC - 1, :],
                                img[b, 1 + k0l:1 + k0l + kwl, :])
            XnMap[b] = Xn

        def pgroup(i, ncols=RQ):
            pl, tg = [(psC, "pd"), (psD, "gr"), (psD, "gz"), (psD, "gn")][i % 4]
            pg_t = pl.tile([128, ncols], FP, tag=tg)
            return pg_t

        # persistent outputs of the phases
        qembT = cpool.tile([128, ET, BL, T], BF, tag="qembT")
        wxb = cpool.tile([BL, G], BF, tag="wxb")
        QtT = persist.tile([128, ET, NH * RQ], BF, tag="QtT")
        goutT = cpool.tile([128, ET, BL], BF, tag="goutT")
        aT = cpool.tile([128, ET, BL], GDT, tag="aT")
        pcxT2 = persist.tile([128, ET, NH, BL], BF, tag="pcxT2")

        # ================= phase B: GRU ===================================
        with tc.tile_pool(name="wbig", bufs=1) as wbig, \
             tc.tile_pool(name="wnat", bufs=2) as wnat, \
             tc.tile_pool(name="wst", bufs=3) as wst, \
             tc.tile_pool(name="g1", bufs=2) as g1:
            combr = wbig.tile([1, G], BF, tag="combr")
            nc.gpsimd.dma_start(combr[:], b_ih[:][None, :])
            bhhrow = wbig.tile([1, G], BF, tag="bhhrow")
            nc.gpsimd.dma_start(bhhrow[:], b_hh[:][None, :])
            nc.vector.tensor_add(combr[:, 0:2 * D], combr[:, 0:2 * D],
                                 bhhrow[:, 0:2 * D])
            bhhr_bf = bhhrow[:, 2 * D:3 * D]

            WT = wbig.tile([128, ET, G], GDT, tag="WT")
            tident = identb if GBF else ident

            def build_WT(w_dram, dma_prio=0):
                jts = kchunks(G)
                for g0 in range(0, len(jts), 5):
                    grp = jts[g0:g0 + 5]
                    ng = len(grp)
                    wn = wst.tile([128, 5, D], GDT, tag="wn")
                    src = w_dram[grp[0][0]:grp[-1][0] + grp[-1][1], :]
                    src = src.rearrange("(c p) d -> p c d", p=128)
                    with tc.high_priority(offset=dma_prio if dma_prio else None) \
                            if dma_prio else _nullcm():
                        if GBF:
                            nc.gpsimd.dma_start(wn[:, :ng, :], src)
                        else:
                            nc.sync.dma_start(wn[:, :ng, :], src)
                    sub = 5 if GBF else 3
                    for et in range(ET):
                        for s0 in range(0, ng, sub):
                            sg = min(sub, ng - s0)
                            pt = psB.tile([128, 128 * sub], GDT, tag="ptw")
                            for i in range(sg):
                                nc.tensor.matmul(pt[:, 128 * i:128 * (i + 1)],
                                                 wn[:, s0 + i, 128 * et:128 * (et + 1)],
                                                 tident[:], is_transpose=True,
                                                 skip_group_check=True)
                            w0 = grp[0][0] + 128 * s0
                            wlen = 128 * sg
                            if (et + s0) % 2 == 0:
                                nc.vector.tensor_copy(_r(WT[:, et, w0:w0 + wlen]),
                                                      pt[:, :wlen])
                            else:
                                nc.scalar.copy(_r(WT[:, et, w0:w0 + wlen]),
                                               pt[:, :wlen])

            build_WT(w_ih)

            # ---- phase A part 1 (cls -> a), interleaved after W_ih ------
            clsn = wbig.tile([BL, D], BF, tag="clsn")
            nc.gpsimd.dma_start(clsn[:], img[0:BL, 0, :])
            wA2 = wnat.tile([128, ET, D], BF, tag="wa")
            nc.gpsimd.dma_start(wA2[:], chunked(ga_w[2], nc))
            wA3 = wnat.tile([128, ET, D], BF, tag="wa")
            nc.gpsimd.dma_start(wA3[:], chunked(ga_w[3], nc))
            ptr = psB.tile([128, 512], BF, tag="ptw")
            for kt in range(ET):
                nc.tensor.matmul(ptr[:, 8 * kt:8 * kt + 8],
                                 clsn[:, 128 * kt:128 * (kt + 1)],
                                 identb[:BL, :BL], is_transpose=True,
                                 skip_group_check=True)
            clsT = wbig.tile([128, ET, BL], BF, tag="clsT")
            nc.vector.tensor_copy(clsT[:].rearrange("p a b -> p (a b)"),
                                  ptr[:, :8 * ET])

            def dense_T(wsb, rhsT, biasT, scaleT, otile, out_r=False):
                for mt in range(ET):
                    p = psC.tile([128, BL], FP, tag="pd")
                    for kt in range(ET):
                        nc.tensor.matmul(p[:], wsb[:, kt, 128 * mt:128 * (mt + 1)],
                                         rhsT[:, kt, :], start=(kt == 0),
                                         stop=(kt == ET - 1))
                    dst = otile[:, mt, :]
                    if out_r:
                        dst = _r(dst)
                    if scaleT is None:
                        nc.vector.tensor_scalar(dst, p[:], biasT[:, mt:mt + 1],
                                                None, OP.add)
                    else:
                        nc.vector.tensor_scalar(dst, p[:], biasT[:, mt:mt + 1],
                                                scaleT[:, 0:1], OP.add, OP.mult)

            A2T = wbig.tile([128, ET, BL], BF, tag="A2T")
            dense_T(wA2, clsT, b2gaT, None, A2T)
            dense_T(wA3, A2T, b3gaT, Sga, aT, out_r=not GBF)

            for (j0, jw) in CH_G:
                p = psA.tile([BL, 512], FP, tag="wh0")
                for kt in range(ET):
                    nc.tensor.matmul(p[:, :jw], aT[:, kt, :] if GBF else _r(aT[:, kt, :]),
                                     _r(WT[:, kt, j0:j0 + jw]),
                                     start=(kt == 0), stop=False)
                nc.tensor.matmul(p[:, :jw], ones1b[:1, :BL],
                                 combr[:, j0:j0 + jw], start=False, stop=True)
                nc.vector.tensor_copy(wxb[:, j0:j0 + jw], p[:, :jw])

            build_WT(w_hh)

            # ---- phase A part 2 (gout path, off critical path) ----------
            wG2 = wnat.tile([128, ET, D], BF, tag="wa")
            nc.gpsimd.dma_start(wG2[:], chunked(go_w[2], nc))
            wG3 = wnat.tile([128, ET, D], BF, tag="wa")
            nc.gpsimd.dma_start(wG3[:], chunked(go_w[3], nc))
            G2T = wbig.tile([128, ET, BL], BF, tag="G2T")
            dense_T(wG2, clsT, b2goT, None, G2T)
            dense_T(wG3, G2T, b3goT, Sgo, goutT)

            # transposed constant wx for the n-gate: [128, ET, BL]
            ptx = psC.tile([128, 64], BF, tag="pd")
            for kt in range(ET):
                nc.tensor.matmul(ptx[:, 8 * kt:8 * kt + 8],
                                 wxb[:, 2 * D + 128 * kt:2 * D + 128 * (kt + 1)],
                                 identb[:BL, :BL], is_transpose=True,
                                 skip_group_check=True)
            wxTn = wbig.tile([128, ET, BL], FP, tag="wxTn")
            nc.vector.tensor_copy(wxTn[:].rearrange("p a b -> p (a b)"),
                                  ptx[:, :8 * ET])

            # initial h0 transposed
            hnat0 = wbig.tile([BL, D], BF, tag="hnat0")
            nc.gpsimd.dma_start(hnat0[:], h0[:, :])
            ptr0 = psC.tile([128, 64], BF, tag="pd")
            for kt in range(ET):
                nc.tensor.matmul(ptr0[:, 8 * kt:8 * kt + 8],
                                 hnat0[:, 128 * kt:128 * (kt + 1)],
                                 identb[:BL, :BL], is_transpose=True,
                                 skip_group_check=True)
            hT = wbig.tile([128, ET, BL], GDT, tag="h0T")
            nc.vector.tensor_copy(_r(hT[:].rearrange("p a b -> p (a b)")),
                                  ptr0[:, :8 * ET])

            # prefetch DMAs for phases C/D/E: deprioritized so they only
            # fill DMA slots the W/A loads are not using
            with tc.high_priority(offset=-100000):
                W0 = persist.tile([128, ET, D], BF, tag="W0")
                nc.gpsimd.dma_start(W0[:], chunked(la_w[0], nc))
                W1 = persist.tile([128, ET, D], BF, tag="W1")
                nc.gpsimd.dma_start(W1[:], chunked(la_w[1], nc))
                W2 = persist.tile([128, ET, D], BF, tag="W2")
                nc.gpsimd.dma_start(W2[:], chunked(la_w[2], nc))
                W3 = persist.tile([128, ET, D], BF, tag="W3")
                nc.gpsimd.dma_start(W3[:], chunked(la_w[3], nc))
                f1 = persist.tile([128, 12, 1024], BF, tag="f1")
                nc.gpsimd.dma_start(f1[:], f1_w.rearrange("(c p) n -> p c n", p=128))
                f2 = persist.tile([128, 8, 512], BF, tag="f2")
                nc.gpsimd.dma_start(f2[:], f2_w.rearrange("(c p) n -> p c n", p=128))
                f3 = persist.tile([128, 4, 1024], BF, tag="f3")
                nc.gpsimd.dma_start(f3[:], f3_w.rearrange("(c p) n -> p c n", p=128))
                load_Xn(0)

            KSTEPS = int(os.environ.get("KSTEPS", str(T)))
            KHALF = os.environ.get("KHALF", "1") == "1"
            HB = BL // 2
            wxTn3 = wxTn[:]
            if not KHALF:
                for t in range(KSTEPS):
                    psR = psD.tile([128, ET * BL], FP, tag="gr")
                    psZ = psD.tile([128, ET * BL], FP, tag="gz")
                    psN = psD.tile([128, ET * BL], FP, tag="gn")

                    def gate_chunk(ps, mi, m):
                        j0 = 128 * m
                        for kt in range(ET):
                            nc.tensor.matmul(ps[:, BL * mi:BL * (mi + 1)],
                                             _r(WT[:, kt, j0:j0 + 128]),
                                             _r(hT[:, kt, :]),
                                             start=(kt == 0), stop=False,
                                             skip_group_check=True)
                        if m < 12:
                            nc.tensor.matmul(ps[:, BL * mi:BL * (mi + 1)],
                                             wxb[:, j0:j0 + 128],
                                             identb[:BL, :BL], start=False,
                                             stop=True, skip_group_check=True)
                        else:
                            nc.tensor.matmul(ps[:, BL * mi:BL * (mi + 1)],
                                             bhhr_bf[:, j0 - 2 * D:j0 - 2 * D + 128],
                                             ones1b[:1, :BL],
                                             start=False, stop=True,
                                             skip_group_check=True)

                    for mi in range(ET):
                        gate_chunk(psR, mi, mi)
                    for mi in range(ET):
                        gate_chunk(psN, mi, 12 + mi)
                    for mi in range(ET):
                        gate_chunk(psZ, mi, 6 + mi)

                    # h_new = (1-z)*n + z*h ; z-products run in tanh's shadow
                    rsig = g1.tile([128, ET * BL], FP, tag="rsig")
                    nc.scalar.activation(rsig[:], psR[:], AF.Sigmoid)
                    zsig = g1.tile([128, ET * BL], FP, tag="zsig")
                    nc.scalar.activation(zsig[:], psZ[:], AF.Sigmoid)
                    rwn = g1.tile([128, ET * BL], FP, tag="rwn")
                    nc.vector.tensor_mul(rwn[:], rsig[:], psN[:])
                    npre = g1.tile([128, ET * BL], FP, tag="npre")
                    nc.vector.tensor_add(npre[:], rwn[:],
                                         wxTn[:].rearrange("p a b -> p (a b)"))
                    nt_ = g1.tile([128, ET * BL], FP, tag="nt")
                    nc.scalar.activation(nt_[:], npre[:], AF.Tanh)
                    zh = g1.tile([128, ET * BL], FP, tag="zh")
                    nc.vector.tensor_mul(zh[:], zsig[:],
                                         hT[:].rearrange("p a b -> p (a b)"))
                    omz = g1.tile([128, ET * BL], FP, tag="omz")
                    nc.vector.tensor_scalar(omz[:], zsig[:], -1.0, 1.0,
                                            OP.mult, OP.add)
                    ozn = g1.tile([128, ET * BL], FP, tag="ozn")
                    nc.vector.tensor_mul(ozn[:], omz[:], nt_[:])
                    hT = g1.tile([128, ET, BL], GDT, tag="hT")
                    nc.vector.tensor_add(_r(hT[:].rearrange("p a b -> p (a b)")),
                                         ozn[:], zh[:])
                    nc.scalar.copy(qembT[:, :, :, t].rearrange("p a b -> p (a b)"),
                                   hT[:].rearrange("p a b -> p (a b)"))
            else:
                # two independent half-batch chains, interleaved so each
                # half's elementwise hides in the other's latency
                hTs = [None, None]
                psmap = [(psD, "gr"), (psD, "gz"), (psA, "wh0"), (psA, "wh1"),
                         (psD, "gn"), (psC, "pd")]
                for t in range(KSTEPS):
                    def hprev(g, kt):
                        if t == 0:
                            return _r(hT[:, kt, HB * g:HB * (g + 1)])
                        return _r(hTs[g][:, kt, :])

                    def ps_half(i):
                        pl, tg = psmap[i]
                        ph_t = pl.tile([128, ET * HB], FP, tag=tg)
                        return ph_t

                    psRs = [ps_half(0), ps_half(1)]
                    psZs = [ps_half(2), ps_half(3)]
                    psNs = [ps_half(4), ps_half(5)]

                    def gate_chunk2(ps, mi, m, g):
                        j0 = 128 * m
                        for kt in range(ET):
                            nc.tensor.matmul(ps[:, HB * mi:HB * (mi + 1)],
                                             _r(WT[:, kt, j0:j0 + 128]),
                                             hprev(g, kt),
                                             start=(kt == 0), stop=False,
                                             skip_group_check=True)
                        if m < 12:
                            nc.tensor.matmul(ps[:, HB * mi:HB * (mi + 1)],
                                             wxb[:, j0:j0 + 128],
                                             identb[:BL, HB * g:HB * (g + 1)],
                                             start=False, stop=True,
                                             skip_group_check=True)
                        else:
                            nc.tensor.matmul(ps[:, HB * mi:HB * (mi + 1)],
                                             bhhr_bf[:, j0 - 2 * D:j0 - 2 * D + 128],
                                             ones1b[:1, :HB],
                                             start=False, stop=True,
                                             skip_group_check=True)

                    for g in (0, 1):
                        for mi in range(ET):
                            gate_chunk2(psRs[g], mi, mi, g)
                    for g in (0, 1):
                        for mi in range(ET):
                            gate_chunk2(psNs[g], mi, 12 + mi, g)
                    for g in (0, 1):
                        for mi in range(ET):
                            gate_chunk2(psZs[g], mi, 6 + mi, g)

                    def tile3(tag):
                        t3 = g1.tile([128, ET, HB], FP, tag=tag)
                        return t3

                    rsig = [tile3("rsig0"), tile3("rsig1")]
                    zsig = [tile3("zsig0"), tile3("zsig1")]
                    rwn = [tile3("rwn0"), tile3("rwn1")]
                    npre = [tile3("npre0"), tile3("npre1")]
                    nt_ = [tile3("nt0"), tile3("nt1")]
                    zh = [tile3("zh0"), tile3("zh1")]
                    omz = [tile3("omz0"), tile3("omz1")]
                    ozn = [tile3("ozn0"), tile3("ozn1")]
                    def tile3g(tag):
                        t3g = g1.tile([128, ET, HB], GDT, tag=tag)
                        return t3g

                    hnew = [tile3g("hTn0"), tile3g("hTn1")]
                    for g in (0, 1):
                        nc.scalar.activation(rsig[g][:].rearrange("p a b -> p (a b)"),
                                             psRs[g][:], AF.Sigmoid)
                    for g in (0, 1):
                        nc.scalar.activation(zsig[g][:].rearrange("p a b -> p (a b)"),
                                             psZs[g][:], AF.Sigmoid)
                    for g in (0, 1):
                        nc.vector.tensor_mul(rwn[g][:].rearrange("p a b -> p (a b)"),
                                             rsig[g][:].rearrange("p a b -> p (a b)"),
                                             psNs[g][:])
                    for g in (0, 1):
                        nc.vector.tensor_add(npre[g][:], rwn[g][:],
                                             wxTn3[:, :, HB * g:HB * (g + 1)])
                    for g in (0, 1):
                        nc.scalar.activation(nt_[g][:].rearrange("p a b -> p (a b)"),
                                             npre[g][:].rearrange("p a b -> p (a b)"),
                                             AF.Tanh)
                    for g in (0, 1):
                        hp = (hT[:, :, HB * g:HB * (g + 1)] if t == 0
                              else hTs[g][:])
                        nc.vector.tensor_mul(zh[g][:], zsig[g][:], hp)
                    for g in (0, 1):
                        nc.vector.tensor_scalar(omz[g][:].rearrange("p a b -> p (a b)"),
                                                zsig[g][:].rearrange("p a b -> p (a b)"),
                                                -1.0, 1.0, OP.mult, OP.add)
                    for g in (0, 1):
                        nc.vector.tensor_mul(ozn[g][:], omz[g][:], nt_[g][:])
                    for g in (0, 1):
                        nc.vector.tensor_add(_r(hnew[g][:]), ozn[g][:], zh[g][:])
                    for g in (0, 1):
                        nc.scalar.copy(qembT[:, :, HB * g:HB * (g + 1), t],
                                       hnew[g][:])
                    hTs = hnew
            load_Xn(1)

        # ================= phase C: Q^T, W1^T, Qt^T =======================
        with tc.tile_pool(name="prep", bufs=1) as prep:
            QT = prep.tile([128, ET, RQ], BF, tag="QT")
            qflat = qembT[:].rearrange("p a b t -> p a (b t)")
            for mt in range(ET):
                p = pgroup(mt)
                for kt in range(ET):
                    nc.tensor.matmul(p[:], W0[:, kt, 128 * mt:128 * (mt + 1)],
                                     qflat[:, kt, :], start=(kt == 0),
                                     stop=(kt == ET - 1))
                nc.vector.tensor_scalar(QT[:, mt, :], p[:], b0laT[:, mt:mt + 1],
                                        None, OP.add)
            W1T = prep.tile([128, ET, D], BF, tag="W1T")
            for hd in range(ET):
                for grp in range(2):
                    pt2 = psB.tile([128, 512], BF, tag="ptw")
                    for i in range(3):
                        e2 = grp * 3 + i
                        nc.tensor.matmul(pt2[:, 128 * i:128 * (i + 1)],
                                         W1[:, e2, 128 * hd:128 * (hd + 1)],
                                         identb[:], is_transpose=True,
                                         skip_group_check=True)
                    if grp == 0:
                        nc.vector.tensor_copy(W1T[:, hd, 0:384], pt2[:, 0:384])
                    else:
                        nc.scalar.copy(W1T[:, hd, 384:768], pt2[:, 0:384])
            scl = 1.0 / float(np.sqrt(DK))
            for h in range(NH):
                for mt in range(ET):
                    p = pgroup(h * ET + mt)
                    for i in range(3):
                        kt = h * 3 + i
                        nc.tensor.matmul(p[:], W1T[:, kt, 128 * mt:128 * (mt + 1)],
                                         QT[:, kt, :], start=(i == 0), stop=(i == 2))
                    dst = QtT[:, mt, :].rearrange("p (b h2 t) -> p b h2 t",
                                                  h2=NH, t=T)[:, :, h, :]
                    if (h * ET + mt) % 3 != 2:
                        nc.vector.tensor_scalar(dst, p[:], scl, None, OP.mult)
                    else:
                        nc.scalar.activation(dst, p[:], AF.Copy, scale=scl)

        # ================= phase D: per-b attention =======================
        with tc.tile_pool(name="ab", bufs=2) as ab, \
             tc.tile_pool(name="xbp2", bufs=1) as xb2_pool:
            xb2[0] = xb2_pool
            for b in range(BL):
                for bn in (b + 1, b + 2):
                    if bn < BL and bn not in XnMap:
                        load_Xn(bn)
                Xn = XnMap.pop(b)
                XT = ab.tile([128, ET, NKC * 128], BF, tag="XT")
                for et in range(ET):
                    if et % 3 == 2:
                        pt = psD.tile([128, 1024], BF, tag="gn")
                    else:
                        pt = psB.tile([128, 1024], BF, tag="ptw")
                    for c in range(NKC):
                        nc.tensor.matmul(pt[:, 128 * c:128 * (c + 1)],
                                         Xn[:, c, 128 * et:128 * (et + 1)],
                                         identb[:], is_transpose=True,
                                         skip_group_check=True)
                    if et != 4:
                        nc.vector.tensor_copy(XT[:, et, :NK], pt[:, :NK])
                    else:
                        nc.scalar.copy(XT[:, et, :NK], pt[:, :NK])
                att = ab.tile([64, NKC * 128], BF, tag="att")
                nc.vector.memset(att[:, NK:], 0.0)
                zacc = ab.tile([64, 2], FP, tag="zacc")
                for ci, (n0, nw) in enumerate(CH_NK):
                    p = psA.tile([64, 512], FP, tag=f"wh{ci}")
                    for kt in range(ET):
                        nc.tensor.matmul(p[:, :nw],
                                         QtT[:, kt, b * 2 * T:(b + 1) * 2 * T],
                                         XT[:, kt, n0:n0 + nw],
                                         start=(kt == 0), stop=(kt == ET - 1))
                    nc.scalar.activation(att[:, n0:n0 + nw], p[:, :nw], AF.Exp,
                                         accum_out=zacc[:, ci:ci + 1])
                zs = ab.tile([64, 1], FP, tag="zs")
                nc.vector.tensor_add(zs[:], zacc[:, 0:1], zacc[:, 1:2])
                rz = ab.tile([64, 1], FP, tag="rz1")
                nc.vector.reciprocal(rz[:], zs[:])
                wm = ab.tile([64, 2], BF, tag="wm")
                nc.vector.tensor_scalar(wm[:], pmask[:], rz[:, 0:1], None, OP.mult)
                # paT[k, i] = sum_q att[q, k] * wm[q, i]  (no transposes!)
                pp = psD.tile([128, 2 * NKC], FP, tag="gr")
                for c in range(NKC):
                    nc.tensor.matmul(pp[:, 2 * c:2 * c + 2],
                                     att[:, 128 * c:128 * (c + 1)], wm[:],
                                     start=True, stop=True,
                                     skip_group_check=True)
                paT = ab.tile([128, NKC, 2], BF, tag="paT")
                nc.vector.tensor_copy(paT[:].rearrange("p a b -> p (a b)"), pp[:])
                # ctxT[d, i] = sum_k Xn[k, d] * paT[k, i]
                pc = psD.tile([128, 2 * ET], FP, tag="gz")
                for dc in range(ET):
                    for c in range(NKC):
                        nc.tensor.matmul(pc[:, 2 * dc:2 * dc + 2],
                                         Xn[:, c, 128 * dc:128 * (dc + 1)],
                                         paT[:, c, :], start=(c == 0),
                                         stop=(c == NKC - 1),
                                         skip_group_check=True)
                nc.vector.tensor_copy(
                    pcxT2[:, :, :, b].rearrange("p a b -> p (a b)"), pc[:])

        # ================= phase E: projections + MLP =====================
        with tc.tile_pool(name="tail", bufs=1) as tail:
            vconT = tail.tile([128, ET], FP, tag="vconT")
            for mt in range(ET):
                p = pgroup(mt)
                for kt in range(ET):
                    nc.tensor.matmul(p[:, 0:1], W3[:, kt, 128 * mt:128 * (mt + 1)],
                                     b2laT_bf[:, kt:kt + 1], start=(kt == 0),
                                     stop=(kt == ET - 1), skip_group_check=True)
                nc.vector.tensor_scalar(vconT[:, mt:mt + 1], p[:, 0:1],
                                        b3laT[:, mt:mt + 1], Sla[:, 0:1],
                                        OP.add, OP.mult)
            pctxT = tail.tile([128, ET, BL], BF, tag="pctxT")
            for h in range(NH):
                for mi in range(3):
                    mt = h * 3 + mi
                    p = pgroup(mt)
                    for kt in range(ET):
                        nc.tensor.matmul(p[:, 0:BL],
                                         W2[:, kt, 128 * mt:128 * (mt + 1)],
                                         pcxT2[:, kt, h, :], start=(kt == 0),
                                         stop=(kt == ET - 1),
                                         skip_group_check=True)
                    nc.vector.tensor_copy(pctxT[:, mt, :], p[:, 0:BL])
            loT = tail.tile([128, ET, BL], BF, tag="loT")
            for mt in range(ET):
                p = pgroup(mt)
                for kt in range(ET):
                    nc.tensor.matmul(p[:, 0:BL], W3[:, kt, 128 * mt:128 * (mt + 1)],
                                     pctxT[:, kt, :], start=(kt == 0),
                                     stop=(kt == ET - 1), skip_group_check=True)
                nc.vector.tensor_scalar(loT[:, mt, :], p[:, 0:BL],
                                        vconT[:, mt:mt + 1], None, OP.add)

            y1T = tail.tile([128, 8, BL], BF, tag="y1T")
            for mt in range(8):
                p = pgroup(mt)
                for i, kt in enumerate(list(range(ET, 12)) + list(range(ET))):
                    r_ = loT[:, kt, :] if kt < ET else goutT[:, kt - ET, :]
                    nc.tensor.matmul(p[:, 0:BL], f1[:, kt, 128 * mt:128 * (mt + 1)],
                                     r_, start=(i == 0), stop=(i == 11),
                                     skip_group_check=True)
                nc.vector.tensor_scalar(y1T[:, mt, :], p[:, 0:BL],
                                        b1fT[:, mt:mt + 1], None, OP.add)
            y2T = tail.tile([128, 4, BL], BF, tag="y2T")
            for mt in range(4):
                p = pgroup(mt)
                for kt in range(8):
                    nc.tensor.matmul(p[:, 0:BL], f2[:, kt, 128 * mt:128 * (mt + 1)],
                                     y1T[:, kt, :], start=(kt == 0), stop=(kt == 7),
                                     skip_group_check=True)
                nc.scalar.activation(y2T[:, mt, :], p[:, 0:BL], AF.Relu,
                                     bias=b2fT[:, mt:mt + 1])
            yT = tail.tile([128, 8, BL], FP, tag="yT")
            for mt in range(8):
                p = pgroup(mt)
                for kt in range(4):
                    nc.tensor.matmul(p[:, 0:BL], f3[:, kt, 128 * mt:128 * (mt + 1)],
                                     y2T[:, kt, :], start=(kt == 0), stop=(kt == 3),
                                     skip_group_check=True)
                nc.vector.tensor_scalar(yT[:, mt, :], p[:, 0:BL],
                                        b3fT[:, mt:mt + 1], None, OP.add)
            ynat = tail.tile([BL, 1024], FP, tag="ynat")
            for g in range(2):
                po = psB.tile([128, 512], FP, tag="ptw")
                for i in range(4):
                    mt = g * 4 + i
                    nc.tensor.matmul(po[:BL, 128 * i:128 * (i + 1)], yT[:, mt, :],
                                     ident[:128, :128], is_transpose=True,
                                     skip_group_check=True)
                nc.vector.tensor_copy(ynat[:, 512 * g:512 * (g + 1)], po[:BL, :])
            nc.sync.dma_start(out_d[:, :], ynat[:])

    nc.compile()
    return nc


_NC = None


def kernel(**inputs):
    global _NC
    if _NC is None:
        _NC = build()
    B = inputs["image_local_embeds"].shape[0]
    per = B // NCORES
    in_maps = []
    for c in range(NCORES):
        sl = slice(c * per, (c + 1) * per)
        m = {
            "img": np.ascontiguousarray(np.asarray(inputs["image_local_embeds"])[sl], dtype=np.float32),
            "h0": np.ascontiguousarray(np.asarray(inputs["h0"])[sl], dtype=np.float32),
        }
        for k in ["gru_w_ih", "gru_w_hh", "gru_b_ih", "gru_b_hh", "ga_w", "ga_b",
                  "ga_pool", "la_w", "la_b", "la_pool", "go_w", "go_b", "go_pool",
                  "f1_w", "f1_b", "f2_w", "f2_b", "f3_w", "f3_b"]:
            m[k] = np.ascontiguousarray(np.asarray(inputs[k], dtype=np.float32))
        in_maps.append(m)
    res = run_bass_kernel_spmd(_NC, in_maps, core_ids=list(range(NCORES)))
    return np.concatenate([res.results[c]["out"] for c in range(NCORES)], axis=0)


# revision 75
# speedup vs baseline: 1.0978x; 1.0015x over previous
"""Trainium2 Bass kernel for nn_BiVision_VQA2 (B=64,T=32,D=768,N=901).

Data-parallel over batch: 8 batch elems per core x 8 cores.
Key math simplifications (validated vs reference, rel err ~1e-4):
  - ga/go attention use a single key token -> softmax==1 -> those paths are
    linear in cls; question_embeds is mathematically unused.
  - GRU input `a` is constant over time; wx computed once.
  - local attention: row-constant score terms drop out of softmax; query
    pooling applied to the attention matrix before the @X contraction.
Performance structure:
  - GRU computed in TRANSPOSED gate layout [128(gate row), batch] via
    weight-stationary matmuls (moving N=8), elementwise on [128, 48].
  - bf16 everywhere outside the GRU recurrence (DMA casts on load).
  - phase D: transpose-free paT/ctxT via natural-operand-stationary matmuls.
"""

import os
import numpy as np
from contextlib import ExitStack

import concourse.bass as bass
import concourse.tile as tile
from concourse import bacc, mybir
from concourse.bass_utils import run_bass_kernel_spmd
from concourse.masks import make_identity

FP = mybir.dt.float32
FPR = mybir.dt.float32r
OP = mybir.AluOpType
AF = mybir.ActivationFunctionType
BF = mybir.dt.bfloat16

NCORES = 8
BL = 8
D = 768
T = 32
G = 3 * D
NK = 900
NH = 2
DK = 384
ET = D // 128
RQ = BL * T

CH_G = [(0, 512), (512, 512), (1024, 512), (1536, 512), (2048, 256)]
CH_NK = [(0, 512), (512, 388)]

GBF = os.environ.get("KGRUBF", "1") == "1"
GDT = BF if GBF else FP


def _r(ap):
    return ap if GBF else ap.bitcast(FPR)


from contextlib import contextmanager


@contextmanager
def _nullcm():
    yield


def kchunks(n):
    out, o = [], 0
    while o < n:
        out.append((o, min(128, n - o)))
        o += 128
    return out


def build():
    nc = bacc.Bacc("TRN2", target_bir_lowering=False, debug=False,
                   enable_asserts=False)

    img = nc.dram_tensor("img", [BL, 901, D], FP, kind="ExternalInput").ap()
    h0 = nc.dram_tensor("h0", [BL, D], FP, kind="ExternalInput").ap()
    w_ih = nc.dram_tensor("gru_w_ih", [G, D], FP, kind="ExternalInput").ap()
    w_hh = nc.dram_tensor("gru_w_hh", [G, D], FP, kind="ExternalInput").ap()
    b_ih = nc.dram_tensor("gru_b_ih", [G], FP, kind="ExternalInput").ap()
    b_hh = nc.dram_tensor("gru_b_hh", [G], FP, kind="ExternalInput").ap()
    ga_w = nc.dram_tensor("ga_w", [4, D, D], FP, kind="ExternalInput").ap()
    ga_b = nc.dram_tensor("ga_b", [4, D], FP, kind="ExternalInput").ap()
    ga_pool = nc.dram_tensor("ga_pool", [1], FP, kind="ExternalInput").ap()
    la_w = nc.dram_tensor("la_w", [4, D, D], FP, kind="ExternalInput").ap()
    la_b = nc.dram_tensor("la_b", [4, D], FP, kind="ExternalInput").ap()
    la_pool = nc.dram_tensor("la_pool", [T], FP, kind="ExternalInput").ap()
    go_w = nc.dram_tensor("go_w", [4, D, D], FP, kind="ExternalInput").ap()
    go_b = nc.dram_tensor("go_b", [4, D], FP, kind="ExternalInput").ap()
    go_pool = nc.dram_tensor("go_pool", [T], FP, kind="ExternalInput").ap()
    f1_w = nc.dram_tensor("f1_w", [2 * D, 1024], FP, kind="ExternalInput").ap()
    f1_b = nc.dram_tensor("f1_b", [1024], FP, kind="ExternalInput").ap()
    f2_w = nc.dram_tensor("f2_w", [1024, 512], FP, kind="ExternalInput").ap()
    f2_b = nc.dram_tensor("f2_b", [512], FP, kind="ExternalInput").ap()
    f3_w = nc.dram_tensor("f3_w", [512, 1024], FP, kind="ExternalInput").ap()
    f3_b = nc.dram_tensor("f3_b", [1024], FP, kind="ExternalInput").ap()
    out_d = nc.dram_tensor("out", [BL, 1024], FP, kind="ExternalOutput").ap()

    def chunked(dram2d, nc_, cw=D):
        # [R, cw] dram viewed as [128, R//128, cw]
        return dram2d.rearrange("(c p) d -> p c d", p=128)

    with tile.TileContext(nc) as tc, ExitStack() as ctx:
        cpool = ctx.enter_context(tc.tile_pool(name="const", bufs=1))
        persist = ctx.enter_context(tc.tile_pool(name="persist", bufs=1))
        xb = ctx.enter_context(tc.tile_pool(name="xb", bufs=2))
        psA = ctx.enter_context(tc.tile_pool(name="psA", bufs=1, space="PSUM"))
        psB = ctx.enter_context(tc.tile_pool(name="psB", bufs=2, space="PSUM"))
        psC = ctx.enter_context(tc.tile_pool(name="psC", bufs=1, space="PSUM"))
        psD = ctx.enter_context(tc.tile_pool(name="psD", bufs=1, space="PSUM"))

        ident = cpool.tile([128, 128], FP, tag="ident")
        make_identity(nc, ident[:])
        identb = cpool.tile([128, 128], BF, tag="identb")
        nc.vector.tensor_copy(identb[:], ident[:])
        ones1 = cpool.tile([1, 128], FP, tag="ones1")
        nc.vector.memset(ones1[:], 1.0)
        ones1b = cpool.tile([1, 128], BF, tag="ones1b")
        nc.vector.memset(ones1b[:], 1.0)
        onesT = cpool.tile([T, 128], FP, tag="onesT")
        nc.vector.memset(onesT[:], 1.0)

        # ---- small bias vectors -> column layout via K=1 matmuls ---------
        def colvec_batch(specs):
            # pipelined: all row-loads first (3 rotating staging slots),
            # then K=1 matmuls into one psum tile, then copies out.
            pdvl = psC.tile([128, 64], FP, tag="pd")
            off = 0
            outs = []
            for idx, (src, n) in enumerate(specs):
                nt = n // 128
                vr = cpool.tile([1, 1024], FP, tag=f"vrow{idx % 2}")
                nc.sync.dma_start(vr[:, :n], src[:][None, :])
                for c in range(nt):
                    nc.tensor.matmul(pdvl[:, off + c:off + c + 1],
                                     vr[0:1, 128 * c:128 * (c + 1)],
                                     ones1[:1, :1], start=True, stop=True,
                                     skip_group_check=True)
                outs.append((off, nt))
                off += nt
            return pdvl, outs

        def colvec_out(pdvl, o_nt, tag):
            o, nt = o_nt
            t_ = cpool.tile([128, nt], FP, tag=tag)
            nc.vector.tensor_copy(t_[:], pdvl[:, o:o + nt])
            return t_

        pdv1, offs1 = colvec_batch([(ga_b[2], D), (ga_b[3], D),
                                    (go_b[2], D), (go_b[3], D),
                                    (la_b[0], D), (la_b[2], D),
                                    (la_b[3], D), (f1_b, 1024),
                                    (f2_b, 512), (f3_b, 1024)])
        b2gaT = colvec_out(pdv1, offs1[0], "b2gaT")
        b3gaT = colvec_out(pdv1, offs1[1], "b3gaT")
        b2goT = colvec_out(pdv1, offs1[2], "b2goT")
        b3goT = colvec_out(pdv1, offs1[3], "b3goT")
        b0laT = colvec_out(pdv1, offs1[4], "b0laT")
        b2laT = colvec_out(pdv1, offs1[5], "b2laT")
        b3laT = colvec_out(pdv1, offs1[6], "b3laT")
        b1fT = colvec_out(pdv1, offs1[7], "b1fT")
        b2fT = colvec_out(pdv1, offs1[8], "b2fT")
        b3fT = colvec_out(pdv1, offs1[9], "b3fT")
        b2laT_bf = cpool.tile([128, ET], BF, tag="b2laT_bf")
        nc.vector.tensor_copy(b2laT_bf[:], b2laT[:])

        lapool_c = cpool.tile([T, 1], FP, tag="lapool_c")
        nc.sync.dma_start(lapool_c[:], la_pool[:][:, None])
        gopool_c = cpool.tile([T, 1], FP, tag="gopool_c")
        nc.sync.dma_start(gopool_c[:], go_pool[:][:, None])
        gapool_c = cpool.tile([1, 1], FP, tag="gapool_c")
        nc.sync.dma_start(gapool_c[:], ga_pool[:][:, None])

        def sum_bcast(vcol, k, tag):
            p = psC.tile([128, 64], FP, tag="pd")
            lhs = onesT if k == T else ones1
            nc.tensor.matmul(p[:, 0:1], lhs[:k, :], vcol[:k, :], start=True,
                             stop=True, skip_group_check=True)
            s = cpool.tile([128, 1], FP, tag=tag)
            nc.vector.tensor_copy(s[:], p[:, 0:1])
            return s

        Sla = sum_bcast(lapool_c, T, "Sla")
        Sgo = sum_bcast(gopool_c, T, "Sgo")
        Sga = sum_bcast(gapool_c, 1, "Sga")

        pmask = cpool.tile([64, 2], FP, tag="pmask")
        nc.vector.memset(pmask[:], 0.0)
        nc.sync.dma_start(pmask[0:T, 0:1], la_pool[:][:, None])
        nc.sync.dma_start(pmask[T:2 * T, 1:2], la_pool[:][:, None])

        # img patch loads (streamed; b0/b1 prefetched early)
        KC = kchunks(NK)
        NKC = len(KC)
        XnMap = {}

        xb2 = [None]

        def load_Xn(b):
            # 3-way buffer rotation: xb holds 2, xb2 (opened for phase D,
            # reusing SBUF freed by the GRU pools) holds the third
            pool = xb2[0] if (b % 3 == 2 and xb2[0] is not None) else xb
            Xn = pool.tile([128, NKC, D], BF, tag="Xn")
            if b < 3:
                # zero the pad rows once per physical buffer (b0,b1 -> xb's
                # two buffers, b2 -> xb2); later b's reuse a buffer and only
                # ever rewrite rows 0..kwl of the last chunk
                nc.vector.memset(Xn[:, NKC - 1, :], 0.0)
            nc.gpsimd.dma_start(
                Xn[:, 0:NKC - 1, :],
                img[b, 1:1 + 128 * (NKC - 1), :].rearrange(
                    "(c p) d -> p c d", p=128))
            k0l, kwl = KC[-1]
            nc.gpsimd.dma_start(Xn[:kwl, NKC - 1, :],
                                img[b, 1 + k0l:1 + k0l + kwl, :])
            XnMap[b] = Xn

        def pgroup(i, ncols=RQ):
            pl, tg = [(psC, "pd"), (psD, "gr"), (psD, "gz"), (psD, "gn")][i % 4]
            pg_t = pl.tile([128, ncols], FP, tag=tg)
            return pg_t

        # persistent outputs of the phases
        qembT = cpool.tile([128, ET, BL, T], BF, tag="qembT")
        wxb = cpool.tile([BL, G], BF, tag="wxb")
        QtT = persist.tile([128, ET, NH * RQ], BF, tag="QtT")
        goutT = cpool.tile([128, ET, BL], BF, tag="goutT")
        aT = cpool.tile([128, ET, BL], GDT, tag="aT")
        pcxT2 = persist.tile([128, ET, NH, BL], BF, tag="pcxT2")

        # ================= phase B: GRU ===================================
        with tc.tile_pool(name="wbig", bufs=1) as wbig, \
             tc.tile_pool(name="wnat", bufs=2) as wnat, \
             tc.tile_pool(name="wst", bufs=3) as wst, \
             tc.tile_pool(name="g1", bufs=2) as g1:
            combr = wbig.tile([1, G], BF, tag="combr")
            nc.gpsimd.dma_start(combr[:], b_ih[:][None, :])
            bhhrow = wbig.tile([1, G], BF, tag="bhhrow")
            nc.gpsimd.dma_start(bhhrow[:], b_hh[:][None, :])
            nc.vector.tensor_add(combr[:, 0:2 * D], combr[:, 0:2 * D],
                                 bhhrow[:, 0:2 * D])
            bhhr_bf = bhhrow[:, 2 * D:3 * D]

            WT = wbig.tile([128, ET, G], GDT, tag="WT")
            tident = identb if GBF else ident

            def build_WT(w_dram, dma_prio=0):
                jts = kchunks(G)
                for g0 in range(0, len(jts), 5):
                    grp = jts[g0:g0 + 5]
                    ng = len(grp)
                    wn = wst.tile([128, 5, D], GDT, tag="wn")
                    src = w_dram[grp[0][0]:grp[-1][0] + grp[-1][1], :]
                    src = src.rearrange("(c p) d -> p c d", p=128)
                    with tc.high_priority(offset=dma_prio if dma_prio else None) \
                            if dma_prio else _nullcm():
                        if GBF:
                            nc.gpsimd.dma_start(wn[:, :ng, :], src)
                        else:
                            nc.sync.dma_start(wn[:, :ng, :], src)
                    sub = 5 if GBF else 3
                    for et in range(ET):
                        for s0 in range(0, ng, sub):
                            sg = min(sub, ng - s0)
                            pt = psB.tile([128, 128 * sub], GDT, tag="ptw")
                            for i in range(sg):
                                nc.tensor.matmul(pt[:, 128 * i:128 * (i + 1)],
                                                 wn[:, s0 + i, 128 * et:128 * (et + 1)],
                                                 tident[:], is_transpose=True,
                                                 skip_group_check=True)
                            w0 = grp[0][0] + 128 * s0
                            wlen = 128 * sg
                            if (et + s0) % 2 == 0:
                                nc.vector.tensor_copy(_r(WT[:, et, w0:w0 + wlen]),
                                                      pt[:, :wlen])
                            else:
                                nc.scalar.copy(_r(WT[:, et, w0:w0 + wlen]),
                                               pt[:, :wlen])

            build_WT(w_ih)

            # ---- phase A part 1 (cls -> a), interleaved after W_ih ------
            clsn = wbig.tile([BL, D], BF, tag="clsn")
            nc.gpsimd.dma_start(clsn[:], img[0:BL, 0, :])
            wA2 = wnat.tile([128, ET, D], BF, tag="wa")
            nc.gpsimd.dma_start(wA2[:], chunked(ga_w[2], nc))
            wA3 = wnat.tile([128, ET, D], BF, tag="wa")
            nc.gpsimd.dma_start(wA3[:], chunked(ga_w[3], nc))
            ptr = psB.tile([128, 512], BF, tag="ptw")
            for kt in range(ET):
                nc.tensor.matmul(ptr[:, 8 * kt:8 * kt + 8],
                                 clsn[:, 128 * kt:128 * (kt + 1)],
                                 identb[:BL, :BL], is_transpose=True,
                                 skip_group_check=True)
            clsT = wbig.tile([128, ET, BL], BF, tag="clsT")
            nc.vector.tensor_copy(clsT[:].rearrange("p a b -> p (a b)"),
                                  ptr[:, :8 * ET])

            def dense_T(wsb, rhsT, biasT, scaleT, otile, out_r=False):
                for mt in range(ET):
                    p = psC.tile([128, BL], FP, tag="pd")
                    for kt in range(ET):
                        nc.tensor.matmul(p[:], wsb[:, kt, 128 * mt:128 * (mt + 1)],
                                         rhsT[:, kt, :], start=(kt == 0),
                                         stop=(kt == ET - 1))
                    dst = otile[:, mt, :]
                    if out_r:
                        dst = _r(dst)
                    if scaleT is None:
                        nc.vector.tensor_scalar(dst, p[:], biasT[:, mt:mt + 1],
                                                None, OP.add)
                    else:
                        nc.vector.tensor_scalar(dst, p[:], biasT[:, mt:mt + 1],
                                                scaleT[:, 0:1], OP.add, OP.mult)

            A2T = wbig.tile([128, ET, BL], BF, tag="A2T")
            dense_T(wA2, clsT, b2gaT, None, A2T)
            dense_T(wA3, A2T, b3gaT, Sga, aT, out_r=not GBF)

            for (j0, jw) in CH_G:
                p = psA.tile([BL, 512], FP, tag="wh0")
                for kt in range(ET):
                    nc.tensor.matmul(p[:, :jw], aT[:, kt, :] if GBF else _r(aT[:, kt, :]),
                                     _r(WT[:, kt, j0:j0 + jw]),
                                     start=(kt == 0), stop=False)
                nc.tensor.matmul(p[:, :jw], ones1b[:1, :BL],
                                 combr[:, j0:j0 + jw], start=False, stop=True)
                nc.vector.tensor_copy(wxb[:, j0:j0 + jw], p[:, :jw])

            build_WT(w_hh)

            # ---- phase A part 2 (gout path) — loads emitted here, the
            # dense compute happens inside the GRU loop (idle engine slack)
            wG2 = wnat.tile([128, ET, D], BF, tag="wa")
            nc.gpsimd.dma_start(wG2[:], chunked(go_w[2], nc))
            wG3 = wnat.tile([128, ET, D], BF, tag="wa")
            nc.gpsimd.dma_start(wG3[:], chunked(go_w[3], nc))
            G2T = wbig.tile([128, ET, BL], BF, tag="G2T")

            def dense_T_ptw(wsb, rhsT, biasT, scaleT, otile):
                # dense_T variant staged in the ptw banks (free during GRU)
                for mt in range(ET):
                    p = psB.tile([128, BL], FP, tag="ptw")
                    for kt in range(ET):
                        nc.tensor.matmul(p[:], wsb[:, kt, 128 * mt:128 * (mt + 1)],
                                         rhsT[:, kt, :], start=(kt == 0),
                                         stop=(kt == ET - 1))
                    if scaleT is None:
                        nc.vector.tensor_scalar(otile[:, mt, :], p[:],
                                                biasT[:, mt:mt + 1], None, OP.add)
                    else:
                        nc.vector.tensor_scalar(otile[:, mt, :], p[:],
                                                biasT[:, mt:mt + 1],
                                                scaleT[:, 0:1], OP.add, OP.mult)

            # transposed constant wx for the n-gate: [128, ET, BL]
            ptx = psC.tile([128, 64], BF, tag="pd")
            for kt in range(ET):
                nc.tensor.matmul(ptx[:, 8 * kt:8 * kt + 8],
                                 wxb[:, 2 * D + 128 * kt:2 * D + 128 * (kt + 1)],
                                 identb[:BL, :BL], is_transpose=True,
                                 skip_group_check=True)
            wxTn = wbig.tile([128, ET, BL], FP, tag="wxTn")
            nc.vector.tensor_copy(wxTn[:].rearrange("p a b -> p (a b)"),
                                  ptx[:, :8 * ET])

            # initial h0 transposed
            hnat0 = wbig.tile([BL, D], BF, tag="hnat0")
            nc.gpsimd.dma_start(hnat0[:], h0[:, :])
            ptr0 = psC.tile([128, 64], BF, tag="pd")
            for kt in range(ET):
                nc.tensor.matmul(ptr0[:, 8 * kt:8 * kt + 8],
                                 hnat0[:, 128 * kt:128 * (kt + 1)],
                                 identb[:BL, :BL], is_transpose=True,
                                 skip_group_check=True)
            hT = wbig.tile([128, ET, BL], GDT, tag="h0T")
            nc.vector.tensor_copy(_r(hT[:].rearrange("p a b -> p (a b)")),
                                  ptr0[:, :8 * ET])

            # prefetch DMAs for phases C/D/E: deprioritized so they only
            # fill DMA slots the W/A loads are not using
            with tc.high_priority(offset=-100000):
                W0 = persist.tile([128, ET, D], BF, tag="W0")
                nc.gpsimd.dma_start(W0[:], chunked(la_w[0], nc))
                W1 = persist.tile([128, ET, D], BF, tag="W1")
                nc.gpsimd.dma_start(W1[:], chunked(la_w[1], nc))
                W2 = persist.tile([128, ET, D], BF, tag="W2")
                nc.gpsimd.dma_start(W2[:], chunked(la_w[2], nc))
                W3 = persist.tile([128, ET, D], BF, tag="W3")
                nc.gpsimd.dma_start(W3[:], chunked(la_w[3], nc))
                f1 = persist.tile([128, 12, 1024], BF, tag="f1")
                nc.gpsimd.dma_start(f1[:], f1_w.rearrange("(c p) n -> p c n", p=128))
                f2 = persist.tile([128, 8, 512], BF, tag="f2")
                nc.gpsimd.dma_start(f2[:], f2_w.rearrange("(c p) n -> p c n", p=128))
                f3 = persist.tile([128, 4, 1024], BF, tag="f3")
                nc.gpsimd.dma_start(f3[:], f3_w.rearrange("(c p) n -> p c n", p=128))
                load_Xn(0)

            KSTEPS = int(os.environ.get("KSTEPS", str(T)))
            KHALF = os.environ.get("KHALF", "1") == "1"
            HB = BL // 2
            wxTn3 = wxTn[:]
            if not KHALF:
                for t in range(KSTEPS):
                    psR = psD.tile([128, ET * BL], FP, tag="gr")
                    psZ = psD.tile([128, ET * BL], FP, tag="gz")
                    psN = psD.tile([128, ET * BL], FP, tag="gn")

                    def gate_chunk(ps, mi, m):
                        j0 = 128 * m
                        for kt in range(ET):
                            nc.tensor.matmul(ps[:, BL * mi:BL * (mi + 1)],
                                             _r(WT[:, kt, j0:j0 + 128]),
                                             _r(hT[:, kt, :]),
                                             start=(kt == 0), stop=False,
                                             skip_group_check=True)
                        if m < 12:
                            nc.tensor.matmul(ps[:, BL * mi:BL * (mi + 1)],
                                             wxb[:, j0:j0 + 128],
                                             identb[:BL, :BL], start=False,
                                             stop=True, skip_group_check=True)
                        else:
                            nc.tensor.matmul(ps[:, BL * mi:BL * (mi + 1)],
                                             bhhr_bf[:, j0 - 2 * D:j0 - 2 * D + 128],
                                             ones1b[:1, :BL],
                                             start=False, stop=True,
                                             skip_group_check=True)

                    for mi in range(ET):
                        gate_chunk(psR, mi, mi)
                    for mi in range(ET):
                        gate_chunk(psN, mi, 12 + mi)
                    for mi in range(ET):
                        gate_chunk(psZ, mi, 6 + mi)

                    # h_new = (1-z)*n + z*h ; z-products run in tanh's shadow
                    rsig = g1.tile([128, ET * BL], FP, tag="rsig")
                    nc.scalar.activation(rsig[:], psR[:], AF.Sigmoid)
                    zsig = g1.tile([128, ET * BL], FP, tag="zsig")
                    nc.scalar.activation(zsig[:], psZ[:], AF.Sigmoid)
                    rwn = g1.tile([128, ET * BL], FP, tag="rwn")
                    nc.vector.tensor_mul(rwn[:], rsig[:], psN[:])
                    npre = g1.tile([128, ET * BL], FP, tag="npre")
                    nc.vector.tensor_add(npre[:], rwn[:],
                                         wxTn[:].rearrange("p a b -> p (a b)"))
                    nt_ = g1.tile([128, ET * BL], FP, tag="nt")
                    nc.scalar.activation(nt_[:], npre[:], AF.Tanh)
                    zh = g1.tile([128, ET * BL], FP, tag="zh")
                    nc.vector.tensor_mul(zh[:], zsig[:],
                                         hT[:].rearrange("p a b -> p (a b)"))
                    omz = g1.tile([128, ET * BL], FP, tag="omz")
                    nc.vector.tensor_scalar(omz[:], zsig[:], -1.0, 1.0,
                                            OP.mult, OP.add)
                    ozn = g1.tile([128, ET * BL], FP, tag="ozn")
                    nc.vector.tensor_mul(ozn[:], omz[:], nt_[:])
                    hT = g1.tile([128, ET, BL], GDT, tag="hT")
                    nc.vector.tensor_add(_r(hT[:].rearrange("p a b -> p (a b)")),
                                         ozn[:], zh[:])
                    nc.scalar.copy(qembT[:, :, :, t].rearrange("p a b -> p (a b)"),
                                   hT[:].rearrange("p a b -> p (a b)"))
            else:
                # two independent half-batch chains, interleaved so each
                # half's elementwise hides in the other's latency
                hTs = [None, None]
                psmap = [(psD, "gr"), (psD, "gz"), (psA, "wh0"), (psA, "wh1"),
                         (psD, "gn"), (psC, "pd")]
                for t in range(KSTEPS):
                    def hprev(g, kt):
                        if t == 0:
                            return _r(hT[:, kt, HB * g:HB * (g + 1)])
                        return _r(hTs[g][:, kt, :])

                    def ps_half(i):
                        pl, tg = psmap[i]
                        ph_t = pl.tile([128, ET * HB], FP, tag=tg)
                        return ph_t

                    psRs = [ps_half(0), ps_half(1)]
                    psZs = [ps_half(2), ps_half(3)]
                    psNs = [ps_half(4), ps_half(5)]

                    def gate_chunk2(ps, mi, m, g):
                        j0 = 128 * m
                        for kt in range(ET):
                            nc.tensor.matmul(ps[:, HB * mi:HB * (mi + 1)],
                                             _r(WT[:, kt, j0:j0 + 128]),
                                             hprev(g, kt),
                                             start=(kt == 0), stop=False,
                                             skip_group_check=True)
                        if m < 12:
                            nc.tensor.matmul(ps[:, HB * mi:HB * (mi + 1)],
                                             wxb[:, j0:j0 + 128],
                                             identb[:BL, HB * g:HB * (g + 1)],
                                             start=False, stop=True,
                                             skip_group_check=True)
                        else:
                            nc.tensor.matmul(ps[:, HB * mi:HB * (mi + 1)],
                                             bhhr_bf[:, j0 - 2 * D:j0 - 2 * D + 128],
                                             ones1b[:1, :HB],
                                             start=False, stop=True,
                                             skip_group_check=True)

                    for g in (0, 1):
                        for mi in range(ET):
                            gate_chunk2(psRs[g], mi, mi, g)
                    for g in (0, 1):
                        for mi in range(ET):
                            gate_chunk2(psNs[g], mi, 12 + mi, g)
                    for g in (0, 1):
                        for mi in range(ET):
                            gate_chunk2(psZs[g], mi, 6 + mi, g)

                    def tile3(tag):
                        t3 = g1.tile([128, ET, HB], FP, tag=tag)
                        return t3

                    rsig = [tile3("rsig0"), tile3("rsig1")]
                    zsig = [tile3("zsig0"), tile3("zsig1")]
                    rwn = [tile3("rwn0"), tile3("rwn1")]
                    npre = [tile3("npre0"), tile3("npre1")]
                    nt_ = [tile3("nt0"), tile3("nt1")]
                    zh = [tile3("zh0"), tile3("zh1")]
                    omz = [tile3("omz0"), tile3("omz1")]
                    ozn = [tile3("ozn0"), tile3("ozn1")]
                    def tile3g(tag):
                        t3g = g1.tile([128, ET, HB], GDT, tag=tag)
                        return t3g

                    hnew = [tile3g("hTn0"), tile3g("hTn1")]
                    for g in (0, 1):
                        nc.scalar.activation(rsig[g][:].rearrange("p a b -> p (a b)"),
                                             psRs[g][:], AF.Sigmoid)
                    for g in (0, 1):
                        nc.scalar.activation(zsig[g][:].rearrange("p a b -> p (a b)"),
                                             psZs[g][:], AF.Sigmoid)
                    for g in (0, 1):
                        nc.vector.tensor_mul(rwn[g][:].rearrange("p a b -> p (a b)"),
                                             rsig[g][:].rearrange("p a b -> p (a b)"),
                                             psNs[g][:])
                    for g in (0, 1):
                        nc.vector.tensor_add(npre[g][:], rwn[g][:],
                                             wxTn3[:, :, HB * g:HB * (g + 1)])
                    for g in (0, 1):
                        nc.scalar.activation(nt_[g][:].rearrange("p a b -> p (a b)"),
                                             npre[g][:].rearrange("p a b -> p (a b)"),
                                             AF.Tanh)
                    for g in (0, 1):
                        hp = (hT[:, :, HB * g:HB * (g + 1)] if t == 0
                              else hTs[g][:])
                        nc.vector.tensor_mul(zh[g][:], zsig[g][:], hp)
                    for g in (0, 1):
                        nc.vector.tensor_scalar(omz[g][:].rearrange("p a b -> p (a b)"),
                                                zsig[g][:].rearrange("p a b -> p (a b)"),
                                                -1.0, 1.0, OP.mult, OP.add)
                    for g in (0, 1):
                        nc.vector.tensor_mul(ozn[g][:], omz[g][:], nt_[g][:])
                    for g in (0, 1):
                        nc.vector.tensor_add(_r(hnew[g][:]), ozn[g][:], zh[g][:])
                    for g in (0, 1):
                        nc.scalar.copy(qembT[:, :, HB * g:HB * (g + 1), t],
                                       hnew[g][:])
                    hTs = hnew
                    if t == 2:
                        dense_T_ptw(wG2, clsT, b2goT, None, G2T)
                    if t == 4:
                        dense_T_ptw(wG3, G2T, b3goT, Sgo, goutT)
            load_Xn(1)

        # ================= phase C: Q^T, W1^T, Qt^T =======================
        with tc.tile_pool(name="prep", bufs=1) as prep:
            QT = prep.tile([128, ET, RQ], BF, tag="QT")
            qflat = qembT[:].rearrange("p a b t -> p a (b t)")
            for mt in range(ET):
                p = pgroup(mt)
                for kt in range(ET):
                    nc.tensor.matmul(p[:], W0[:, kt, 128 * mt:128 * (mt + 1)],
                                     qflat[:, kt, :], start=(kt == 0),
                                     stop=(kt == ET - 1))
                nc.vector.tensor_scalar(QT[:, mt, :], p[:], b0laT[:, mt:mt + 1],
                                        None, OP.add)
            W1T = prep.tile([128, ET, D], BF, tag="W1T")
            for hd in range(ET):
                for grp in range(2):
                    pt2 = psB.tile([128, 512], BF, tag="ptw")
                    for i in range(3):
                        e2 = grp * 3 + i
                        nc.tensor.matmul(pt2[:, 128 * i:128 * (i + 1)],
                                         W1[:, e2, 128 * hd:128 * (hd + 1)],
                                         identb[:], is_transpose=True,
                                         skip_group_check=True)
                    if grp == 0:
                        nc.vector.tensor_copy(W1T[:, hd, 0:384], pt2[:, 0:384])
                    else:
                        nc.scalar.copy(W1T[:, hd, 384:768], pt2[:, 0:384])
            scl = 1.0 / float(np.sqrt(DK))
            for h in range(NH):
                for mt in range(ET):
                    p = pgroup(h * ET + mt)
                    for i in range(3):
                        kt = h * 3 + i
                        nc.tensor.matmul(p[:], W1T[:, kt, 128 * mt:128 * (mt + 1)],
                                         QT[:, kt, :], start=(i == 0), stop=(i == 2))
                    dst = QtT[:, mt, :].rearrange("p (b h2 t) -> p b h2 t",
                                                  h2=NH, t=T)[:, :, h, :]
                    if (h * ET + mt) % 3 != 2:
                        nc.vector.tensor_scalar(dst, p[:], scl, None, OP.mult)
                    else:
                        nc.scalar.activation(dst, p[:], AF.Copy, scale=scl)

        # ================= phase D: per-b attention =======================
        with tc.tile_pool(name="ab", bufs=2) as ab, \
             tc.tile_pool(name="xbp2", bufs=1) as xb2_pool:
            xb2[0] = xb2_pool
            for b in range(BL):
                for bn in (b + 1, b + 2):
                    if bn < BL and bn not in XnMap:
                        load_Xn(bn)
                Xn = XnMap.pop(b)
                XT = ab.tile([128, ET, NKC * 128], BF, tag="XT")
                for et in range(ET):
                    if et % 3 == 2:
                        pt = psD.tile([128, 1024], BF, tag="gn")
                    else:
                        pt = psB.tile([128, 1024], BF, tag="ptw")
                    for c in range(NKC):
                        nc.tensor.matmul(pt[:, 128 * c:128 * (c + 1)],
                                         Xn[:, c, 128 * et:128 * (et + 1)],
                                         identb[:], is_transpose=True,
                                         skip_group_check=True)
                    if et != 4:
                        nc.vector.tensor_copy(XT[:, et, :NK], pt[:, :NK])
                    else:
                        nc.scalar.copy(XT[:, et, :NK], pt[:, :NK])
                att = ab.tile([64, NKC * 128], BF, tag="att")
                nc.vector.memset(att[:, NK:], 0.0)
                zacc = ab.tile([64, 2], FP, tag="zacc")
                for ci, (n0, nw) in enumerate(CH_NK):
                    p = psA.tile([64, 512], FP, tag=f"wh{ci}")
                    for kt in range(ET):
                        nc.tensor.matmul(p[:, :nw],
                                         QtT[:, kt, b * 2 * T:(b + 1) * 2 * T],
                                         XT[:, kt, n0:n0 + nw],
                                         start=(kt == 0), stop=(kt == ET - 1))
                    nc.scalar.activation(att[:, n0:n0 + nw], p[:, :nw], AF.Exp,
                                         accum_out=zacc[:, ci:ci + 1])
                zs = ab.tile([64, 1], FP, tag="zs")
                nc.vector.tensor_add(zs[:], zacc[:, 0:1], zacc[:, 1:2])
                rz = ab.tile([64, 1], FP, tag="rz1")
                nc.vector.reciprocal(rz[:], zs[:])
                wm = ab.tile([64, 2], BF, tag="wm")
                nc.vector.tensor_scalar(wm[:], pmask[:], rz[:, 0:1], None, OP.mult)
                # paT[k, i] = sum_q att[q, k] * wm[q, i]  (no transposes!)
                pp = psD.tile([128, 2 * NKC], FP, tag="gr")
                for c in range(NKC):
                    nc.tensor.matmul(pp[:, 2 * c:2 * c + 2],
                                     att[:, 128 * c:128 * (c + 1)], wm[:],
                                     start=True, stop=True,
                                     skip_group_check=True)
                paT = ab.tile([128, NKC, 2], BF, tag="paT")
                nc.vector.tensor_copy(paT[:].rearrange("p a b -> p (a b)"), pp[:])
                # ctxT[d, i] = sum_k Xn[k, d] * paT[k, i]
                pc = psD.tile([128, 2 * ET], FP, tag="gz")
                for dc in range(ET):
                    for c in range(NKC):
                        nc.tensor.matmul(pc[:, 2 * dc:2 * dc + 2],
                                         Xn[:, c, 128 * dc:128 * (dc + 1)],
                                         paT[:, c, :], start=(c == 0),
                                         stop=(c == NKC - 1),
                                         skip_group_check=True)
                nc.vector.tensor_copy(
                    pcxT2[:, :, :, b].rearrange("p a b -> p (a b)"), pc[:])

        # ================= phase E: projections + MLP =====================
        with tc.tile_pool(name="tail", bufs=1) as tail:
            vconT = tail.tile([128, ET], FP, tag="vconT")
            for mt in range(ET):
                p = pgroup(mt)
                for kt in range(ET):
                    nc.tensor.matmul(p[:, 0:1], W3[:, kt, 128 * mt:128 * (mt + 1)],
                                     b2laT_bf[:, kt:kt + 1], start=(kt == 0),
                                     stop=(kt == ET - 1), skip_group_check=True)
                nc.vector.tensor_scalar(vconT[:, mt:mt + 1], p[:, 0:1],
                                        b3laT[:, mt:mt + 1], Sla[:, 0:1],
                                        OP.add, OP.mult)
            pctxT = tail.tile([128, ET, BL], BF, tag="pctxT")
            for h in range(NH):
                for mi in range(3):
                    mt = h * 3 + mi
                    p = pgroup(mt)
                    for kt in range(ET):
                        nc.tensor.matmul(p[:, 0:BL],
                                         W2[:, kt, 128 * mt:128 * (mt + 1)],
                                         pcxT2[:, kt, h, :], start=(kt == 0),
                                         stop=(kt == ET - 1),
                                         skip_group_check=True)
                    nc.vector.tensor_copy(pctxT[:, mt, :], p[:, 0:BL])
            loT = tail.tile([128, ET, BL], BF, tag="loT")
            for mt in range(ET):
                p = pgroup(mt)
                for kt in range(ET):
                    nc.tensor.matmul(p[:, 0:BL], W3[:, kt, 128 * mt:128 * (mt + 1)],
                                     pctxT[:, kt, :], start=(kt == 0),
                                     stop=(kt == ET - 1), skip_group_check=True)
                nc.vector.tensor_scalar(loT[:, mt, :], p[:, 0:BL],
                                        vconT[:, mt:mt + 1], None, OP.add)

            y1T = tail.tile([128, 8, BL], BF, tag="y1T")
            for mt in range(8):
                p = pgroup(mt)
                for i, kt in enumerate(list(range(ET, 12)) + list(range(ET))):
                    r_ = loT[:, kt, :] if kt < ET else goutT[:, kt - ET, :]
                    nc.tensor.matmul(p[:, 0:BL], f1[:, kt, 128 * mt:128 * (mt + 1)],
                                     r_, start=(i == 0), stop=(i == 11),
                                     skip_group_check=True)
                nc.vector.tensor_scalar(y1T[:, mt, :], p[:, 0:BL],
                                        b1fT[:, mt:mt + 1], None, OP.add)
            y2T = tail.tile([128, 4, BL], BF, tag="y2T")
            for mt in range(4):
                p = pgroup(mt)
                for kt in range(8):
                    nc.tensor.matmul(p[:, 0:BL], f2[:, kt, 128 * mt:128 * (mt + 1)],
                                     y1T[:, kt, :], start=(kt == 0), stop=(kt == 7),
                                     skip_group_check=True)
                nc.scalar.activation(y2T[:, mt, :], p[:, 0:BL], AF.Relu,
                                     bias=b2fT[:, mt:mt + 1])
            yT = tail.tile([128, 8, BL], FP, tag="yT")
            for mt in range(8):
                p = pgroup(mt)
                for kt in range(4):
                    nc.tensor.matmul(p[:, 0:BL], f3[:, kt, 128 * mt:128 * (mt + 1)],
                                     y2T[:, kt, :], start=(kt == 0), stop=(kt == 3),
                                     skip_group_check=True)
                nc.vector.tensor_scalar(yT[:, mt, :], p[:, 0:BL],
                                        b3fT[:, mt:mt + 1], None, OP.add)
            ynat = tail.tile([BL, 1024], FP, tag="ynat")
            for g in range(2):
                po = psB.tile([128, 512], FP, tag="ptw")
                for i in range(4):
                    mt = g * 4 + i
                    nc.tensor.matmul(po[:BL, 128 * i:128 * (i + 1)], yT[:, mt, :],
                                     ident[:128, :128], is_transpose=True,
                                     skip_group_check=True)
                nc.vector.tensor_copy(ynat[:, 512 * g:512 * (g + 1)], po[:BL, :])
            nc.sync.dma_start(out_d[:, :], ynat[:])

    nc.compile()
    return nc


_NC = None


def kernel(**inputs):
    global _NC
    if _NC is None:
        _NC = build()
    B = inputs["image_local_embeds"].shape[0]
    per = B // NCORES
    in_maps = []
    for c in range(NCORES):
        sl = slice(c * per, (c + 1) * per)
        m = {
            "img": np.ascontiguousarray(np.asarray(inputs["image_local_embeds"])[sl], dtype=np.float32),
            "h0": np.ascontiguousarray(np.asarray(inputs["h0"])[sl], dtype=np.float32),
        }
        for k in ["gru_w_ih", "gru_w_hh", "gru_b_ih", "gru_b_hh", "ga_w", "ga_b",
                  "ga_pool", "la_w", "la_b", "la_pool", "go_w", "go_b", "go_pool",
                  "f1_w", "f1_b", "f2_w", "f2_b", "f3_w", "f3_b"]:
            m[k] = np.ascontiguousarray(np.asarray(inputs[k], dtype=np.float32))
        in_maps.append(m)
    res = run_bass_kernel_spmd(_NC, in_maps, core_ids=list(range(NCORES)))
    return np.concatenate([res.results[c]["out"] for c in range(NCORES)], axis=0)


# revision 77
# speedup vs baseline: 1.1241x; 1.0239x over previous
"""Trainium2 Bass kernel for nn_BiVision_VQA2 (B=64,T=32,D=768,N=901).

Data-parallel over batch: 8 batch elems per core x 8 cores.
Key math simplifications (validated vs reference, rel err ~1e-4):
  - ga/go attention use a single key token -> softmax==1 -> those paths are
    linear in cls; question_embeds is mathematically unused.
  - GRU input `a` is constant over time; wx computed once.
  - local attention: row-constant score terms drop out of softmax; query
    pooling applied to the attention matrix before the @X contraction.
Performance structure:
  - GRU computed in TRANSPOSED gate layout [128(gate row), batch] via
    weight-stationary matmuls (moving N=8), elementwise on [128, 48].
  - bf16 everywhere outside the GRU recurrence (DMA casts on load).
  - phase D: transpose-free paT/ctxT via natural-operand-stationary matmuls.
"""

import os
import numpy as np
from contextlib import ExitStack

import concourse.bass as bass
import concourse.tile as tile
from concourse import bacc, mybir
from concourse.bass_utils import run_bass_kernel_spmd
from concourse.masks import make_identity

FP = mybir.dt.float32
FPR = mybir.dt.float32r
OP = mybir.AluOpType
AF = mybir.ActivationFunctionType
BF = mybir.dt.bfloat16

NCORES = 8
BL = 8
D = 768
T = 32
G = 3 * D
NK = 900
NH = 2
DK = 384
ET = D // 128
RQ = BL * T

CH_G = [(0, 512), (512, 512), (1024, 512), (1536, 512), (2048, 256)]
CH_NK = [(0, 512), (512, 388)]

GBF = os.environ.get("KGRUBF", "1") == "1"
GDT = BF if GBF else FP


def _r(ap):
    return ap if GBF else ap.bitcast(FPR)


from contextlib import contextmanager


@contextmanager
def _nullcm():
    yield


def kchunks(n):
    out, o = [], 0
    while o < n:
        out.append((o, min(128, n - o)))
        o += 128
    return out


def build():
    nc = bacc.Bacc("TRN2", target_bir_lowering=False, debug=False,
                   enable_asserts=False)

    img = nc.dram_tensor("img", [BL, 901, D], FP, kind="ExternalInput").ap()
    h0 = nc.dram_tensor("h0", [BL, D], FP, kind="ExternalInput").ap()
    w_ih = nc.dram_tensor("gru_w_ih", [G, D], FP, kind="ExternalInput").ap()
    w_hh = nc.dram_tensor("gru_w_hh", [G, D], FP, kind="ExternalInput").ap()
    b_ih = nc.dram_tensor("gru_b_ih", [G], FP, kind="ExternalInput").ap()
    b_hh = nc.dram_tensor("gru_b_hh", [G], FP, kind="ExternalInput").ap()
    ga_w = nc.dram_tensor("ga_w", [4, D, D], FP, kind="ExternalInput").ap()
    ga_b = nc.dram_tensor("ga_b", [4, D], FP, kind="ExternalInput").ap()
    ga_pool = nc.dram_tensor("ga_pool", [1], FP, kind="ExternalInput").ap()
    la_w = nc.dram_tensor("la_w", [4, D, D], FP, kind="ExternalInput").ap()
    la_b = nc.dram_tensor("la_b", [4, D], FP, kind="ExternalInput").ap()
    la_pool = nc.dram_tensor("la_pool", [T], FP, kind="ExternalInput").ap()
    go_w = nc.dram_tensor("go_w", [4, D, D], FP, kind="ExternalInput").ap()
    go_b = nc.dram_tensor("go_b", [4, D], FP, kind="ExternalInput").ap()
    go_pool = nc.dram_tensor("go_pool", [T], FP, kind="ExternalInput").ap()
    f1_w = nc.dram_tensor("f1_w", [2 * D, 1024], FP, kind="ExternalInput").ap()
    f1_b = nc.dram_tensor("f1_b", [1024], FP, kind="ExternalInput").ap()
    f2_w = nc.dram_tensor("f2_w", [1024, 512], FP, kind="ExternalInput").ap()
    f2_b = nc.dram_tensor("f2_b", [512], FP, kind="ExternalInput").ap()
    f3_w = nc.dram_tensor("f3_w", [512, 1024], FP, kind="ExternalInput").ap()
    f3_b = nc.dram_tensor("f3_b", [1024], FP, kind="ExternalInput").ap()
    out_d = nc.dram_tensor("out", [BL, 1024], FP, kind="ExternalOutput").ap()

    def chunked(dram2d, nc_, cw=D):
        # [R, cw] dram viewed as [128, R//128, cw]
        return dram2d.rearrange("(c p) d -> p c d", p=128)

    with tile.TileContext(nc) as tc, ExitStack() as ctx:
        cpool = ctx.enter_context(tc.tile_pool(name="const", bufs=1))
        persist = ctx.enter_context(tc.tile_pool(name="persist", bufs=1))
        xb = ctx.enter_context(tc.tile_pool(name="xb", bufs=2))
        psA = ctx.enter_context(tc.tile_pool(name="psA", bufs=1, space="PSUM"))
        psB = ctx.enter_context(tc.tile_pool(name="psB", bufs=2, space="PSUM"))
        psC = ctx.enter_context(tc.tile_pool(name="psC", bufs=1, space="PSUM"))
        psD = ctx.enter_context(tc.tile_pool(name="psD", bufs=1, space="PSUM"))

        ident = cpool.tile([128, 128], FP, tag="ident")
        make_identity(nc, ident[:])
        identb = cpool.tile([128, 128], BF, tag="identb")
        nc.vector.tensor_copy(identb[:], ident[:])
        ones1 = cpool.tile([1, 128], FP, tag="ones1")
        nc.vector.memset(ones1[:], 1.0)
        ones1b = cpool.tile([1, 128], BF, tag="ones1b")
        nc.vector.memset(ones1b[:], 1.0)
        onesT = cpool.tile([T, 128], FP, tag="onesT")
        nc.vector.memset(onesT[:], 1.0)

        # ---- small bias vectors -> column layout via K=1 matmuls ---------
        def colvec_batch(specs):
            # pipelined: all row-loads first (3 rotating staging slots),
            # then K=1 matmuls into one psum tile, then copies out.
            pdvl = psC.tile([128, 64], FP, tag="pd")
            off = 0
            outs = []
            for idx, (src, n) in enumerate(specs):
                nt = n // 128
                vr = cpool.tile([1, 1024], FP, tag=f"vrow{idx % 2}")
                nc.sync.dma_start(vr[:, :n], src[:][None, :])
                for c in range(nt):
                    nc.tensor.matmul(pdvl[:, off + c:off + c + 1],
                                     vr[0:1, 128 * c:128 * (c + 1)],
                                     ones1[:1, :1], start=True, stop=True,
                                     skip_group_check=True)
                outs.append((off, nt))
                off += nt
            return pdvl, outs

        def colvec_out(pdvl, o_nt, tag):
            o, nt = o_nt
            t_ = cpool.tile([128, nt], FP, tag=tag)
            nc.vector.tensor_copy(t_[:], pdvl[:, o:o + nt])
            return t_

        pdv1, offs1 = colvec_batch([(ga_b[2], D), (ga_b[3], D),
                                    (go_b[2], D), (go_b[3], D),
                                    (la_b[0], D), (la_b[2], D),
                                    (la_b[3], D), (f1_b, 1024),
                                    (f2_b, 512), (f3_b, 1024)])
        b2gaT = colvec_out(pdv1, offs1[0], "b2gaT")
        b3gaT = colvec_out(pdv1, offs1[1], "b3gaT")
        b2goT = colvec_out(pdv1, offs1[2], "b2goT")
        b3goT = colvec_out(pdv1, offs1[3], "b3goT")
        b0laT = colvec_out(pdv1, offs1[4], "b0laT")
        b2laT = colvec_out(pdv1, offs1[5], "b2laT")
        b3laT = colvec_out(pdv1, offs1[6], "b3laT")
        b1fT = colvec_out(pdv1, offs1[7], "b1fT")
        b2fT = colvec_out(pdv1, offs1[8], "b2fT")
        b3fT = colvec_out(pdv1, offs1[9], "b3fT")
        b2laT_bf = cpool.tile([128, ET], BF, tag="b2laT_bf")
        nc.vector.tensor_copy(b2laT_bf[:], b2laT[:])

        lapool_c = cpool.tile([T, 1], FP, tag="lapool_c")
        nc.sync.dma_start(lapool_c[:], la_pool[:][:, None])
        gopool_c = cpool.tile([T, 1], FP, tag="gopool_c")
        nc.sync.dma_start(gopool_c[:], go_pool[:][:, None])
        gapool_c = cpool.tile([1, 1], FP, tag="gapool_c")
        nc.sync.dma_start(gapool_c[:], ga_pool[:][:, None])

        def sum_bcast(vcol, k, tag):
            p = psC.tile([128, 64], FP, tag="pd")
            lhs = onesT if k == T else ones1
            nc.tensor.matmul(p[:, 0:1], lhs[:k, :], vcol[:k, :], start=True,
                             stop=True, skip_group_check=True)
            s = cpool.tile([128, 1], FP, tag=tag)
            nc.vector.tensor_copy(s[:], p[:, 0:1])
            return s

        Sla = sum_bcast(lapool_c, T, "Sla")
        Sgo = sum_bcast(gopool_c, T, "Sgo")
        Sga = sum_bcast(gapool_c, 1, "Sga")

        pmask = cpool.tile([64, 2], FP, tag="pmask")
        nc.vector.memset(pmask[:], 0.0)
        nc.sync.dma_start(pmask[0:T, 0:1], la_pool[:][:, None])
        nc.sync.dma_start(pmask[T:2 * T, 1:2], la_pool[:][:, None])

        # img patch loads (streamed; b0/b1 prefetched early)
        KC = kchunks(NK)
        NKC = len(KC)
        XnMap = {}

        xb2 = [None]

        def load_Xn(b):
            # 3-way buffer rotation: xb holds 2, xb2 (opened for phase D,
            # reusing SBUF freed by the GRU pools) holds the third
            pool = xb2[0] if (b % 3 == 2 and xb2[0] is not None) else xb
            Xn = pool.tile([128, NKC, D], BF, tag="Xn")
            if b < 3:
                # zero the pad rows once per physical buffer (b0,b1 -> xb's
                # two buffers, b2 -> xb2); later b's reuse a buffer and only
                # ever rewrite rows 0..kwl of the last chunk
                nc.vector.memset(Xn[:, NKC - 1, :], 0.0)
            nc.gpsimd.dma_start(
                Xn[:, 0:NKC - 1, :],
                img[b, 1:1 + 128 * (NKC - 1), :].rearrange(
                    "(c p) d -> p c d", p=128))
            k0l, kwl = KC[-1]
            nc.gpsimd.dma_start(Xn[:kwl, NKC - 1, :],
                                img[b, 1 + k0l:1 + k0l + kwl, :])
            XnMap[b] = Xn

        def pgroup(i, ncols=RQ):
            pl, tg = [(psC, "pd"), (psD, "gr"), (psD, "gz"), (psD, "gn")][i % 4]
            pg_t = pl.tile([128, ncols], FP, tag=tg)
            return pg_t

        # persistent outputs of the phases
        qembT = cpool.tile([128, ET, BL, T], BF, tag="qembT")
        wxb = cpool.tile([BL, G], BF, tag="wxb")
        QtT = persist.tile([128, ET, NH * RQ], BF, tag="QtT")
        goutT = cpool.tile([128, ET, BL], BF, tag="goutT")
        aT = cpool.tile([128, ET, BL], GDT, tag="aT")
        pcxT2 = persist.tile([128, ET, NH, BL], BF, tag="pcxT2")

        # ================= phase B: GRU ===================================
        with tc.tile_pool(name="wbig", bufs=1) as wbig, \
             tc.tile_pool(name="wnat", bufs=2) as wnat, \
             tc.tile_pool(name="wst", bufs=3) as wst, \
             tc.tile_pool(name="g1", bufs=2) as g1:
            combr = wbig.tile([1, G], BF, tag="combr")
            nc.gpsimd.dma_start(combr[:], b_ih[:][None, :])
            bhhrow = wbig.tile([1, G], BF, tag="bhhrow")
            nc.gpsimd.dma_start(bhhrow[:], b_hh[:][None, :])
            nc.vector.tensor_add(combr[:, 0:2 * D], combr[:, 0:2 * D],
                                 bhhrow[:, 0:2 * D])
            bhhr_bf = bhhrow[:, 2 * D:3 * D]

            WT = wbig.tile([128, ET, G], GDT, tag="WT")
            tident = identb if GBF else ident

            def build_WT(w_dram, dma_prio=0):
                jts = kchunks(G)
                for g0 in range(0, len(jts), 5):
                    grp = jts[g0:g0 + 5]
                    ng = len(grp)
                    wn = wst.tile([128, 5, D], GDT, tag="wn")
                    src = w_dram[grp[0][0]:grp[-1][0] + grp[-1][1], :]
                    src = src.rearrange("(c p) d -> p c d", p=128)
                    with tc.high_priority(offset=dma_prio if dma_prio else None) \
                            if dma_prio else _nullcm():
                        if GBF:
                            nc.gpsimd.dma_start(wn[:, :ng, :], src)
                        else:
                            nc.sync.dma_start(wn[:, :ng, :], src)
                    sub = 5 if GBF else 3
                    for et in range(ET):
                        for s0 in range(0, ng, sub):
                            sg = min(sub, ng - s0)
                            pt = psB.tile([128, 128 * sub], GDT, tag="ptw")
                            for i in range(sg):
                                nc.tensor.matmul(pt[:, 128 * i:128 * (i + 1)],
                                                 wn[:, s0 + i, 128 * et:128 * (et + 1)],
                                                 tident[:], is_transpose=True,
                                                 skip_group_check=True)
                            w0 = grp[0][0] + 128 * s0
                            wlen = 128 * sg
                            if (et + s0) % 2 == 0:
                                nc.vector.tensor_copy(_r(WT[:, et, w0:w0 + wlen]),
                                                      pt[:, :wlen])
                            else:
                                nc.scalar.copy(_r(WT[:, et, w0:w0 + wlen]),
                                               pt[:, :wlen])

            build_WT(w_ih)

            # ---- phase A part 1 (cls -> a), interleaved after W_ih ------
            clsn = wbig.tile([BL, D], BF, tag="clsn")
            nc.gpsimd.dma_start(clsn[:], img[0:BL, 0, :])
            wA2 = wnat.tile([128, ET, D], BF, tag="wa")
            nc.gpsimd.dma_start(wA2[:], chunked(ga_w[2], nc))
            wA3 = wnat.tile([128, ET, D], BF, tag="wa")
            nc.gpsimd.dma_start(wA3[:], chunked(ga_w[3], nc))
            ptr = psB.tile([128, 512], BF, tag="ptw")
            for kt in range(ET):
                nc.tensor.matmul(ptr[:, 8 * kt:8 * kt + 8],
                                 clsn[:, 128 * kt:128 * (kt + 1)],
                                 identb[:BL, :BL], is_transpose=True,
                                 skip_group_check=True)
            clsT = wbig.tile([128, ET, BL], BF, tag="clsT")
            nc.vector.tensor_copy(clsT[:].rearrange("p a b -> p (a b)"),
                                  ptr[:, :8 * ET])

            def dense_T(wsb, rhsT, biasT, scaleT, otile, out_r=False):
                for mt in range(ET):
                    p = psC.tile([128, BL], FP, tag="pd")
                    for kt in range(ET):
                        nc.tensor.matmul(p[:], wsb[:, kt, 128 * mt:128 * (mt + 1)],
                                         rhsT[:, kt, :], start=(kt == 0),
                                         stop=(kt == ET - 1))
                    dst = otile[:, mt, :]
                    if out_r:
                        dst = _r(dst)
                    if scaleT is None:
                        nc.vector.tensor_scalar(dst, p[:], biasT[:, mt:mt + 1],
                                                None, OP.add)
                    else:
                        nc.vector.tensor_scalar(dst, p[:], biasT[:, mt:mt + 1],
                                                scaleT[:, 0:1], OP.add, OP.mult)

            A2T = wbig.tile([128, ET, BL], BF, tag="A2T")
            dense_T(wA2, clsT, b2gaT, None, A2T)
            dense_T(wA3, A2T, b3gaT, Sga, aT, out_r=not GBF)

            for (j0, jw) in CH_G:
                p = psA.tile([BL, 512], FP, tag="wh0")
                for kt in range(ET):
                    nc.tensor.matmul(p[:, :jw], aT[:, kt, :] if GBF else _r(aT[:, kt, :]),
                                     _r(WT[:, kt, j0:j0 + jw]),
                                     start=(kt == 0), stop=False)
                nc.tensor.matmul(p[:, :jw], ones1b[:1, :BL],
                                 combr[:, j0:j0 + jw], start=False, stop=True)
                nc.vector.tensor_copy(wxb[:, j0:j0 + jw], p[:, :jw])

            build_WT(w_hh)

            # ---- phase A part 2 (gout path) — loads emitted here, the
            # dense compute happens inside the GRU loop (idle engine slack)
            wG2 = wnat.tile([128, ET, D], BF, tag="wa")
            nc.gpsimd.dma_start(wG2[:], chunked(go_w[2], nc))
            wG3 = wnat.tile([128, ET, D], BF, tag="wa")
            nc.gpsimd.dma_start(wG3[:], chunked(go_w[3], nc))
            G2T = wbig.tile([128, ET, BL], BF, tag="G2T")

            def dense_T_ptw(wsb, rhsT, biasT, scaleT, otile):
                # dense_T variant staged in the ptw banks (free during GRU)
                for mt in range(ET):
                    p = psB.tile([128, BL], FP, tag="ptw")
                    for kt in range(ET):
                        nc.tensor.matmul(p[:], wsb[:, kt, 128 * mt:128 * (mt + 1)],
                                         rhsT[:, kt, :], start=(kt == 0),
                                         stop=(kt == ET - 1))
                    if scaleT is None:
                        nc.vector.tensor_scalar(otile[:, mt, :], p[:],
                                                biasT[:, mt:mt + 1], None, OP.add)
                    else:
                        nc.vector.tensor_scalar(otile[:, mt, :], p[:],
                                                biasT[:, mt:mt + 1],
                                                scaleT[:, 0:1], OP.add, OP.mult)

            # transposed constant wx for the n-gate: [128, ET, BL]
            ptx = psC.tile([128, 64], BF, tag="pd")
            for kt in range(ET):
                nc.tensor.matmul(ptx[:, 8 * kt:8 * kt + 8],
                                 wxb[:, 2 * D + 128 * kt:2 * D + 128 * (kt + 1)],
                                 identb[:BL, :BL], is_transpose=True,
                                 skip_group_check=True)
            wxTn = wbig.tile([128, ET, BL], FP, tag="wxTn")
            nc.vector.tensor_copy(wxTn[:].rearrange("p a b -> p (a b)"),
                                  ptx[:, :8 * ET])

            # initial h0 transposed
            hnat0 = wbig.tile([BL, D], BF, tag="hnat0")
            nc.gpsimd.dma_start(hnat0[:], h0[:, :])
            ptr0 = psC.tile([128, 64], BF, tag="pd")
            for kt in range(ET):
                nc.tensor.matmul(ptr0[:, 8 * kt:8 * kt + 8],
                                 hnat0[:, 128 * kt:128 * (kt + 1)],
                                 identb[:BL, :BL], is_transpose=True,
                                 skip_group_check=True)
            hT = wbig.tile([128, ET, BL], GDT, tag="h0T")
            nc.vector.tensor_copy(_r(hT[:].rearrange("p a b -> p (a b)")),
                                  ptr0[:, :8 * ET])

            # prefetch DMAs for phases C/D/E: deprioritized so they only
            # fill DMA slots the W/A loads are not using
            with tc.high_priority(offset=-100000):
                W0 = persist.tile([128, ET, D], BF, tag="W0")
                nc.gpsimd.dma_start(W0[:], chunked(la_w[0], nc))
                W1 = persist.tile([128, ET, D], BF, tag="W1")
                nc.gpsimd.dma_start(W1[:], chunked(la_w[1], nc))
                W2 = persist.tile([128, ET, D], BF, tag="W2")
                nc.gpsimd.dma_start(W2[:], chunked(la_w[2], nc))
                W3 = persist.tile([128, ET, D], BF, tag="W3")
                nc.gpsimd.dma_start(W3[:], chunked(la_w[3], nc))
                f1 = persist.tile([128, 12, 1024], BF, tag="f1")
                nc.gpsimd.dma_start(f1[:], f1_w.rearrange("(c p) n -> p c n", p=128))
                f2 = persist.tile([128, 8, 512], BF, tag="f2")
                nc.gpsimd.dma_start(f2[:], f2_w.rearrange("(c p) n -> p c n", p=128))
                f3 = persist.tile([128, 4, 1024], BF, tag="f3")
                nc.gpsimd.dma_start(f3[:], f3_w.rearrange("(c p) n -> p c n", p=128))
                load_Xn(0)

            KSTEPS = int(os.environ.get("KSTEPS", str(T)))
            KHALF = os.environ.get("KHALF", "1") == "1"
            HB = BL // 2
            wxTn3 = wxTn[:]
            if not KHALF:
                for t in range(KSTEPS):
                    psR = psD.tile([128, ET * BL], FP, tag="gr")
                    psZ = psD.tile([128, ET * BL], FP, tag="gz")
                    psN = psD.tile([128, ET * BL], FP, tag="gn")

                    def gate_chunk(ps, mi, m):
                        j0 = 128 * m
                        for kt in range(ET):
                            nc.tensor.matmul(ps[:, BL * mi:BL * (mi + 1)],
                                             _r(WT[:, kt, j0:j0 + 128]),
                                             _r(hT[:, kt, :]),
                                             start=(kt == 0), stop=False,
                                             skip_group_check=True)
                        if m < 12:
                            nc.tensor.matmul(ps[:, BL * mi:BL * (mi + 1)],
                                             wxb[:, j0:j0 + 128],
                                             identb[:BL, :BL], start=False,
                                             stop=True, skip_group_check=True)
                        else:
                            nc.tensor.matmul(ps[:, BL * mi:BL * (mi + 1)],
                                             bhhr_bf[:, j0 - 2 * D:j0 - 2 * D + 128],
                                             ones1b[:1, :BL],
                                             start=False, stop=True,
                                             skip_group_check=True)

                    for mi in range(ET):
                        gate_chunk(psR, mi, mi)
                    for mi in range(ET):
                        gate_chunk(psN, mi, 12 + mi)
                    for mi in range(ET):
                        gate_chunk(psZ, mi, 6 + mi)

                    # h_new = (1-z)*n + z*h ; z-products run in tanh's shadow
                    rsig = g1.tile([128, ET * BL], FP, tag="rsig")
                    nc.scalar.activation(rsig[:], psR[:], AF.Sigmoid)
                    zsig = g1.tile([128, ET * BL], FP, tag="zsig")
                    nc.scalar.activation(zsig[:], psZ[:], AF.Sigmoid)
                    rwn = g1.tile([128, ET * BL], FP, tag="rwn")
                    nc.vector.tensor_mul(rwn[:], rsig[:], psN[:])
                    npre = g1.tile([128, ET * BL], FP, tag="npre")
                    nc.vector.tensor_add(npre[:], rwn[:],
                                         wxTn[:].rearrange("p a b -> p (a b)"))
                    nt_ = g1.tile([128, ET * BL], FP, tag="nt")
                    nc.scalar.activation(nt_[:], npre[:], AF.Tanh)
                    zh = g1.tile([128, ET * BL], FP, tag="zh")
                    nc.vector.tensor_mul(zh[:], zsig[:],
                                         hT[:].rearrange("p a b -> p (a b)"))
                    omz = g1.tile([128, ET * BL], FP, tag="omz")
                    nc.vector.tensor_scalar(omz[:], zsig[:], -1.0, 1.0,
                                            OP.mult, OP.add)
                    ozn = g1.tile([128, ET * BL], FP, tag="ozn")
                    nc.vector.tensor_mul(ozn[:], omz[:], nt_[:])
                    hT = g1.tile([128, ET, BL], GDT, tag="hT")
                    nc.vector.tensor_add(_r(hT[:].rearrange("p a b -> p (a b)")),
                                         ozn[:], zh[:])
                    nc.scalar.copy(qembT[:, :, :, t].rearrange("p a b -> p (a b)"),
                                   hT[:].rearrange("p a b -> p (a b)"))
            else:
                # two independent half-batch chains, interleaved so each
                # half's elementwise hides in the other's latency
                hTs = [None, None]
                psmap = [(psD, "gr"), (psD, "gz"), (psA, "wh0"), (psA, "wh1"),
                         (psD, "gn"), (psC, "pd")]
                for t in range(KSTEPS):
                    def hprev(g, kt):
                        if t == 0:
                            return _r(hT[:, kt, HB * g:HB * (g + 1)])
                        return _r(hTs[g][:, kt, :])

                    def ps_half(i):
                        pl, tg = psmap[i]
                        ph_t = pl.tile([128, ET * HB], FP, tag=tg)
                        return ph_t

                    psRs = [ps_half(0), ps_half(1)]
                    psZs = [ps_half(2), ps_half(3)]
                    psNs = [ps_half(4), ps_half(5)]

                    def gate_const(ps, mi, m, g):
                        # constant (wx/bhh) opener: no h dependency, so it
                        # runs during the previous step's elementwise tail
                        j0 = 128 * m
                        if m < 12:
                            nc.tensor.matmul(ps[:, HB * mi:HB * (mi + 1)],
                                             wxb[:, j0:j0 + 128],
                                             identb[:BL, HB * g:HB * (g + 1)],
                                             start=True, stop=False,
                                             skip_group_check=True)
                        else:
                            nc.tensor.matmul(ps[:, HB * mi:HB * (mi + 1)],
                                             bhhr_bf[:, j0 - 2 * D:j0 - 2 * D + 128],
                                             ones1b[:1, :HB],
                                             start=True, stop=False,
                                             skip_group_check=True)

                    def gate_h(ps, mi, m, g):
                        j0 = 128 * m
                        for kt in range(ET):
                            nc.tensor.matmul(ps[:, HB * mi:HB * (mi + 1)],
                                             _r(WT[:, kt, j0:j0 + 128]),
                                             hprev(g, kt),
                                             start=False, stop=(kt == ET - 1),
                                             skip_group_check=True)

                    for ps_list, m0 in ((psRs, 0), (psNs, 12), (psZs, 6)):
                        for g in (0, 1):
                            for mi in range(ET):
                                gate_const(ps_list[g], mi, m0 + mi, g)
                    for ps_list, m0 in ((psRs, 0), (psNs, 12), (psZs, 6)):
                        for g in (0, 1):
                            for mi in range(ET):
                                gate_h(ps_list[g], mi, m0 + mi, g)

                    def tile3(tag):
                        t3 = g1.tile([128, ET, HB], FP, tag=tag)
                        return t3

                    rsig = [tile3("rsig0"), tile3("rsig1")]
                    zsig = [tile3("zsig0"), tile3("zsig1")]
                    rwn = [tile3("rwn0"), tile3("rwn1")]
                    npre = [tile3("npre0"), tile3("npre1")]
                    nt_ = [tile3("nt0"), tile3("nt1")]
                    zh = [tile3("zh0"), tile3("zh1")]
                    omz = [tile3("omz0"), tile3("omz1")]
                    ozn = [tile3("ozn0"), tile3("ozn1")]
                    def tile3g(tag):
                        t3g = g1.tile([128, ET, HB], GDT, tag=tag)
                        return t3g

                    hnew = [tile3g("hTn0"), tile3g("hTn1")]
                    for g in (0, 1):
                        nc.scalar.activation(rsig[g][:].rearrange("p a b -> p (a b)"),
                                             psRs[g][:], AF.Sigmoid)
                    for g in (0, 1):
                        nc.scalar.activation(zsig[g][:].rearrange("p a b -> p (a b)"),
                                             psZs[g][:], AF.Sigmoid)
                    for g in (0, 1):
                        nc.vector.tensor_mul(rwn[g][:].rearrange("p a b -> p (a b)"),
                                             rsig[g][:].rearrange("p a b -> p (a b)"),
                                             psNs[g][:])
                    for g in (0, 1):
                        nc.vector.tensor_add(npre[g][:], rwn[g][:],
                                             wxTn3[:, :, HB * g:HB * (g + 1)])
                    for g in (0, 1):
                        nc.scalar.activation(nt_[g][:].rearrange("p a b -> p (a b)"),
                                             npre[g][:].rearrange("p a b -> p (a b)"),
                                             AF.Tanh)
                    for g in (0, 1):
                        hp = (hT[:, :, HB * g:HB * (g + 1)] if t == 0
                              else hTs[g][:])
                        nc.vector.tensor_mul(zh[g][:], zsig[g][:], hp)
                    for g in (0, 1):
                        nc.vector.tensor_scalar(omz[g][:].rearrange("p a b -> p (a b)"),
                                                zsig[g][:].rearrange("p a b -> p (a b)"),
                                                -1.0, 1.0, OP.mult, OP.add)
                    for g in (0, 1):
                        nc.vector.tensor_mul(ozn[g][:], omz[g][:], nt_[g][:])
                    for g in (0, 1):
                        nc.vector.tensor_add(_r(hnew[g][:]), ozn[g][:], zh[g][:])
                    for g in (0, 1):
                        nc.scalar.copy(qembT[:, :, HB * g:HB * (g + 1), t],
                                       hnew[g][:])
                    hTs = hnew
                    if t == 2:
                        dense_T_ptw(wG2, clsT, b2goT, None, G2T)
                    if t == 4:
                        dense_T_ptw(wG3, G2T, b3goT, Sgo, goutT)
            load_Xn(1)

        # ================= phase C: Q^T, W1^T, Qt^T =======================
        with tc.tile_pool(name="prep", bufs=1) as prep:
            QT = prep.tile([128, ET, RQ], BF, tag="QT")
            qflat = qembT[:].rearrange("p a b t -> p a (b t)")
            for mt in range(ET):
                p = pgroup(mt)
                for kt in range(ET):
                    nc.tensor.matmul(p[:], W0[:, kt, 128 * mt:128 * (mt + 1)],
                                     qflat[:, kt, :], start=(kt == 0),
                                     stop=(kt == ET - 1))
                nc.vector.tensor_scalar(QT[:, mt, :], p[:], b0laT[:, mt:mt + 1],
                                        None, OP.add)
            W1T = prep.tile([128, ET, D], BF, tag="W1T")
            for hd in range(ET):
                for grp in range(2):
                    pt2 = psB.tile([128, 512], BF, tag="ptw")
                    for i in range(3):
                        e2 = grp * 3 + i
                        nc.tensor.matmul(pt2[:, 128 * i:128 * (i + 1)],
                                         W1[:, e2, 128 * hd:128 * (hd + 1)],
                                         identb[:], is_transpose=True,
                                         skip_group_check=True)
                    if grp == 0:
                        nc.vector.tensor_copy(W1T[:, hd, 0:384], pt2[:, 0:384])
                    else:
                        nc.scalar.copy(W1T[:, hd, 384:768], pt2[:, 0:384])
            scl = 1.0 / float(np.sqrt(DK))
            for h in range(NH):
                for mt in range(ET):
                    p = pgroup(h * ET + mt)
                    for i in range(3):
                        kt = h * 3 + i
                        nc.tensor.matmul(p[:], W1T[:, kt, 128 * mt:128 * (mt + 1)],
                                         QT[:, kt, :], start=(i == 0), stop=(i == 2))
                    dst = QtT[:, mt, :].rearrange("p (b h2 t) -> p b h2 t",
                                                  h2=NH, t=T)[:, :, h, :]
                    if (h * ET + mt) % 3 != 2:
                        nc.vector.tensor_scalar(dst, p[:], scl, None, OP.mult)
                    else:
                        nc.scalar.activation(dst, p[:], AF.Copy, scale=scl)

        # ================= phase D: per-b attention =======================
        with tc.tile_pool(name="ab", bufs=2) as ab, \
             tc.tile_pool(name="xbp2", bufs=1) as xb2_pool:
            xb2[0] = xb2_pool
            for b in range(BL):
                for bn in (b + 1, b + 2):
                    if bn < BL and bn not in XnMap:
                        load_Xn(bn)
                Xn = XnMap.pop(b)
                XT = ab.tile([128, ET, NKC * 128], BF, tag="XT")
                for et in range(ET):
                    if et % 3 == 2:
                        pt = psD.tile([128, 1024], BF, tag="gn")
                    else:
                        pt = psB.tile([128, 1024], BF, tag="ptw")
                    for c in range(NKC):
                        nc.tensor.matmul(pt[:, 128 * c:128 * (c + 1)],
                                         Xn[:, c, 128 * et:128 * (et + 1)],
                                         identb[:], is_transpose=True,
                                         skip_group_check=True)
                    if et != 4:
                        nc.vector.tensor_copy(XT[:, et, :NK], pt[:, :NK])
                    else:
                        nc.scalar.copy(XT[:, et, :NK], pt[:, :NK])
                att = ab.tile([64, NKC * 128], BF, tag="att")
                nc.vector.memset(att[:, NK:], 0.0)
                zacc = ab.tile([64, 2], FP, tag="zacc")
                for ci, (n0, nw) in enumerate(CH_NK):
                    p = psA.tile([64, 512], FP, tag=f"wh{ci}")
                    for kt in range(ET):
                        nc.tensor.matmul(p[:, :nw],
                                         QtT[:, kt, b * 2 * T:(b + 1) * 2 * T],
                                         XT[:, kt, n0:n0 + nw],
                                         start=(kt == 0), stop=(kt == ET - 1))
                    nc.scalar.activation(att[:, n0:n0 + nw], p[:, :nw], AF.Exp,
                                         accum_out=zacc[:, ci:ci + 1])
                zs = ab.tile([64, 1], FP, tag="zs")
                nc.vector.tensor_add(zs[:], zacc[:, 0:1], zacc[:, 1:2])
                rz = ab.tile([64, 1], FP, tag="rz1")
                nc.vector.reciprocal(rz[:], zs[:])
                wm = ab.tile([64, 2], BF, tag="wm")
                nc.vector.tensor_scalar(wm[:], pmask[:], rz[:, 0:1], None, OP.mult)
                # paT[k, i] = sum_q att[q, k] * wm[q, i]  (no transposes!)
                pp = psD.tile([128, 2 * NKC], FP, tag="gr")
                for c in range(NKC):
                    nc.tensor.matmul(pp[:, 2 * c:2 * c + 2],
                                     att[:, 128 * c:128 * (c + 1)], wm[:],
                                     start=True, stop=True,
                                     skip_group_check=True)
                paT = ab.tile([128, NKC, 2], BF, tag="paT")
                nc.vector.tensor_copy(paT[:].rearrange("p a b -> p (a b)"), pp[:])
                # ctxT[d, i] = sum_k Xn[k, d] * paT[k, i]
                pc = psD.tile([128, 2 * ET], FP, tag="gz")
                for dc in range(ET):
                    for c in range(NKC):
                        nc.tensor.matmul(pc[:, 2 * dc:2 * dc + 2],
                                         Xn[:, c, 128 * dc:128 * (dc + 1)],
                                         paT[:, c, :], start=(c == 0),
                                         stop=(c == NKC - 1),
                                         skip_group_check=True)
                nc.vector.tensor_copy(
                    pcxT2[:, :, :, b].rearrange("p a b -> p (a b)"), pc[:])

        # ================= phase E: projections + MLP =====================
        with tc.tile_pool(name="tail", bufs=1) as tail:
            vconT = tail.tile([128, ET], FP, tag="vconT")
            for mt in range(ET):
                p = pgroup(mt)
                for kt in range(ET):
                    nc.tensor.matmul(p[:, 0:1], W3[:, kt, 128 * mt:128 * (mt + 1)],
                                     b2laT_bf[:, kt:kt + 1], start=(kt == 0),
                                     stop=(kt == ET - 1), skip_group_check=True)
                nc.vector.tensor_scalar(vconT[:, mt:mt + 1], p[:, 0:1],
                                        b3laT[:, mt:mt + 1], Sla[:, 0:1],
                                        OP.add, OP.mult)
            pctxT = tail.tile([128, ET, BL], BF, tag="pctxT")
            for h in range(NH):
                for mi in range(3):
                    mt = h * 3 + mi
                    p = pgroup(mt)
                    for kt in range(ET):
                        nc.tensor.matmul(p[:, 0:BL],
                                         W2[:, kt, 128 * mt:128 * (mt + 1)],
                                         pcxT2[:, kt, h, :], start=(kt == 0),
                                         stop=(kt == ET - 1),
                                         skip_group_check=True)
                    nc.vector.tensor_copy(pctxT[:, mt, :], p[:, 0:BL])
            loT = tail.tile([128, ET, BL], BF, tag="loT")
            for mt in range(ET):
                p = pgroup(mt)
                for kt in range(ET):
                    nc.tensor.matmul(p[:, 0:BL], W3[:, kt, 128 * mt:128 * (mt + 1)],
                                     pctxT[:, kt, :], start=(kt == 0),
                                     stop=(kt == ET - 1), skip_group_check=True)
                nc.vector.tensor_scalar(loT[:, mt, :], p[:, 0:BL],
                                        vconT[:, mt:mt + 1], None, OP.add)

            y1T = tail.tile([128, 8, BL], BF, tag="y1T")
            for mt in range(8):
                p = pgroup(mt)
                for i, kt in enumerate(list(range(ET, 12)) + list(range(ET))):
                    r_ = loT[:, kt, :] if kt < ET else goutT[:, kt - ET, :]
                    nc.tensor.matmul(p[:, 0:BL], f1[:, kt, 128 * mt:128 * (mt + 1)],
                                     r_, start=(i == 0), stop=(i == 11),
                                     skip_group_check=True)
                nc.vector.tensor_scalar(y1T[:, mt, :], p[:, 0:BL],
                                        b1fT[:, mt:mt + 1], None, OP.add)
            y2T = tail.tile([128, 4, BL], BF, tag="y2T")
            for mt in range(4):
                p = pgroup(mt)
                for kt in range(8):
                    nc.tensor.matmul(p[:, 0:BL], f2[:, kt, 128 * mt:128 * (mt + 1)],
                                     y1T[:, kt, :], start=(kt == 0), stop=(kt == 7),
                                     skip_group_check=True)
                nc.scalar.activation(y2T[:, mt, :], p[:, 0:BL], AF.Relu,
                                     bias=b2fT[:, mt:mt + 1])
            yT = tail.tile([128, 8, BL], FP, tag="yT")
            for mt in range(8):
                p = pgroup(mt)
                for kt in range(4):
                    nc.tensor.matmul(p[:, 0:BL], f3[:, kt, 128 * mt:128 * (mt + 1)],
                                     y2T[:, kt, :], start=(kt == 0), stop=(kt == 3),
                                     skip_group_check=True)
                nc.vector.tensor_scalar(yT[:, mt, :], p[:, 0:BL],
                                        b3fT[:, mt:mt + 1], None, OP.add)
            ynat = tail.tile([BL, 1024], FP, tag="ynat")
            for g in range(2):
                po = psB.tile([128, 512], FP, tag="ptw")
                for i in range(4):
                    mt = g * 4 + i
                    nc.tensor.matmul(po[:BL, 128 * i:128 * (i + 1)], yT[:, mt, :],
                                     ident[:128, :128], is_transpose=True,
                                     skip_group_check=True)
                nc.vector.tensor_copy(ynat[:, 512 * g:512 * (g + 1)], po[:BL, :])
            nc.sync.dma_start(out_d[:, :], ynat[:])

    nc.compile()
    return nc


_NC = None


def kernel(**inputs):
    global _NC
    if _NC is None:
        _NC = build()
    B = inputs["image_local_embeds"].shape[0]
    per = B // NCORES
    in_maps = []
    for c in range(NCORES):
        sl = slice(c * per, (c + 1) * per)
        m = {
            "img": np.ascontiguousarray(np.asarray(inputs["image_local_embeds"])[sl], dtype=np.float32),
            "h0": np.ascontiguousarray(np.asarray(inputs["h0"])[sl], dtype=np.float32),
        }
        for k in ["gru_w_ih", "gru_w_hh", "gru_b_ih", "gru_b_hh", "ga_w", "ga_b",
                  "ga_pool", "la_w", "la_b", "la_pool", "go_w", "go_b", "go_pool",
                  "f1_w", "f1_b", "f2_w", "f2_b", "f3_w", "f3_b"]:
            m[k] = np.ascontiguousarray(np.asarray(inputs[k], dtype=np.float32))
        in_maps.append(m)
    res = run_bass_kernel_spmd(_NC, in_maps, core_ids=list(range(NCORES)))
    return np.concatenate([res.results[c]["out"] for c in range(NCORES)], axis=0)


# revision 78
# speedup vs baseline: 1.1592x; 1.0313x over previous
"""Trainium2 Bass kernel for nn_BiVision_VQA2 (B=64,T=32,D=768,N=901).

Data-parallel over batch: 8 batch elems per core x 8 cores.
Key math simplifications (validated vs reference, rel err ~1e-4):
  - ga/go attention use a single key token -> softmax==1 -> those paths are
    linear in cls; question_embeds is mathematically unused.
  - GRU input `a` is constant over time; wx computed once.
  - local attention: row-constant score terms drop out of softmax; query
    pooling applied to the attention matrix before the @X contraction.
Performance structure:
  - GRU computed in TRANSPOSED gate layout [128(gate row), batch] via
    weight-stationary matmuls (moving N=8), elementwise on [128, 48].
  - bf16 everywhere outside the GRU recurrence (DMA casts on load).
  - phase D: transpose-free paT/ctxT via natural-operand-stationary matmuls.
"""

import os
import numpy as np
from contextlib import ExitStack

import concourse.bass as bass
import concourse.tile as tile
from concourse import bacc, mybir
from concourse.bass_utils import run_bass_kernel_spmd
from concourse.masks import make_identity

FP = mybir.dt.float32
FPR = mybir.dt.float32r
OP = mybir.AluOpType
AF = mybir.ActivationFunctionType
BF = mybir.dt.bfloat16

NCORES = 8
BL = 8
D = 768
T = 32
G = 3 * D
NK = 900
NH = 2
DK = 384
ET = D // 128
RQ = BL * T

CH_G = [(0, 512), (512, 512), (1024, 512), (1536, 512), (2048, 256)]
CH_NK = [(0, 512), (512, 388)]

GBF = os.environ.get("KGRUBF", "1") == "1"
GDT = BF if GBF else FP


def _r(ap):
    return ap if GBF else ap.bitcast(FPR)


from contextlib import contextmanager


@contextmanager
def _nullcm():
    yield


def kchunks(n):
    out, o = [], 0
    while o < n:
        out.append((o, min(128, n - o)))
        o += 128
    return out


def build():
    nc = bacc.Bacc("TRN2", target_bir_lowering=False, debug=False,
                   enable_asserts=False)

    img = nc.dram_tensor("img", [BL, 901, D], FP, kind="ExternalInput").ap()
    h0 = nc.dram_tensor("h0", [BL, D], FP, kind="ExternalInput").ap()
    w_ih = nc.dram_tensor("gru_w_ih", [G, D], FP, kind="ExternalInput").ap()
    w_hh = nc.dram_tensor("gru_w_hh", [G, D], FP, kind="ExternalInput").ap()
    b_ih = nc.dram_tensor("gru_b_ih", [G], FP, kind="ExternalInput").ap()
    b_hh = nc.dram_tensor("gru_b_hh", [G], FP, kind="ExternalInput").ap()
    ga_w = nc.dram_tensor("ga_w", [4, D, D], FP, kind="ExternalInput").ap()
    ga_b = nc.dram_tensor("ga_b", [4, D], FP, kind="ExternalInput").ap()
    ga_pool = nc.dram_tensor("ga_pool", [1], FP, kind="ExternalInput").ap()
    la_w = nc.dram_tensor("la_w", [4, D, D], FP, kind="ExternalInput").ap()
    la_b = nc.dram_tensor("la_b", [4, D], FP, kind="ExternalInput").ap()
    la_pool = nc.dram_tensor("la_pool", [T], FP, kind="ExternalInput").ap()
    go_w = nc.dram_tensor("go_w", [4, D, D], FP, kind="ExternalInput").ap()
    go_b = nc.dram_tensor("go_b", [4, D], FP, kind="ExternalInput").ap()
    go_pool = nc.dram_tensor("go_pool", [T], FP, kind="ExternalInput").ap()
    f1_w = nc.dram_tensor("f1_w", [2 * D, 1024], FP, kind="ExternalInput").ap()
    f1_b = nc.dram_tensor("f1_b", [1024], FP, kind="ExternalInput").ap()
    f2_w = nc.dram_tensor("f2_w", [1024, 512], FP, kind="ExternalInput").ap()
    f2_b = nc.dram_tensor("f2_b", [512], FP, kind="ExternalInput").ap()
    f3_w = nc.dram_tensor("f3_w", [512, 1024], FP, kind="ExternalInput").ap()
    f3_b = nc.dram_tensor("f3_b", [1024], FP, kind="ExternalInput").ap()
    out_d = nc.dram_tensor("out", [BL, 1024], FP, kind="ExternalOutput").ap()

    def chunked(dram2d, nc_, cw=D):
        # [R, cw] dram viewed as [128, R//128, cw]
        return dram2d.rearrange("(c p) d -> p c d", p=128)

    with tile.TileContext(nc) as tc, ExitStack() as ctx:
        cpool = ctx.enter_context(tc.tile_pool(name="const", bufs=1))
        persist = ctx.enter_context(tc.tile_pool(name="persist", bufs=1))
        xb = ctx.enter_context(tc.tile_pool(name="xb", bufs=2))
        psA = ctx.enter_context(tc.tile_pool(name="psA", bufs=1, space="PSUM"))
        psB = ctx.enter_context(tc.tile_pool(name="psB", bufs=2, space="PSUM"))
        psC = ctx.enter_context(tc.tile_pool(name="psC", bufs=1, space="PSUM"))
        psD = ctx.enter_context(tc.tile_pool(name="psD", bufs=1, space="PSUM"))

        ident = cpool.tile([128, 128], FP, tag="ident")
        make_identity(nc, ident[:])
        identb = cpool.tile([128, 128], BF, tag="identb")
        nc.vector.tensor_copy(identb[:], ident[:])
        ones1 = cpool.tile([1, 128], FP, tag="ones1")
        nc.vector.memset(ones1[:], 1.0)
        ones1b = cpool.tile([1, 128], BF, tag="ones1b")
        nc.vector.memset(ones1b[:], 1.0)
        onesT = cpool.tile([T, 128], FP, tag="onesT")
        nc.vector.memset(onesT[:], 1.0)

        # ---- small bias vectors -> column layout via K=1 matmuls ---------
        def colvec_batch(specs):
            # pipelined: all row-loads first (3 rotating staging slots),
            # then K=1 matmuls into one psum tile, then copies out.
            pdvl = psC.tile([128, 64], FP, tag="pd")
            off = 0
            outs = []
            for idx, (src, n) in enumerate(specs):
                nt = n // 128
                vr = cpool.tile([1, 1024], FP, tag=f"vrow{idx % 2}")
                nc.sync.dma_start(vr[:, :n], src[:][None, :])
                for c in range(nt):
                    nc.tensor.matmul(pdvl[:, off + c:off + c + 1],
                                     vr[0:1, 128 * c:128 * (c + 1)],
                                     ones1[:1, :1], start=True, stop=True,
                                     skip_group_check=True)
                outs.append((off, nt))
                off += nt
            return pdvl, outs

        def colvec_out(pdvl, o_nt, tag):
            o, nt = o_nt
            t_ = cpool.tile([128, nt], FP, tag=tag)
            nc.vector.tensor_copy(t_[:], pdvl[:, o:o + nt])
            return t_

        pdv1, offs1 = colvec_batch([(ga_b[2], D), (ga_b[3], D),
                                    (go_b[2], D), (go_b[3], D),
                                    (la_b[0], D), (la_b[2], D),
                                    (la_b[3], D), (f1_b, 1024),
                                    (f2_b, 512), (f3_b, 1024)])
        b2gaT = colvec_out(pdv1, offs1[0], "b2gaT")
        b3gaT = colvec_out(pdv1, offs1[1], "b3gaT")
        b2goT = colvec_out(pdv1, offs1[2], "b2goT")
        b3goT = colvec_out(pdv1, offs1[3], "b3goT")
        b0laT = colvec_out(pdv1, offs1[4], "b0laT")
        b2laT = colvec_out(pdv1, offs1[5], "b2laT")
        b3laT = colvec_out(pdv1, offs1[6], "b3laT")
        b1fT = colvec_out(pdv1, offs1[7], "b1fT")
        b2fT = colvec_out(pdv1, offs1[8], "b2fT")
        b3fT = colvec_out(pdv1, offs1[9], "b3fT")
        b2laT_bf = cpool.tile([128, ET], BF, tag="b2laT_bf")
        nc.vector.tensor_copy(b2laT_bf[:], b2laT[:])

        lapool_c = cpool.tile([T, 1], FP, tag="lapool_c")
        nc.sync.dma_start(lapool_c[:], la_pool[:][:, None])
        gopool_c = cpool.tile([T, 1], FP, tag="gopool_c")
        nc.sync.dma_start(gopool_c[:], go_pool[:][:, None])
        gapool_c = cpool.tile([1, 1], FP, tag="gapool_c")
        nc.sync.dma_start(gapool_c[:], ga_pool[:][:, None])

        def sum_bcast(vcol, k, tag):
            p = psC.tile([128, 64], FP, tag="pd")
            lhs = onesT if k == T else ones1
            nc.tensor.matmul(p[:, 0:1], lhs[:k, :], vcol[:k, :], start=True,
                             stop=True, skip_group_check=True)
            s = cpool.tile([128, 1], FP, tag=tag)
            nc.vector.tensor_copy(s[:], p[:, 0:1])
            return s

        Sla = sum_bcast(lapool_c, T, "Sla")
        Sgo = sum_bcast(gopool_c, T, "Sgo")
        Sga = sum_bcast(gapool_c, 1, "Sga")

        pmask = cpool.tile([64, 2], FP, tag="pmask")
        nc.vector.memset(pmask[:], 0.0)
        nc.sync.dma_start(pmask[0:T, 0:1], la_pool[:][:, None])
        nc.sync.dma_start(pmask[T:2 * T, 1:2], la_pool[:][:, None])

        # img patch loads (streamed; b0/b1 prefetched early)
        KC = kchunks(NK)
        NKC = len(KC)
        XnMap = {}

        xb2 = [None]

        def load_Xn(b):
            # 3-way buffer rotation: xb holds 2, xb2 (opened for phase D,
            # reusing SBUF freed by the GRU pools) holds the third
            pool = xb2[0] if (b % 3 == 2 and xb2[0] is not None) else xb
            Xn = pool.tile([128, NKC, D], BF, tag="Xn")
            if b < 3:
                # zero the pad rows once per physical buffer (b0,b1 -> xb's
                # two buffers, b2 -> xb2); later b's reuse a buffer and only
                # ever rewrite rows 0..kwl of the last chunk
                nc.vector.memset(Xn[:, NKC - 1, :], 0.0)
            nc.gpsimd.dma_start(
                Xn[:, 0:NKC - 1, :],
                img[b, 1:1 + 128 * (NKC - 1), :].rearrange(
                    "(c p) d -> p c d", p=128))
            k0l, kwl = KC[-1]
            nc.gpsimd.dma_start(Xn[:kwl, NKC - 1, :],
                                img[b, 1 + k0l:1 + k0l + kwl, :])
            XnMap[b] = Xn

        def pgroup(i, ncols=RQ):
            pl, tg = [(psC, "pd"), (psD, "gr"), (psD, "gz"), (psD, "gn")][i % 4]
            pg_t = pl.tile([128, ncols], FP, tag=tg)
            return pg_t

        # persistent outputs of the phases
        qembT = cpool.tile([128, ET, BL, T], BF, tag="qembT")
        wxb = cpool.tile([BL, G], BF, tag="wxb")
        QtT = persist.tile([128, ET, NH * RQ], BF, tag="QtT")
        goutT = cpool.tile([128, ET, BL], BF, tag="goutT")
        aT = cpool.tile([128, ET, BL], GDT, tag="aT")
        pcxT2 = persist.tile([128, ET, NH, BL], BF, tag="pcxT2")

        # ================= phase B: GRU ===================================
        with tc.tile_pool(name="wbig", bufs=1) as wbig, \
             tc.tile_pool(name="wnat", bufs=2) as wnat, \
             tc.tile_pool(name="wst", bufs=3) as wst, \
             tc.tile_pool(name="g1", bufs=2) as g1:
            combr = wbig.tile([1, G], BF, tag="combr")
            nc.gpsimd.dma_start(combr[:], b_ih[:][None, :])
            bhhrow = wbig.tile([1, G], BF, tag="bhhrow")
            nc.gpsimd.dma_start(bhhrow[:], b_hh[:][None, :])
            nc.vector.tensor_add(combr[:, 0:2 * D], combr[:, 0:2 * D],
                                 bhhrow[:, 0:2 * D])
            bhhr_bf = bhhrow[:, 2 * D:3 * D]

            WT = wbig.tile([128, ET, G], GDT, tag="WT")
            tident = identb if GBF else ident

            _rc = [0]

            def build_WT(w_dram, dma_prio=0):
                jts = kchunks(G)
                for g0 in range(0, len(jts), 5):
                    grp = jts[g0:g0 + 5]
                    ng = len(grp)
                    wn = wst.tile([128, 5, D], GDT, tag="wn")
                    src = w_dram[grp[0][0]:grp[-1][0] + grp[-1][1], :]
                    src = src.rearrange("(c p) d -> p c d", p=128)
                    with tc.high_priority(offset=dma_prio if dma_prio else None) \
                            if dma_prio else _nullcm():
                        if GBF:
                            nc.gpsimd.dma_start(wn[:, :ng, :], src)
                        else:
                            nc.sync.dma_start(wn[:, :ng, :], src)
                    sub = 5 if GBF else 3
                    for et in range(ET):
                        for s0 in range(0, ng, sub):
                            sg = min(sub, ng - s0)
                            # rotate staging over 4 banks (gr/gz idle pre-GRU)
                            _rc[0] += 1
                            pl, tg = [(psB, "ptw"), (psD, "gr"),
                                      (psB, "ptw"), (psD, "gz")][_rc[0] % 4]
                            pt = pl.tile([128, 128 * sub], GDT, tag=tg)
                            for i in range(sg):
                                nc.tensor.matmul(pt[:, 128 * i:128 * (i + 1)],
                                                 wn[:, s0 + i, 128 * et:128 * (et + 1)],
                                                 tident[:], is_transpose=True,
                                                 skip_group_check=True)
                            w0 = grp[0][0] + 128 * s0
                            wlen = 128 * sg
                            if (et + s0) % 2 == 0:
                                nc.vector.tensor_copy(_r(WT[:, et, w0:w0 + wlen]),
                                                      pt[:, :wlen])
                            else:
                                nc.scalar.copy(_r(WT[:, et, w0:w0 + wlen]),
                                               pt[:, :wlen])

            build_WT(w_ih)

            # ---- phase A part 1 (cls -> a), interleaved after W_ih ------
            clsn = wbig.tile([BL, D], BF, tag="clsn")
            nc.gpsimd.dma_start(clsn[:], img[0:BL, 0, :])
            wA2 = wnat.tile([128, ET, D], BF, tag="wa")
            nc.gpsimd.dma_start(wA2[:], chunked(ga_w[2], nc))
            wA3 = wnat.tile([128, ET, D], BF, tag="wa")
            nc.gpsimd.dma_start(wA3[:], chunked(ga_w[3], nc))
            ptr = psB.tile([128, 512], BF, tag="ptw")
            for kt in range(ET):
                nc.tensor.matmul(ptr[:, 8 * kt:8 * kt + 8],
                                 clsn[:, 128 * kt:128 * (kt + 1)],
                                 identb[:BL, :BL], is_transpose=True,
                                 skip_group_check=True)
            clsT = wbig.tile([128, ET, BL], BF, tag="clsT")
            nc.vector.tensor_copy(clsT[:].rearrange("p a b -> p (a b)"),
                                  ptr[:, :8 * ET])

            def dense_T(wsb, rhsT, biasT, scaleT, otile, out_r=False):
                for mt in range(ET):
                    p = psC.tile([128, BL], FP, tag="pd")
                    for kt in range(ET):
                        nc.tensor.matmul(p[:], wsb[:, kt, 128 * mt:128 * (mt + 1)],
                                         rhsT[:, kt, :], start=(kt == 0),
                                         stop=(kt == ET - 1))
                    dst = otile[:, mt, :]
                    if out_r:
                        dst = _r(dst)
                    if scaleT is None:
                        nc.vector.tensor_scalar(dst, p[:], biasT[:, mt:mt + 1],
                                                None, OP.add)
                    else:
                        nc.vector.tensor_scalar(dst, p[:], biasT[:, mt:mt + 1],
                                                scaleT[:, 0:1], OP.add, OP.mult)

            A2T = wbig.tile([128, ET, BL], BF, tag="A2T")
            dense_T(wA2, clsT, b2gaT, None, A2T)
            dense_T(wA3, A2T, b3gaT, Sga, aT, out_r=not GBF)

            for (j0, jw) in CH_G:
                p = psA.tile([BL, 512], FP, tag="wh0")
                for kt in range(ET):
                    nc.tensor.matmul(p[:, :jw], aT[:, kt, :] if GBF else _r(aT[:, kt, :]),
                                     _r(WT[:, kt, j0:j0 + jw]),
                                     start=(kt == 0), stop=False)
                nc.tensor.matmul(p[:, :jw], ones1b[:1, :BL],
                                 combr[:, j0:j0 + jw], start=False, stop=True)
                nc.vector.tensor_copy(wxb[:, j0:j0 + jw], p[:, :jw])

            build_WT(w_hh)

            # ---- phase A part 2 (gout path) — loads emitted here, the
            # dense compute happens inside the GRU loop (idle engine slack)
            wG2 = wnat.tile([128, ET, D], BF, tag="wa")
            nc.gpsimd.dma_start(wG2[:], chunked(go_w[2], nc))
            wG3 = wnat.tile([128, ET, D], BF, tag="wa")
            nc.gpsimd.dma_start(wG3[:], chunked(go_w[3], nc))
            G2T = wbig.tile([128, ET, BL], BF, tag="G2T")

            def dense_T_ptw(wsb, rhsT, biasT, scaleT, otile):
                # dense_T variant staged in the ptw banks (free during GRU)
                for mt in range(ET):
                    p = psB.tile([128, BL], FP, tag="ptw")
                    for kt in range(ET):
                        nc.tensor.matmul(p[:], wsb[:, kt, 128 * mt:128 * (mt + 1)],
                                         rhsT[:, kt, :], start=(kt == 0),
                                         stop=(kt == ET - 1))
                    if scaleT is None:
                        nc.vector.tensor_scalar(otile[:, mt, :], p[:],
                                                biasT[:, mt:mt + 1], None, OP.add)
                    else:
                        nc.vector.tensor_scalar(otile[:, mt, :], p[:],
                                                biasT[:, mt:mt + 1],
                                                scaleT[:, 0:1], OP.add, OP.mult)

            # transposed constant wx for the n-gate: [128, ET, BL]
            ptx = psC.tile([128, 64], BF, tag="pd")
            for kt in range(ET):
                nc.tensor.matmul(ptx[:, 8 * kt:8 * kt + 8],
                                 wxb[:, 2 * D + 128 * kt:2 * D + 128 * (kt + 1)],
                                 identb[:BL, :BL], is_transpose=True,
                                 skip_group_check=True)
            wxTn = wbig.tile([128, ET, BL], FP, tag="wxTn")
            nc.vector.tensor_copy(wxTn[:].rearrange("p a b -> p (a b)"),
                                  ptx[:, :8 * ET])

            # initial h0 transposed
            hnat0 = wbig.tile([BL, D], BF, tag="hnat0")
            nc.gpsimd.dma_start(hnat0[:], h0[:, :])
            ptr0 = psC.tile([128, 64], BF, tag="pd")
            for kt in range(ET):
                nc.tensor.matmul(ptr0[:, 8 * kt:8 * kt + 8],
                                 hnat0[:, 128 * kt:128 * (kt + 1)],
                                 identb[:BL, :BL], is_transpose=True,
                                 skip_group_check=True)
            hT = wbig.tile([128, ET, BL], GDT, tag="h0T")
            nc.vector.tensor_copy(_r(hT[:].rearrange("p a b -> p (a b)")),
                                  ptr0[:, :8 * ET])

            # prefetch DMAs for phases C/D/E: deprioritized so they only
            # fill DMA slots the W/A loads are not using
            with tc.high_priority(offset=-100000):
                W0 = persist.tile([128, ET, D], BF, tag="W0")
                nc.gpsimd.dma_start(W0[:], chunked(la_w[0], nc))
                W1 = persist.tile([128, ET, D], BF, tag="W1")
                nc.gpsimd.dma_start(W1[:], chunked(la_w[1], nc))
                W2 = persist.tile([128, ET, D], BF, tag="W2")
                nc.gpsimd.dma_start(W2[:], chunked(la_w[2], nc))
                W3 = persist.tile([128, ET, D], BF, tag="W3")
                nc.gpsimd.dma_start(W3[:], chunked(la_w[3], nc))
                f1 = persist.tile([128, 12, 1024], BF, tag="f1")
                nc.gpsimd.dma_start(f1[:], f1_w.rearrange("(c p) n -> p c n", p=128))
                f2 = persist.tile([128, 8, 512], BF, tag="f2")
                nc.gpsimd.dma_start(f2[:], f2_w.rearrange("(c p) n -> p c n", p=128))
                f3 = persist.tile([128, 4, 1024], BF, tag="f3")
                nc.gpsimd.dma_start(f3[:], f3_w.rearrange("(c p) n -> p c n", p=128))
                load_Xn(0)

            KSTEPS = int(os.environ.get("KSTEPS", str(T)))
            KHALF = os.environ.get("KHALF", "1") == "1"
            HB = BL // 2
            wxTn3 = wxTn[:]
            if not KHALF:
                for t in range(KSTEPS):
                    psR = psD.tile([128, ET * BL], FP, tag="gr")
                    psZ = psD.tile([128, ET * BL], FP, tag="gz")
                    psN = psD.tile([128, ET * BL], FP, tag="gn")

                    def gate_chunk(ps, mi, m):
                        j0 = 128 * m
                        for kt in range(ET):
                            nc.tensor.matmul(ps[:, BL * mi:BL * (mi + 1)],
                                             _r(WT[:, kt, j0:j0 + 128]),
                                             _r(hT[:, kt, :]),
                                             start=(kt == 0), stop=False,
                                             skip_group_check=True)
                        if m < 12:
                            nc.tensor.matmul(ps[:, BL * mi:BL * (mi + 1)],
                                             wxb[:, j0:j0 + 128],
                                             identb[:BL, :BL], start=False,
                                             stop=True, skip_group_check=True)
                        else:
                            nc.tensor.matmul(ps[:, BL * mi:BL * (mi + 1)],
                                             bhhr_bf[:, j0 - 2 * D:j0 - 2 * D + 128],
                                             ones1b[:1, :BL],
                                             start=False, stop=True,
                                             skip_group_check=True)

                    for mi in range(ET):
                        gate_chunk(psR, mi, mi)
                    for mi in range(ET):
                        gate_chunk(psN, mi, 12 + mi)
                    for mi in range(ET):
                        gate_chunk(psZ, mi, 6 + mi)

                    # h_new = (1-z)*n + z*h ; z-products run in tanh's shadow
                    rsig = g1.tile([128, ET * BL], FP, tag="rsig")
                    nc.scalar.activation(rsig[:], psR[:], AF.Sigmoid)
                    zsig = g1.tile([128, ET * BL], FP, tag="zsig")
                    nc.scalar.activation(zsig[:], psZ[:], AF.Sigmoid)
                    rwn = g1.tile([128, ET * BL], FP, tag="rwn")
                    nc.vector.tensor_mul(rwn[:], rsig[:], psN[:])
                    npre = g1.tile([128, ET * BL], FP, tag="npre")
                    nc.vector.tensor_add(npre[:], rwn[:],
                                         wxTn[:].rearrange("p a b -> p (a b)"))
                    nt_ = g1.tile([128, ET * BL], FP, tag="nt")
                    nc.scalar.activation(nt_[:], npre[:], AF.Tanh)
                    zh = g1.tile([128, ET * BL], FP, tag="zh")
                    nc.vector.tensor_mul(zh[:], zsig[:],
                                         hT[:].rearrange("p a b -> p (a b)"))
                    omz = g1.tile([128, ET * BL], FP, tag="omz")
                    nc.vector.tensor_scalar(omz[:], zsig[:], -1.0, 1.0,
                                            OP.mult, OP.add)
                    ozn = g1.tile([128, ET * BL], FP, tag="ozn")
                    nc.vector.tensor_mul(ozn[:], omz[:], nt_[:])
                    hT = g1.tile([128, ET, BL], GDT, tag="hT")
                    nc.vector.tensor_add(_r(hT[:].rearrange("p a b -> p (a b)")),
                                         ozn[:], zh[:])
                    nc.scalar.copy(qembT[:, :, :, t].rearrange("p a b -> p (a b)"),
                                   hT[:].rearrange("p a b -> p (a b)"))
            else:
                # two independent half-batch chains, interleaved so each
                # half's elementwise hides in the other's latency
                hTs = [None, None]
                psmap = [(psD, "gr"), (psD, "gz"), (psA, "wh0"), (psA, "wh1"),
                         (psD, "gn"), (psC, "pd")]
                for t in range(KSTEPS):
                    def hprev(g, kt):
                        if t == 0:
                            return _r(hT[:, kt, HB * g:HB * (g + 1)])
                        return _r(hTs[g][:, kt, :])

                    def ps_half(i):
                        pl, tg = psmap[i]
                        ph_t = pl.tile([128, ET * HB], FP, tag=tg)
                        return ph_t

                    psRs = [ps_half(0), ps_half(1)]
                    psZs = [ps_half(2), ps_half(3)]
                    psNs = [ps_half(4), ps_half(5)]

                    def gate_const(ps, mi, m, g):
                        # constant (wx/bhh) opener: no h dependency, so it
                        # runs during the previous step's elementwise tail
                        j0 = 128 * m
                        if m < 12:
                            nc.tensor.matmul(ps[:, HB * mi:HB * (mi + 1)],
                                             wxb[:, j0:j0 + 128],
                                             identb[:BL, HB * g:HB * (g + 1)],
                                             start=True, stop=False,
                                             skip_group_check=True)
                        else:
                            nc.tensor.matmul(ps[:, HB * mi:HB * (mi + 1)],
                                             bhhr_bf[:, j0 - 2 * D:j0 - 2 * D + 128],
                                             ones1b[:1, :HB],
                                             start=True, stop=False,
                                             skip_group_check=True)

                    def gate_h(ps, mi, m, g):
                        j0 = 128 * m
                        for kt in range(ET):
                            nc.tensor.matmul(ps[:, HB * mi:HB * (mi + 1)],
                                             _r(WT[:, kt, j0:j0 + 128]),
                                             hprev(g, kt),
                                             start=False, stop=(kt == ET - 1),
                                             skip_group_check=True)

                    for ps_list, m0 in ((psRs, 0), (psNs, 12), (psZs, 6)):
                        for g in (0, 1):
                            for mi in range(ET):
                                gate_const(ps_list[g], mi, m0 + mi, g)
                    for ps_list, m0 in ((psRs, 0), (psNs, 12), (psZs, 6)):
                        for g in (0, 1):
                            for mi in range(ET):
                                gate_h(ps_list[g], mi, m0 + mi, g)

                    def tile3(tag):
                        t3 = g1.tile([128, ET, HB], FP, tag=tag)
                        return t3

                    rsig = [tile3("rsig0"), tile3("rsig1")]
                    zsig = [tile3("zsig0"), tile3("zsig1")]
                    rwn = [tile3("rwn0"), tile3("rwn1")]
                    npre = [tile3("npre0"), tile3("npre1")]
                    nt_ = [tile3("nt0"), tile3("nt1")]
                    zh = [tile3("zh0"), tile3("zh1")]
                    omz = [tile3("omz0"), tile3("omz1")]
                    ozn = [tile3("ozn0"), tile3("ozn1")]
                    def tile3g(tag):
                        t3g = g1.tile([128, ET, HB], GDT, tag=tag)
                        return t3g

                    hnew = [tile3g("hTn0"), tile3g("hTn1")]
                    for g in (0, 1):
                        nc.scalar.activation(rsig[g][:].rearrange("p a b -> p (a b)"),
                                             psRs[g][:], AF.Sigmoid)
                    for g in (0, 1):
                        nc.scalar.activation(zsig[g][:].rearrange("p a b -> p (a b)"),
                                             psZs[g][:], AF.Sigmoid)
                    for g in (0, 1):
                        nc.vector.tensor_mul(rwn[g][:].rearrange("p a b -> p (a b)"),
                                             rsig[g][:].rearrange("p a b -> p (a b)"),
                                             psNs[g][:])
                    for g in (0, 1):
                        nc.vector.tensor_add(npre[g][:], rwn[g][:],
                                             wxTn3[:, :, HB * g:HB * (g + 1)])
                    for g in (0, 1):
                        nc.scalar.activation(nt_[g][:].rearrange("p a b -> p (a b)"),
                                             npre[g][:].rearrange("p a b -> p (a b)"),
                                             AF.Tanh)
                    for g in (0, 1):
                        hp = (hT[:, :, HB * g:HB * (g + 1)] if t == 0
                              else hTs[g][:])
                        nc.vector.tensor_mul(zh[g][:], zsig[g][:], hp)
                    for g in (0, 1):
                        nc.vector.tensor_scalar(omz[g][:].rearrange("p a b -> p (a b)"),
                                                zsig[g][:].rearrange("p a b -> p (a b)"),
                                                -1.0, 1.0, OP.mult, OP.add)
                    for g in (0, 1):
                        nc.vector.tensor_mul(ozn[g][:], omz[g][:], nt_[g][:])
                    for g in (0, 1):
                        nc.vector.tensor_add(_r(hnew[g][:]), ozn[g][:], zh[g][:])
                    for g in (0, 1):
                        nc.scalar.copy(qembT[:, :, HB * g:HB * (g + 1), t],
                                       hnew[g][:])
                    hTs = hnew
                    if t == 2:
                        dense_T_ptw(wG2, clsT, b2goT, None, G2T)
                    if t == 4:
                        dense_T_ptw(wG3, G2T, b3goT, Sgo, goutT)
            load_Xn(1)

        # ================= phase C: Q^T, W1^T, Qt^T =======================
        with tc.tile_pool(name="prep", bufs=1) as prep:
            QT = prep.tile([128, ET, RQ], BF, tag="QT")
            qflat = qembT[:].rearrange("p a b t -> p a (b t)")
            for mt in range(ET):
                p = pgroup(mt)
                for kt in range(ET):
                    nc.tensor.matmul(p[:], W0[:, kt, 128 * mt:128 * (mt + 1)],
                                     qflat[:, kt, :], start=(kt == 0),
                                     stop=(kt == ET - 1))
                nc.vector.tensor_scalar(QT[:, mt, :], p[:], b0laT[:, mt:mt + 1],
                                        None, OP.add)
            W1T = prep.tile([128, ET, D], BF, tag="W1T")
            for hd in range(ET):
                for grp in range(2):
                    pt2 = psB.tile([128, 512], BF, tag="ptw")
                    for i in range(3):
                        e2 = grp * 3 + i
                        nc.tensor.matmul(pt2[:, 128 * i:128 * (i + 1)],
                                         W1[:, e2, 128 * hd:128 * (hd + 1)],
                                         identb[:], is_transpose=True,
                                         skip_group_check=True)
                    if grp == 0:
                        nc.vector.tensor_copy(W1T[:, hd, 0:384], pt2[:, 0:384])
                    else:
                        nc.scalar.copy(W1T[:, hd, 384:768], pt2[:, 0:384])
            scl = 1.0 / float(np.sqrt(DK))
            for h in range(NH):
                for mt in range(ET):
                    p = pgroup(h * ET + mt)
                    for i in range(3):
                        kt = h * 3 + i
                        nc.tensor.matmul(p[:], W1T[:, kt, 128 * mt:128 * (mt + 1)],
                                         QT[:, kt, :], start=(i == 0), stop=(i == 2))
                    dst = QtT[:, mt, :].rearrange("p (b h2 t) -> p b h2 t",
                                                  h2=NH, t=T)[:, :, h, :]
                    if (h * ET + mt) % 3 != 2:
                        nc.vector.tensor_scalar(dst, p[:], scl, None, OP.mult)
                    else:
                        nc.scalar.activation(dst, p[:], AF.Copy, scale=scl)

        # ================= phase D: per-b attention =======================
        with tc.tile_pool(name="ab", bufs=2) as ab, \
             tc.tile_pool(name="xbp2", bufs=1) as xb2_pool:
            xb2[0] = xb2_pool
            for b in range(BL):
                for bn in (b + 1, b + 2):
                    if bn < BL and bn not in XnMap:
                        load_Xn(bn)
                Xn = XnMap.pop(b)
                XT = ab.tile([128, ET, NKC * 128], BF, tag="XT")
                for et in range(ET):
                    if et % 3 == 2:
                        pt = psD.tile([128, 1024], BF, tag="gn")
                    else:
                        pt = psB.tile([128, 1024], BF, tag="ptw")
                    for c in range(NKC):
                        nc.tensor.matmul(pt[:, 128 * c:128 * (c + 1)],
                                         Xn[:, c, 128 * et:128 * (et + 1)],
                                         identb[:], is_transpose=True,
                                         skip_group_check=True)
                    if et != 4:
                        nc.vector.tensor_copy(XT[:, et, :NK], pt[:, :NK])
                    else:
                        nc.scalar.copy(XT[:, et, :NK], pt[:, :NK])
                att = ab.tile([64, NKC * 128], BF, tag="att")
                nc.vector.memset(att[:, NK:], 0.0)
                zacc = ab.tile([64, 2], FP, tag="zacc")
                for ci, (n0, nw) in enumerate(CH_NK):
                    p = psA.tile([64, 512], FP, tag=f"wh{ci}")
                    for kt in range(ET):
                        nc.tensor.matmul(p[:, :nw],
                                         QtT[:, kt, b * 2 * T:(b + 1) * 2 * T],
                                         XT[:, kt, n0:n0 + nw],
                                         start=(kt == 0), stop=(kt == ET - 1))
                    nc.scalar.activation(att[:, n0:n0 + nw], p[:, :nw], AF.Exp,
                                         accum_out=zacc[:, ci:ci + 1])
                zs = ab.tile([64, 1], FP, tag="zs")
                nc.vector.tensor_add(zs[:], zacc[:, 0:1], zacc[:, 1:2])
                rz = ab.tile([64, 1], FP, tag="rz1")
                nc.vector.reciprocal(rz[:], zs[:])
                wm = ab.tile([64, 2], BF, tag="wm")
                nc.vector.tensor_scalar(wm[:], pmask[:], rz[:, 0:1], None, OP.mult)
                # paT[k, i] = sum_q att[q, k] * wm[q, i]  (no transposes!)
                pp = psD.tile([128, 2 * NKC], FP, tag="gr")
                for c in range(NKC):
                    nc.tensor.matmul(pp[:, 2 * c:2 * c + 2],
                                     att[:, 128 * c:128 * (c + 1)], wm[:],
                                     start=True, stop=True,
                                     skip_group_check=True)
                paT = ab.tile([128, NKC, 2], BF, tag="paT")
                nc.vector.tensor_copy(paT[:].rearrange("p a b -> p (a b)"), pp[:])
                # ctxT[d, i] = sum_k Xn[k, d] * paT[k, i]
                pc = psD.tile([128, 2 * ET], FP, tag="gz")
                for dc in range(ET):
                    for c in range(NKC):
                        nc.tensor.matmul(pc[:, 2 * dc:2 * dc + 2],
                                         Xn[:, c, 128 * dc:128 * (dc + 1)],
                                         paT[:, c, :], start=(c == 0),
                                         stop=(c == NKC - 1),
                                         skip_group_check=True)
                nc.vector.tensor_copy(
                    pcxT2[:, :, :, b].rearrange("p a b -> p (a b)"), pc[:])

        # ================= phase E: projections + MLP =====================
        with tc.tile_pool(name="tail", bufs=1) as tail:
            vconT = tail.tile([128, ET], FP, tag="vconT")
            for mt in range(ET):
                p = pgroup(mt)
                for kt in range(ET):
                    nc.tensor.matmul(p[:, 0:1], W3[:, kt, 128 * mt:128 * (mt + 1)],
                                     b2laT_bf[:, kt:kt + 1], start=(kt == 0),
                                     stop=(kt == ET - 1), skip_group_check=True)
                nc.vector.tensor_scalar(vconT[:, mt:mt + 1], p[:, 0:1],
                                        b3laT[:, mt:mt + 1], Sla[:, 0:1],
                                        OP.add, OP.mult)
            pctxT = tail.tile([128, ET, BL], BF, tag="pctxT")
            for h in range(NH):
                for mi in range(3):
                    mt = h * 3 + mi
                    p = pgroup(mt)
                    for kt in range(ET):
                        nc.tensor.matmul(p[:, 0:BL],
                                         W2[:, kt, 128 * mt:128 * (mt + 1)],
                                         pcxT2[:, kt, h, :], start=(kt == 0),
                                         stop=(kt == ET - 1),
                                         skip_group_check=True)
                    nc.vector.tensor_copy(pctxT[:, mt, :], p[:, 0:BL])
            loT = tail.tile([128, ET, BL], BF, tag="loT")
            for mt in range(ET):
                p = pgroup(mt)
                for kt in range(ET):
                    nc.tensor.matmul(p[:, 0:BL], W3[:, kt, 128 * mt:128 * (mt + 1)],
                                     pctxT[:, kt, :], start=(kt == 0),
                                     stop=(kt == ET - 1), skip_group_check=True)
                nc.vector.tensor_scalar(loT[:, mt, :], p[:, 0:BL],
                                        vconT[:, mt:mt + 1], None, OP.add)

            y1T = tail.tile([128, 8, BL], BF, tag="y1T")
            for mt in range(8):
                p = pgroup(mt)
                for i, kt in enumerate(list(range(ET, 12)) + list(range(ET))):
                    r_ = loT[:, kt, :] if kt < ET else goutT[:, kt - ET, :]
                    nc.tensor.matmul(p[:, 0:BL], f1[:, kt, 128 * mt:128 * (mt + 1)],
                                     r_, start=(i == 0), stop=(i == 11),
                                     skip_group_check=True)
                nc.vector.tensor_scalar(y1T[:, mt, :], p[:, 0:BL],
                                        b1fT[:, mt:mt + 1], None, OP.add)
            y2T = tail.tile([128, 4, BL], BF, tag="y2T")
            for mt in range(4):
                p = pgroup(mt)
                for kt in range(8):
                    nc.tensor.matmul(p[:, 0:BL], f2[:, kt, 128 * mt:128 * (mt + 1)],
                                     y1T[:, kt, :], start=(kt == 0), stop=(kt == 7),
                                     skip_group_check=True)
                nc.scalar.activation(y2T[:, mt, :], p[:, 0:BL], AF.Relu,
                                     bias=b2fT[:, mt:mt + 1])
            yT = tail.tile([128, 8, BL], FP, tag="yT")
            for mt in range(8):
                p = pgroup(mt)
                for kt in range(4):
                    nc.tensor.matmul(p[:, 0:BL], f3[:, kt, 128 * mt:128 * (mt + 1)],
                                     y2T[:, kt, :], start=(kt == 0), stop=(kt == 3),
                                     skip_group_check=True)
                nc.vector.tensor_scalar(yT[:, mt, :], p[:, 0:BL],
                                        b3fT[:, mt:mt + 1], None, OP.add)
            ynat = tail.tile([BL, 1024], FP, tag="ynat")
            for g in range(2):
                po = psB.tile([128, 512], FP, tag="ptw")
                for i in range(4):
                    mt = g * 4 + i
                    nc.tensor.matmul(po[:BL, 128 * i:128 * (i + 1)], yT[:, mt, :],
                                     ident[:128, :128], is_transpose=True,
                                     skip_group_check=True)
                nc.vector.tensor_copy(ynat[:, 512 * g:512 * (g + 1)], po[:BL, :])
            nc.sync.dma_start(out_d[:, :], ynat[:])

    nc.compile()
    return nc


_NC = None


def kernel(**inputs):
    global _NC
    if _NC is None:
        _NC = build()
    B = inputs["image_local_embeds"].shape[0]
    per = B // NCORES
    in_maps = []
    for c in range(NCORES):
        sl = slice(c * per, (c + 1) * per)
        m = {
            "img": np.ascontiguousarray(np.asarray(inputs["image_local_embeds"])[sl], dtype=np.float32),
            "h0": np.ascontiguousarray(np.asarray(inputs["h0"])[sl], dtype=np.float32),
        }
        for k in ["gru_w_ih", "gru_w_hh", "gru_b_ih", "gru_b_hh", "ga_w", "ga_b",
                  "ga_pool", "la_w", "la_b", "la_pool", "go_w", "go_b", "go_pool",
                  "f1_w", "f1_b", "f2_w", "f2_b", "f3_w", "f3_b"]:
            m[k] = np.ascontiguousarray(np.asarray(inputs[k], dtype=np.float32))
        in_maps.append(m)
    res = run_bass_kernel_spmd(_NC, in_maps, core_ids=list(range(NCORES)))
    return np.concatenate([res.results[c]["out"] for c in range(NCORES)], axis=0)


# revision 81
# speedup vs baseline: 1.1668x; 1.0065x over previous
"""Trainium2 Bass kernel for nn_BiVision_VQA2 (B=64,T=32,D=768,N=901).

Data-parallel over batch: 8 batch elems per core x 8 cores.
Key math simplifications (validated vs reference, rel err ~1e-4):
  - ga/go attention use a single key token -> softmax==1 -> those paths are
    linear in cls; question_embeds is mathematically unused.
  - GRU input `a` is constant over time; wx computed once.
  - local attention: row-constant score terms drop out of softmax; query
    pooling applied to the attention matrix before the @X contraction.
Performance structure:
  - GRU computed in TRANSPOSED gate layout [128(gate row), batch] via
    weight-stationary matmuls (moving N=8), elementwise on [128, 48].
  - bf16 everywhere outside the GRU recurrence (DMA casts on load).
  - phase D: transpose-free paT/ctxT via natural-operand-stationary matmuls.
"""

import os
import numpy as np
from contextlib import ExitStack

import concourse.bass as bass
import concourse.tile as tile
from concourse import bacc, mybir
from concourse.bass_utils import run_bass_kernel_spmd
from concourse.masks import make_identity

FP = mybir.dt.float32
FPR = mybir.dt.float32r
OP = mybir.AluOpType
AF = mybir.ActivationFunctionType
BF = mybir.dt.bfloat16

NCORES = 8
BL = 8
D = 768
T = 32
G = 3 * D
NK = 900
NH = 2
DK = 384
ET = D // 128
RQ = BL * T

CH_G = [(0, 512), (512, 512), (1024, 512), (1536, 512), (2048, 256)]
CH_NK = [(0, 512), (512, 388)]

GBF = os.environ.get("KGRUBF", "1") == "1"
GDT = BF if GBF else FP


def _r(ap):
    return ap if GBF else ap.bitcast(FPR)


from contextlib import contextmanager


@contextmanager
def _nullcm():
    yield


def kchunks(n):
    out, o = [], 0
    while o < n:
        out.append((o, min(128, n - o)))
        o += 128
    return out


def build():
    nc = bacc.Bacc("TRN2", target_bir_lowering=False, debug=False,
                   enable_asserts=False)

    img = nc.dram_tensor("img", [BL, 901, D], FP, kind="ExternalInput").ap()
    h0 = nc.dram_tensor("h0", [BL, D], FP, kind="ExternalInput").ap()
    w_ih = nc.dram_tensor("gru_w_ih", [G, D], FP, kind="ExternalInput").ap()
    w_hh = nc.dram_tensor("gru_w_hh", [G, D], FP, kind="ExternalInput").ap()
    b_ih = nc.dram_tensor("gru_b_ih", [G], FP, kind="ExternalInput").ap()
    b_hh = nc.dram_tensor("gru_b_hh", [G], FP, kind="ExternalInput").ap()
    ga_w = nc.dram_tensor("ga_w", [4, D, D], FP, kind="ExternalInput").ap()
    ga_b = nc.dram_tensor("ga_b", [4, D], FP, kind="ExternalInput").ap()
    ga_pool = nc.dram_tensor("ga_pool", [1], FP, kind="ExternalInput").ap()
    la_w = nc.dram_tensor("la_w", [4, D, D], FP, kind="ExternalInput").ap()
    la_b = nc.dram_tensor("la_b", [4, D], FP, kind="ExternalInput").ap()
    la_pool = nc.dram_tensor("la_pool", [T], FP, kind="ExternalInput").ap()
    go_w = nc.dram_tensor("go_w", [4, D, D], FP, kind="ExternalInput").ap()
    go_b = nc.dram_tensor("go_b", [4, D], FP, kind="ExternalInput").ap()
    go_pool = nc.dram_tensor("go_pool", [T], FP, kind="ExternalInput").ap()
    f1_w = nc.dram_tensor("f1_w", [2 * D, 1024], FP, kind="ExternalInput").ap()
    f1_b = nc.dram_tensor("f1_b", [1024], FP, kind="ExternalInput").ap()
    f2_w = nc.dram_tensor("f2_w", [1024, 512], FP, kind="ExternalInput").ap()
    f2_b = nc.dram_tensor("f2_b", [512], FP, kind="ExternalInput").ap()
    f3_w = nc.dram_tensor("f3_w", [512, 1024], FP, kind="ExternalInput").ap()
    f3_b = nc.dram_tensor("f3_b", [1024], FP, kind="ExternalInput").ap()
    out_d = nc.dram_tensor("out", [BL, 1024], FP, kind="ExternalOutput").ap()

    def chunked(dram2d, nc_, cw=D):
        # [R, cw] dram viewed as [128, R//128, cw]
        return dram2d.rearrange("(c p) d -> p c d", p=128)

    with tile.TileContext(nc) as tc, ExitStack() as ctx:
        cpool = ctx.enter_context(tc.tile_pool(name="const", bufs=1))
        persist = ctx.enter_context(tc.tile_pool(name="persist", bufs=1))
        xb = ctx.enter_context(tc.tile_pool(name="xb", bufs=2))
        psA = ctx.enter_context(tc.tile_pool(name="psA", bufs=1, space="PSUM"))
        psB = ctx.enter_context(tc.tile_pool(name="psB", bufs=2, space="PSUM"))
        psC = ctx.enter_context(tc.tile_pool(name="psC", bufs=1, space="PSUM"))
        psD = ctx.enter_context(tc.tile_pool(name="psD", bufs=1, space="PSUM"))

        ident = cpool.tile([128, 128], FP, tag="ident")
        make_identity(nc, ident[:])
        identb = cpool.tile([128, 128], BF, tag="identb")
        nc.vector.tensor_copy(identb[:], ident[:])
        ones1 = cpool.tile([1, 128], FP, tag="ones1")
        nc.vector.memset(ones1[:], 1.0)
        ones1b = cpool.tile([1, 128], BF, tag="ones1b")
        nc.vector.memset(ones1b[:], 1.0)
        onesT = cpool.tile([T, 128], FP, tag="onesT")
        nc.vector.memset(onesT[:], 1.0)

        # ---- small bias vectors -> column layout via K=1 matmuls ---------
        def colvec_batch(specs):
            # pipelined: all row-loads first (3 rotating staging slots),
            # then K=1 matmuls into one psum tile, then copies out.
            pdvl = psC.tile([128, 64], FP, tag="pd")
            off = 0
            outs = []
            for idx, (src, n) in enumerate(specs):
                nt = n // 128
                vr = cpool.tile([1, 1024], FP, tag=f"vrow{idx % 2}")
                nc.sync.dma_start(vr[:, :n], src[:][None, :])
                for c in range(nt):
                    nc.tensor.matmul(pdvl[:, off + c:off + c + 1],
                                     vr[0:1, 128 * c:128 * (c + 1)],
                                     ones1[:1, :1], start=True, stop=True,
                                     skip_group_check=True)
                outs.append((off, nt))
                off += nt
            return pdvl, outs

        def colvec_out(pdvl, o_nt, tag):
            o, nt = o_nt
            t_ = cpool.tile([128, nt], FP, tag=tag)
            nc.vector.tensor_copy(t_[:], pdvl[:, o:o + nt])
            return t_

        pdv1, offs1 = colvec_batch([(ga_b[2], D), (ga_b[3], D),
                                    (go_b[2], D), (go_b[3], D),
                                    (la_b[0], D), (la_b[2], D),
                                    (la_b[3], D), (f1_b, 1024),
                                    (f2_b, 512), (f3_b, 1024)])
        b2gaT = colvec_out(pdv1, offs1[0], "b2gaT")
        b3gaT = colvec_out(pdv1, offs1[1], "b3gaT")
        b2goT = colvec_out(pdv1, offs1[2], "b2goT")
        b3goT = colvec_out(pdv1, offs1[3], "b3goT")
        b0laT = colvec_out(pdv1, offs1[4], "b0laT")
        b2laT = colvec_out(pdv1, offs1[5], "b2laT")
        b3laT = colvec_out(pdv1, offs1[6], "b3laT")
        b1fT = colvec_out(pdv1, offs1[7], "b1fT")
        b2fT = colvec_out(pdv1, offs1[8], "b2fT")
        b3fT = colvec_out(pdv1, offs1[9], "b3fT")
        b2laT_bf = cpool.tile([128, ET], BF, tag="b2laT_bf")
        nc.vector.tensor_copy(b2laT_bf[:], b2laT[:])

        lapool_c = cpool.tile([T, 1], FP, tag="lapool_c")
        nc.sync.dma_start(lapool_c[:], la_pool[:][:, None])
        gopool_c = cpool.tile([T, 1], FP, tag="gopool_c")
        nc.sync.dma_start(gopool_c[:], go_pool[:][:, None])
        gapool_c = cpool.tile([1, 1], FP, tag="gapool_c")
        nc.sync.dma_start(gapool_c[:], ga_pool[:][:, None])

        def sum_bcast(vcol, k, tag):
            p = psC.tile([128, 64], FP, tag="pd")
            lhs = onesT if k == T else ones1
            nc.tensor.matmul(p[:, 0:1], lhs[:k, :], vcol[:k, :], start=True,
                             stop=True, skip_group_check=True)
            s = cpool.tile([128, 1], FP, tag=tag)
            nc.vector.tensor_copy(s[:], p[:, 0:1])
            return s

        Sla = sum_bcast(lapool_c, T, "Sla")
        Sgo = sum_bcast(gopool_c, T, "Sgo")
        Sga = sum_bcast(gapool_c, 1, "Sga")

        pmask = cpool.tile([64, 2], FP, tag="pmask")
        nc.vector.memset(pmask[:], 0.0)
        nc.sync.dma_start(pmask[0:T, 0:1], la_pool[:][:, None])
        nc.sync.dma_start(pmask[T:2 * T, 1:2], la_pool[:][:, None])

        # img patch loads (streamed; b0/b1 prefetched early)
        KC = kchunks(NK)
        NKC = len(KC)
        XnMap = {}

        xb2 = [None]

        def load_Xn(b):
            # 3-way buffer rotation: xb holds 2, xb2 (opened for phase D,
            # reusing SBUF freed by the GRU pools) holds the third
            pool = xb2[0] if (b % 3 == 2 and xb2[0] is not None) else xb
            Xn = pool.tile([128, NKC, D], BF, tag="Xn")
            if b < 3:
                # zero the pad rows once per physical buffer (b0,b1 -> xb's
                # two buffers, b2 -> xb2); later b's reuse a buffer and only
                # ever rewrite rows 0..kwl of the last chunk
                nc.vector.memset(Xn[:, NKC - 1, :], 0.0)
            nc.gpsimd.dma_start(
                Xn[:, 0:NKC - 1, :],
                img[b, 1:1 + 128 * (NKC - 1), :].rearrange(
                    "(c p) d -> p c d", p=128))
            k0l, kwl = KC[-1]
            nc.gpsimd.dma_start(Xn[:kwl, NKC - 1, :],
                                img[b, 1 + k0l:1 + k0l + kwl, :])
            XnMap[b] = Xn

        def pgroup(i, ncols=RQ):
            pl, tg = [(psC, "pd"), (psD, "gr"), (psD, "gz"), (psD, "gn")][i % 4]
            pg_t = pl.tile([128, ncols], FP, tag=tg)
            return pg_t

        # persistent outputs of the phases
        qembT = cpool.tile([128, ET, BL, T], BF, tag="qembT")
        wxb = cpool.tile([BL, G], BF, tag="wxb")
        QtT = persist.tile([128, ET, NH * RQ], BF, tag="QtT")
        goutT = cpool.tile([128, ET, BL], BF, tag="goutT")
        aT = cpool.tile([128, ET, BL], GDT, tag="aT")
        pcxT2 = persist.tile([128, ET, NH, BL], BF, tag="pcxT2")

        # ================= phase B: GRU ===================================
        with tc.tile_pool(name="wbig", bufs=1) as wbig, \
             tc.tile_pool(name="wnat", bufs=2) as wnat, \
             tc.tile_pool(name="wst", bufs=3) as wst, \
             tc.tile_pool(name="g1", bufs=2) as g1:
            combr = wbig.tile([1, G], BF, tag="combr")
            nc.gpsimd.dma_start(combr[:], b_ih[:][None, :])
            bhhrow = wbig.tile([1, G], BF, tag="bhhrow")
            nc.gpsimd.dma_start(bhhrow[:], b_hh[:][None, :])
            nc.vector.tensor_add(combr[:, 0:2 * D], combr[:, 0:2 * D],
                                 bhhrow[:, 0:2 * D])
            bhhr_bf = bhhrow[:, 2 * D:3 * D]

            WT = wbig.tile([128, ET, G], GDT, tag="WT")
            tident = identb if GBF else ident

            _rc = [0]

            def build_WT(w_dram, dma_prio=0):
                jts = kchunks(G)
                for g0 in range(0, len(jts), 5):
                    grp = jts[g0:g0 + 5]
                    ng = len(grp)
                    wn = wst.tile([128, 5, D], GDT, tag="wn")
                    src = w_dram[grp[0][0]:grp[-1][0] + grp[-1][1], :]
                    src = src.rearrange("(c p) d -> p c d", p=128)
                    with tc.high_priority(offset=dma_prio if dma_prio else None) \
                            if dma_prio else _nullcm():
                        if GBF:
                            nc.gpsimd.dma_start(wn[:, :ng, :], src)
                        else:
                            nc.sync.dma_start(wn[:, :ng, :], src)
                    sub = 5 if GBF else 3
                    for et in range(ET):
                        for s0 in range(0, ng, sub):
                            sg = min(sub, ng - s0)
                            # rotate staging over 4 banks (gr/gz idle pre-GRU)
                            _rc[0] += 1
                            pl, tg = [(psB, "ptw"), (psD, "gr"),
                                      (psB, "ptw"), (psD, "gz")][_rc[0] % 4]
                            pt = pl.tile([128, 128 * sub], GDT, tag=tg)
                            for i in range(sg):
                                nc.tensor.matmul(pt[:, 128 * i:128 * (i + 1)],
                                                 wn[:, s0 + i, 128 * et:128 * (et + 1)],
                                                 tident[:], is_transpose=True,
                                                 skip_group_check=True)
                            w0 = grp[0][0] + 128 * s0
                            wlen = 128 * sg
                            if (et + s0) % 2 == 0:
                                nc.vector.tensor_copy(_r(WT[:, et, w0:w0 + wlen]),
                                                      pt[:, :wlen])
                            else:
                                nc.scalar.copy(_r(WT[:, et, w0:w0 + wlen]),
                                               pt[:, :wlen])

            build_WT(w_ih)

            # ---- phase A part 1 (cls -> a), interleaved after W_ih ------
            clsn = wbig.tile([BL, D], BF, tag="clsn")
            nc.gpsimd.dma_start(clsn[:], img[0:BL, 0, :])
            wA2 = wnat.tile([128, ET, D], BF, tag="wa")
            nc.gpsimd.dma_start(wA2[:], chunked(ga_w[2], nc))
            wA3 = wnat.tile([128, ET, D], BF, tag="wa")
            nc.gpsimd.dma_start(wA3[:], chunked(ga_w[3], nc))
            ptr = psB.tile([128, 512], BF, tag="ptw")
            for kt in range(ET):
                nc.tensor.matmul(ptr[:, 8 * kt:8 * kt + 8],
                                 clsn[:, 128 * kt:128 * (kt + 1)],
                                 identb[:BL, :BL], is_transpose=True,
                                 skip_group_check=True)
            clsT = wbig.tile([128, ET, BL], BF, tag="clsT")
            nc.vector.tensor_copy(clsT[:].rearrange("p a b -> p (a b)"),
                                  ptr[:, :8 * ET])

            def dense_T(wsb, rhsT, biasT, scaleT, otile, out_r=False):
                for mt in range(ET):
                    p = psC.tile([128, BL], FP, tag="pd")
                    for kt in range(ET):
                        nc.tensor.matmul(p[:], wsb[:, kt, 128 * mt:128 * (mt + 1)],
                                         rhsT[:, kt, :], start=(kt == 0),
                                         stop=(kt == ET - 1))
                    dst = otile[:, mt, :]
                    if out_r:
                        dst = _r(dst)
                    if scaleT is None:
                        nc.vector.tensor_scalar(dst, p[:], biasT[:, mt:mt + 1],
                                                None, OP.add)
                    else:
                        nc.vector.tensor_scalar(dst, p[:], biasT[:, mt:mt + 1],
                                                scaleT[:, 0:1], OP.add, OP.mult)

            A2T = wbig.tile([128, ET, BL], BF, tag="A2T")
            dense_T(wA2, clsT, b2gaT, None, A2T)
            dense_T(wA3, A2T, b3gaT, Sga, aT, out_r=not GBF)

            for (j0, jw) in CH_G:
                p = psA.tile([BL, 512], FP, tag="wh0")
                for kt in range(ET):
                    nc.tensor.matmul(p[:, :jw], aT[:, kt, :] if GBF else _r(aT[:, kt, :]),
                                     _r(WT[:, kt, j0:j0 + jw]),
                                     start=(kt == 0), stop=False)
                nc.tensor.matmul(p[:, :jw], ones1b[:1, :BL],
                                 combr[:, j0:j0 + jw], start=False, stop=True)
                nc.vector.tensor_copy(wxb[:, j0:j0 + jw], p[:, :jw])

            build_WT(w_hh)

            # ---- phase A part 2 (gout path) — loads emitted here, the
            # dense compute happens inside the GRU loop (idle engine slack)
            wG2 = wnat.tile([128, ET, D], BF, tag="wa")
            nc.gpsimd.dma_start(wG2[:], chunked(go_w[2], nc))
            wG3 = wnat.tile([128, ET, D], BF, tag="wa")
            nc.gpsimd.dma_start(wG3[:], chunked(go_w[3], nc))
            G2T = wbig.tile([128, ET, BL], BF, tag="G2T")

            def dense_T_ptw(wsb, rhsT, biasT, scaleT, otile):
                # dense_T variant staged in the ptw banks (free during GRU)
                for mt in range(ET):
                    p = psB.tile([128, BL], FP, tag="ptw")
                    for kt in range(ET):
                        nc.tensor.matmul(p[:], wsb[:, kt, 128 * mt:128 * (mt + 1)],
                                         rhsT[:, kt, :], start=(kt == 0),
                                         stop=(kt == ET - 1))
                    if scaleT is None:
                        nc.vector.tensor_scalar(otile[:, mt, :], p[:],
                                                biasT[:, mt:mt + 1], None, OP.add)
                    else:
                        nc.vector.tensor_scalar(otile[:, mt, :], p[:],
                                                biasT[:, mt:mt + 1],
                                                scaleT[:, 0:1], OP.add, OP.mult)

            # transposed constant wx for the n-gate: [128, ET, BL]
            ptx = psC.tile([128, 64], BF, tag="pd")
            for kt in range(ET):
                nc.tensor.matmul(ptx[:, 8 * kt:8 * kt + 8],
                                 wxb[:, 2 * D + 128 * kt:2 * D + 128 * (kt + 1)],
                                 identb[:BL, :BL], is_transpose=True,
                                 skip_group_check=True)
            wxTn = wbig.tile([128, ET, BL], FP, tag="wxTn")
            nc.vector.tensor_copy(wxTn[:].rearrange("p a b -> p (a b)"),
                                  ptx[:, :8 * ET])

            # initial h0 transposed
            hnat0 = wbig.tile([BL, D], BF, tag="hnat0")
            nc.gpsimd.dma_start(hnat0[:], h0[:, :])
            ptr0 = psC.tile([128, 64], BF, tag="pd")
            for kt in range(ET):
                nc.tensor.matmul(ptr0[:, 8 * kt:8 * kt + 8],
                                 hnat0[:, 128 * kt:128 * (kt + 1)],
                                 identb[:BL, :BL], is_transpose=True,
                                 skip_group_check=True)
            hT = wbig.tile([128, ET, BL], GDT, tag="h0T")
            nc.vector.tensor_copy(_r(hT[:].rearrange("p a b -> p (a b)")),
                                  ptr0[:, :8 * ET])

            # prefetch DMAs for phases C/D/E: deprioritized so they only
            # fill DMA slots the W/A loads are not using
            with tc.high_priority(offset=-100000):
                W0 = persist.tile([128, ET, D], BF, tag="W0")
                nc.gpsimd.dma_start(W0[:], chunked(la_w[0], nc))
                W1 = persist.tile([128, ET, D], BF, tag="W1")
                nc.gpsimd.dma_start(W1[:], chunked(la_w[1], nc))
                W2 = persist.tile([128, ET, D], BF, tag="W2")
                nc.gpsimd.dma_start(W2[:], chunked(la_w[2], nc))
                W3 = persist.tile([128, ET, D], BF, tag="W3")
                nc.gpsimd.dma_start(W3[:], chunked(la_w[3], nc))
                f1 = persist.tile([128, 12, 1024], BF, tag="f1")
                nc.gpsimd.dma_start(f1[:], f1_w.rearrange("(c p) n -> p c n", p=128))
                f2 = persist.tile([128, 8, 512], BF, tag="f2")
                nc.gpsimd.dma_start(f2[:], f2_w.rearrange("(c p) n -> p c n", p=128))
                f3 = persist.tile([128, 4, 1024], BF, tag="f3")
                nc.gpsimd.dma_start(f3[:], f3_w.rearrange("(c p) n -> p c n", p=128))
                load_Xn(0)

            KSTEPS = int(os.environ.get("KSTEPS", str(T)))
            KHALF = os.environ.get("KHALF", "1") == "1"
            HB = BL // 2
            wxTn3 = wxTn[:]
            if not KHALF:
                for t in range(KSTEPS):
                    psR = psD.tile([128, ET * BL], FP, tag="gr")
                    psZ = psD.tile([128, ET * BL], FP, tag="gz")
                    psN = psD.tile([128, ET * BL], FP, tag="gn")

                    def gate_chunk(ps, mi, m):
                        j0 = 128 * m
                        for kt in range(ET):
                            nc.tensor.matmul(ps[:, BL * mi:BL * (mi + 1)],
                                             _r(WT[:, kt, j0:j0 + 128]),
                                             _r(hT[:, kt, :]),
                                             start=(kt == 0), stop=False,
                                             skip_group_check=True)
                        if m < 12:
                            nc.tensor.matmul(ps[:, BL * mi:BL * (mi + 1)],
                                             wxb[:, j0:j0 + 128],
                                             identb[:BL, :BL], start=False,
                                             stop=True, skip_group_check=True)
                        else:
                            nc.tensor.matmul(ps[:, BL * mi:BL * (mi + 1)],
                                             bhhr_bf[:, j0 - 2 * D:j0 - 2 * D + 128],
                                             ones1b[:1, :BL],
                                             start=False, stop=True,
                                             skip_group_check=True)

                    for mi in range(ET):
                        gate_chunk(psR, mi, mi)
                    for mi in range(ET):
                        gate_chunk(psN, mi, 12 + mi)
                    for mi in range(ET):
                        gate_chunk(psZ, mi, 6 + mi)

                    # h_new = (1-z)*n + z*h ; z-products run in tanh's shadow
                    rsig = g1.tile([128, ET * BL], FP, tag="rsig")
                    nc.scalar.activation(rsig[:], psR[:], AF.Sigmoid)
                    zsig = g1.tile([128, ET * BL], FP, tag="zsig")
                    nc.scalar.activation(zsig[:], psZ[:], AF.Sigmoid)
                    rwn = g1.tile([128, ET * BL], FP, tag="rwn")
                    nc.vector.tensor_mul(rwn[:], rsig[:], psN[:])
                    npre = g1.tile([128, ET * BL], FP, tag="npre")
                    nc.vector.tensor_add(npre[:], rwn[:],
                                         wxTn[:].rearrange("p a b -> p (a b)"))
                    nt_ = g1.tile([128, ET * BL], FP, tag="nt")
                    nc.scalar.activation(nt_[:], npre[:], AF.Tanh)
                    zh = g1.tile([128, ET * BL], FP, tag="zh")
                    nc.vector.tensor_mul(zh[:], zsig[:],
                                         hT[:].rearrange("p a b -> p (a b)"))
                    omz = g1.tile([128, ET * BL], FP, tag="omz")
                    nc.vector.tensor_scalar(omz[:], zsig[:], -1.0, 1.0,
                                            OP.mult, OP.add)
                    ozn = g1.tile([128, ET * BL], FP, tag="ozn")
                    nc.vector.tensor_mul(ozn[:], omz[:], nt_[:])
                    hT = g1.tile([128, ET, BL], GDT, tag="hT")
                    nc.vector.tensor_add(_r(hT[:].rearrange("p a b -> p (a b)")),
                                         ozn[:], zh[:])
                    nc.scalar.copy(qembT[:, :, :, t].rearrange("p a b -> p (a b)"),
                                   hT[:].rearrange("p a b -> p (a b)"))
            else:
                # two independent half-batch chains, interleaved so each
                # half's elementwise hides in the other's latency
                hTs = [None, None]
                psmap = [(psD, "gr"), (psD, "gz"), (psA, "wh0"), (psA, "wh1"),
                         (psD, "gn"), (psC, "pd")]
                for t in range(KSTEPS):
                    def hprev(g, kt):
                        if t == 0:
                            return _r(hT[:, kt, HB * g:HB * (g + 1)])
                        return _r(hTs[g][:, kt, :])

                    def ps_half(i):
                        pl, tg = psmap[i]
                        ph_t = pl.tile([128, ET * HB], FP, tag=tg)
                        return ph_t

                    psRs = [ps_half(0), ps_half(1)]
                    psZs = [ps_half(2), ps_half(3)]
                    psNs = [ps_half(4), ps_half(5)]

                    def gate_const(ps, mi, m, g):
                        # constant (wx/bhh) opener: no h dependency, so it
                        # runs during the previous step's elementwise tail
                        j0 = 128 * m
                        if m < 12:
                            nc.tensor.matmul(ps[:, HB * mi:HB * (mi + 1)],
                                             wxb[:, j0:j0 + 128],
                                             identb[:BL, HB * g:HB * (g + 1)],
                                             start=True, stop=False,
                                             skip_group_check=True)
                        else:
                            nc.tensor.matmul(ps[:, HB * mi:HB * (mi + 1)],
                                             bhhr_bf[:, j0 - 2 * D:j0 - 2 * D + 128],
                                             ones1b[:1, :HB],
                                             start=True, stop=False,
                                             skip_group_check=True)

                    def gate_h(ps, mi, m, g):
                        j0 = 128 * m
                        for kt in range(ET):
                            nc.tensor.matmul(ps[:, HB * mi:HB * (mi + 1)],
                                             _r(WT[:, kt, j0:j0 + 128]),
                                             hprev(g, kt),
                                             start=False, stop=(kt == ET - 1),
                                             skip_group_check=True)

                    for ps_list, m0 in ((psRs, 0), (psNs, 12), (psZs, 6)):
                        for g in (0, 1):
                            for mi in range(ET):
                                gate_const(ps_list[g], mi, m0 + mi, g)
                    for ps_list, m0 in ((psRs, 0), (psNs, 12), (psZs, 6)):
                        for g in (0, 1):
                            for mi in range(ET):
                                gate_h(ps_list[g], mi, m0 + mi, g)

                    def tile3(tag):
                        t3 = g1.tile([128, ET, HB], FP, tag=tag)
                        return t3

                    rsig = [tile3("rsig0"), tile3("rsig1")]
                    zsig = [tile3("zsig0"), tile3("zsig1")]
                    rwn = [tile3("rwn0"), tile3("rwn1")]
                    npre = [tile3("npre0"), tile3("npre1")]
                    nt_ = [tile3("nt0"), tile3("nt1")]
                    zh = [tile3("zh0"), tile3("zh1")]
                    omz = [tile3("omz0"), tile3("omz1")]
                    ozn = [tile3("ozn0"), tile3("ozn1")]
                    def tile3g(tag):
                        t3g = g1.tile([128, ET, HB], GDT, tag=tag)
                        return t3g

                    hnew = [tile3g("hTn0"), tile3g("hTn1")]
                    for g in (0, 1):
                        nc.scalar.activation(rsig[g][:].rearrange("p a b -> p (a b)"),
                                             psRs[g][:], AF.Sigmoid)
                    for g in (0, 1):
                        nc.scalar.activation(zsig[g][:].rearrange("p a b -> p (a b)"),
                                             psZs[g][:], AF.Sigmoid)
                    for g in (0, 1):
                        nc.vector.tensor_mul(rwn[g][:].rearrange("p a b -> p (a b)"),
                                             rsig[g][:].rearrange("p a b -> p (a b)"),
                                             psNs[g][:])
                    for g in (0, 1):
                        nc.vector.tensor_add(npre[g][:], rwn[g][:],
                                             wxTn3[:, :, HB * g:HB * (g + 1)])
                    for g in (0, 1):
                        nc.scalar.activation(nt_[g][:].rearrange("p a b -> p (a b)"),
                                             npre[g][:].rearrange("p a b -> p (a b)"),
                                             AF.Tanh)
                    for g in (0, 1):
                        hp = (hT[:, :, HB * g:HB * (g + 1)] if t == 0
                              else hTs[g][:])
                        nc.vector.tensor_mul(zh[g][:], zsig[g][:], hp)
                    for g in (0, 1):
                        nc.vector.tensor_scalar(omz[g][:].rearrange("p a b -> p (a b)"),
                                                zsig[g][:].rearrange("p a b -> p (a b)"),
                                                -1.0, 1.0, OP.mult, OP.add)
                    for g in (0, 1):
                        nc.vector.tensor_mul(ozn[g][:], omz[g][:], nt_[g][:])
                    for g in (0, 1):
                        nc.vector.tensor_add(_r(hnew[g][:]), ozn[g][:], zh[g][:])
                    for g in (0, 1):
                        nc.scalar.copy(qembT[:, :, HB * g:HB * (g + 1), t],
                                       hnew[g][:])
                    hTs = hnew
                    if t == 2:
                        dense_T_ptw(wG2, clsT, b2goT, None, G2T)
                    if t == 4:
                        dense_T_ptw(wG3, G2T, b3goT, Sgo, goutT)
            load_Xn(1)

        # ================= phase C: Q^T, W1^T, Qt^T =======================
        with tc.tile_pool(name="prep", bufs=1) as prep:
            QT = prep.tile([128, ET, RQ], BF, tag="QT")
            qflat = qembT[:].rearrange("p a b t -> p a (b t)")
            for mt in range(ET):
                p = pgroup(mt)
                for kt in range(ET):
                    nc.tensor.matmul(p[:], W0[:, kt, 128 * mt:128 * (mt + 1)],
                                     qflat[:, kt, :], start=(kt == 0),
                                     stop=(kt == ET - 1))
                nc.vector.tensor_scalar(QT[:, mt, :], p[:], b0laT[:, mt:mt + 1],
                                        None, OP.add)
            W1T = prep.tile([128, ET, D], BF, tag="W1T")
            for hd in range(ET):
                for grp in range(2):
                    pt2 = psB.tile([128, 512], BF, tag="ptw")
                    for i in range(3):
                        e2 = grp * 3 + i
                        nc.tensor.matmul(pt2[:, 128 * i:128 * (i + 1)],
                                         W1[:, e2, 128 * hd:128 * (hd + 1)],
                                         identb[:], is_transpose=True,
                                         skip_group_check=True)
                    if grp == 0:
                        nc.vector.tensor_copy(W1T[:, hd, 0:384], pt2[:, 0:384])
                    else:
                        nc.scalar.copy(W1T[:, hd, 384:768], pt2[:, 0:384])
            scl = 1.0 / float(np.sqrt(DK))
            for h in range(NH):
                for mt in range(ET):
                    p = pgroup(h * ET + mt)
                    for i in range(3):
                        kt = h * 3 + i
                        nc.tensor.matmul(p[:], W1T[:, kt, 128 * mt:128 * (mt + 1)],
                                         QT[:, kt, :], start=(i == 0), stop=(i == 2))
                    dst = QtT[:, mt, :].rearrange("p (b h2 t) -> p b h2 t",
                                                  h2=NH, t=T)[:, :, h, :]
                    if (h * ET + mt) % 3 != 2:
                        nc.vector.tensor_scalar(dst, p[:], scl, None, OP.mult)
                    else:
                        nc.scalar.activation(dst, p[:], AF.Copy, scale=scl)

        # ================= phase D: per-b attention =======================
        with tc.tile_pool(name="ab", bufs=2) as ab, \
             tc.tile_pool(name="xbp2", bufs=1) as xb2_pool:
            xb2[0] = xb2_pool
            for b in range(BL):
                for bn in (b + 1, b + 2):
                    if bn < BL and bn not in XnMap:
                        load_Xn(bn)
                Xn = XnMap.pop(b)
                XT = ab.tile([128, ET, NKC * 128], BF, tag="XT")
                for et in range(ET):
                    pl, tg = [(psB, "ptw"), (psD, "gn"), (psB, "ptw"),
                              (psC, "pd")][et % 4]
                    pt = pl.tile([128, 1024], BF, tag=tg)
                    for c in range(NKC):
                        nc.tensor.matmul(pt[:, 128 * c:128 * (c + 1)],
                                         Xn[:, c, 128 * et:128 * (et + 1)],
                                         identb[:], is_transpose=True,
                                         skip_group_check=True)
                    if et != 4:
                        nc.vector.tensor_copy(XT[:, et, :NK], pt[:, :NK])
                    else:
                        nc.scalar.copy(XT[:, et, :NK], pt[:, :NK])
                att = ab.tile([64, NKC * 128], BF, tag="att")
                nc.vector.memset(att[:, NK:], 0.0)
                zacc = ab.tile([64, 2], FP, tag="zacc")
                for ci, (n0, nw) in enumerate(CH_NK):
                    p = psA.tile([64, 512], FP, tag=f"wh{ci}")
                    for kt in range(ET):
                        nc.tensor.matmul(p[:, :nw],
                                         QtT[:, kt, b * 2 * T:(b + 1) * 2 * T],
                                         XT[:, kt, n0:n0 + nw],
                                         start=(kt == 0), stop=(kt == ET - 1))
                    nc.scalar.activation(att[:, n0:n0 + nw], p[:, :nw], AF.Exp,
                                         accum_out=zacc[:, ci:ci + 1])
                zs = ab.tile([64, 1], FP, tag="zs")
                nc.vector.tensor_add(zs[:], zacc[:, 0:1], zacc[:, 1:2])
                rz = ab.tile([64, 1], FP, tag="rz1")
                nc.vector.reciprocal(rz[:], zs[:])
                wm = ab.tile([64, 2], BF, tag="wm")
                nc.vector.tensor_scalar(wm[:], pmask[:], rz[:, 0:1], None, OP.mult)
                # paT[k, i] = sum_q att[q, k] * wm[q, i]  (no transposes!)
                pp = psD.tile([128, 2 * NKC], FP, tag="gr")
                for c in range(NKC):
                    nc.tensor.matmul(pp[:, 2 * c:2 * c + 2],
                                     att[:, 128 * c:128 * (c + 1)], wm[:],
                                     start=True, stop=True,
                                     skip_group_check=True)
                paT = ab.tile([128, NKC, 2], BF, tag="paT")
                nc.vector.tensor_copy(paT[:].rearrange("p a b -> p (a b)"), pp[:])
                # ctxT[d, i] = sum_k Xn[k, d] * paT[k, i]
                pc = psD.tile([128, 2 * ET], FP, tag="gz")
                for dc in range(ET):
                    for c in range(NKC):
                        nc.tensor.matmul(pc[:, 2 * dc:2 * dc + 2],
                                         Xn[:, c, 128 * dc:128 * (dc + 1)],
                                         paT[:, c, :], start=(c == 0),
                                         stop=(c == NKC - 1),
                                         skip_group_check=True)
                nc.vector.tensor_copy(
                    pcxT2[:, :, :, b].rearrange("p a b -> p (a b)"), pc[:])

        # ================= phase E: projections + MLP =====================
        with tc.tile_pool(name="tail", bufs=1) as tail:
            vconT = tail.tile([128, ET], FP, tag="vconT")
            for mt in range(ET):
                p = pgroup(mt)
                for kt in range(ET):
                    nc.tensor.matmul(p[:, 0:1], W3[:, kt, 128 * mt:128 * (mt + 1)],
                                     b2laT_bf[:, kt:kt + 1], start=(kt == 0),
                                     stop=(kt == ET - 1), skip_group_check=True)
                nc.vector.tensor_scalar(vconT[:, mt:mt + 1], p[:, 0:1],
                                        b3laT[:, mt:mt + 1], Sla[:, 0:1],
                                        OP.add, OP.mult)
            pctxT = tail.tile([128, ET, BL], BF, tag="pctxT")
            for h in range(NH):
                for mi in range(3):
                    mt = h * 3 + mi
                    p = pgroup(mt)
                    for kt in range(ET):
                        nc.tensor.matmul(p[:, 0:BL],
                                         W2[:, kt, 128 * mt:128 * (mt + 1)],
                                         pcxT2[:, kt, h, :], start=(kt == 0),
                                         stop=(kt == ET - 1),
                                         skip_group_check=True)
                    nc.vector.tensor_copy(pctxT[:, mt, :], p[:, 0:BL])
            loT = tail.tile([128, ET, BL], BF, tag="loT")
            for mt in range(ET):
                p = pgroup(mt)
                for kt in range(ET):
                    nc.tensor.matmul(p[:, 0:BL], W3[:, kt, 128 * mt:128 * (mt + 1)],
                                     pctxT[:, kt, :], start=(kt == 0),
                                     stop=(kt == ET - 1), skip_group_check=True)
                nc.vector.tensor_scalar(loT[:, mt, :], p[:, 0:BL],
                                        vconT[:, mt:mt + 1], None, OP.add)

            y1T = tail.tile([128, 8, BL], BF, tag="y1T")
            for mt in range(8):
                p = pgroup(mt)
                for i, kt in enumerate(list(range(ET, 12)) + list(range(ET))):
                    r_ = loT[:, kt, :] if kt < ET else goutT[:, kt - ET, :]
                    nc.tensor.matmul(p[:, 0:BL], f1[:, kt, 128 * mt:128 * (mt + 1)],
                                     r_, start=(i == 0), stop=(i == 11),
                                     skip_group_check=True)
                nc.vector.tensor_scalar(y1T[:, mt, :], p[:, 0:BL],
                                        b1fT[:, mt:mt + 1], None, OP.add)
            y2T = tail.tile([128, 4, BL], BF, tag="y2T")
            for mt in range(4):
                p = pgroup(mt)
                for kt in range(8):
                    nc.tensor.matmul(p[:, 0:BL], f2[:, kt, 128 * mt:128 * (mt + 1)],
                                     y1T[:, kt, :], start=(kt == 0), stop=(kt == 7),
                                     skip_group_check=True)
                nc.scalar.activation(y2T[:, mt, :], p[:, 0:BL], AF.Relu,
                                     bias=b2fT[:, mt:mt + 1])
            yT = tail.tile([128, 8, BL], FP, tag="yT")
            for mt in range(8):
                p = pgroup(mt)
                for kt in range(4):
                    nc.tensor.matmul(p[:, 0:BL], f3[:, kt, 128 * mt:128 * (mt + 1)],
                                     y2T[:, kt, :], start=(kt == 0), stop=(kt == 3),
                                     skip_group_check=True)
                nc.vector.tensor_scalar(yT[:, mt, :], p[:, 0:BL],
                                        b3fT[:, mt:mt + 1], None, OP.add)
            ynat = tail.tile([BL, 1024], FP, tag="ynat")
            for g in range(2):
                po = psB.tile([128, 512], FP, tag="ptw")
                for i in range(4):
                    mt = g * 4 + i
                    nc.tensor.matmul(po[:BL, 128 * i:128 * (i + 1)], yT[:, mt, :],
                                     ident[:128, :128], is_transpose=True,
                                     skip_group_check=True)
                nc.vector.tensor_copy(ynat[:, 512 * g:512 * (g + 1)], po[:BL, :])
            nc.sync.dma_start(out_d[:, :], ynat[:])

    nc.compile()
    return nc


_NC = None


def kernel(**inputs):
    global _NC
    if _NC is None:
        _NC = build()
    B = inputs["image_local_embeds"].shape[0]
    per = B // NCORES
    in_maps = []
    for c in range(NCORES):
        sl = slice(c * per, (c + 1) * per)
        m = {
            "img": np.ascontiguousarray(np.asarray(inputs["image_local_embeds"])[sl], dtype=np.float32),
            "h0": np.ascontiguousarray(np.asarray(inputs["h0"])[sl], dtype=np.float32),
        }
        for k in ["gru_w_ih", "gru_w_hh", "gru_b_ih", "gru_b_hh", "ga_w", "ga_b",
                  "ga_pool", "la_w", "la_b", "la_pool", "go_w", "go_b", "go_pool",
                  "f1_w", "f1_b", "f2_w", "f2_b", "f3_w", "f3_b"]:
            m[k] = np.ascontiguousarray(np.asarray(inputs[k], dtype=np.float32))
        in_maps.append(m)
    res = run_bass_kernel_spmd(_NC, in_maps, core_ids=list(range(NCORES)))
    return np.concatenate([res.results[c]["out"] for c in range(NCORES)], axis=0)


# revision 86
# speedup vs baseline: 1.1880x; 1.0182x over previous
"""Trainium2 Bass kernel for nn_BiVision_VQA2 (B=64,T=32,D=768,N=901).

Data-parallel over batch: 8 batch elems per core x 8 cores.
Key math simplifications (validated vs reference, rel err ~1e-4):
  - ga/go attention use a single key token -> softmax==1 -> those paths are
    linear in cls; question_embeds is mathematically unused.
  - GRU input `a` is constant over time; wx computed once.
  - local attention: row-constant score terms drop out of softmax; query
    pooling applied to the attention matrix before the @X contraction.
Performance structure:
  - GRU computed in TRANSPOSED gate layout [128(gate row), batch] via
    weight-stationary matmuls (moving N=8), elementwise on [128, 48].
  - bf16 everywhere outside the GRU recurrence (DMA casts on load).
  - phase D: transpose-free paT/ctxT via natural-operand-stationary matmuls.
"""

import os
import numpy as np
from contextlib import ExitStack

import concourse.bass as bass
import concourse.tile as tile
from concourse import bacc, mybir
from concourse.bass_utils import run_bass_kernel_spmd
from concourse.masks import make_identity

FP = mybir.dt.float32
FPR = mybir.dt.float32r
OP = mybir.AluOpType
AF = mybir.ActivationFunctionType
BF = mybir.dt.bfloat16

NCORES = 8
BL = 8
D = 768
T = 32
G = 3 * D
NK = 900
NH = 2
DK = 384
ET = D // 128
RQ = BL * T

CH_G = [(0, 512), (512, 512), (1024, 512), (1536, 512), (2048, 256)]
CH_NK = [(0, 512), (512, 388)]

GBF = os.environ.get("KGRUBF", "1") == "1"
GDT = BF if GBF else FP


def _r(ap):
    return ap if GBF else ap.bitcast(FPR)


from contextlib import contextmanager


@contextmanager
def _nullcm():
    yield


def kchunks(n):
    out, o = [], 0
    while o < n:
        out.append((o, min(128, n - o)))
        o += 128
    return out


def build():
    nc = bacc.Bacc("TRN2", target_bir_lowering=False, debug=False,
                   enable_asserts=False)

    img = nc.dram_tensor("img", [BL, 901, D], FP, kind="ExternalInput").ap()
    h0 = nc.dram_tensor("h0", [BL, D], FP, kind="ExternalInput").ap()
    w_ih = nc.dram_tensor("gru_w_ih", [G, D], FP, kind="ExternalInput").ap()
    w_hh = nc.dram_tensor("gru_w_hh", [G, D], FP, kind="ExternalInput").ap()
    b_ih = nc.dram_tensor("gru_b_ih", [G], FP, kind="ExternalInput").ap()
    b_hh = nc.dram_tensor("gru_b_hh", [G], FP, kind="ExternalInput").ap()
    ga_w = nc.dram_tensor("ga_w", [4, D, D], FP, kind="ExternalInput").ap()
    ga_b = nc.dram_tensor("ga_b", [4, D], FP, kind="ExternalInput").ap()
    ga_pool = nc.dram_tensor("ga_pool", [1], FP, kind="ExternalInput").ap()
    la_w = nc.dram_tensor("la_w", [4, D, D], FP, kind="ExternalInput").ap()
    la_b = nc.dram_tensor("la_b", [4, D], FP, kind="ExternalInput").ap()
    la_pool = nc.dram_tensor("la_pool", [T], FP, kind="ExternalInput").ap()
    go_w = nc.dram_tensor("go_w", [4, D, D], FP, kind="ExternalInput").ap()
    go_b = nc.dram_tensor("go_b", [4, D], FP, kind="ExternalInput").ap()
    go_pool = nc.dram_tensor("go_pool", [T], FP, kind="ExternalInput").ap()
    f1_w = nc.dram_tensor("f1_w", [2 * D, 1024], FP, kind="ExternalInput").ap()
    f1_b = nc.dram_tensor("f1_b", [1024], FP, kind="ExternalInput").ap()
    f2_w = nc.dram_tensor("f2_w", [1024, 512], FP, kind="ExternalInput").ap()
    f2_b = nc.dram_tensor("f2_b", [512], FP, kind="ExternalInput").ap()
    f3_w = nc.dram_tensor("f3_w", [512, 1024], FP, kind="ExternalInput").ap()
    f3_b = nc.dram_tensor("f3_b", [1024], FP, kind="ExternalInput").ap()
    out_d = nc.dram_tensor("out", [BL, 1024], FP, kind="ExternalOutput").ap()

    def chunked(dram2d, nc_, cw=D):
        # [R, cw] dram viewed as [128, R//128, cw]
        return dram2d.rearrange("(c p) d -> p c d", p=128)

    with tile.TileContext(nc) as tc, ExitStack() as ctx:
        cpool = ctx.enter_context(tc.tile_pool(name="const", bufs=1))
        persist = ctx.enter_context(tc.tile_pool(name="persist", bufs=1))
        xb = ctx.enter_context(tc.tile_pool(name="xb", bufs=2))
        psA = ctx.enter_context(tc.tile_pool(name="psA", bufs=1, space="PSUM"))
        psB = ctx.enter_context(tc.tile_pool(name="psB", bufs=2, space="PSUM"))
        psC = ctx.enter_context(tc.tile_pool(name="psC", bufs=1, space="PSUM"))
        psD = ctx.enter_context(tc.tile_pool(name="psD", bufs=1, space="PSUM"))

        ident = cpool.tile([128, 128], FP, tag="ident")
        make_identity(nc, ident[:])
        identb = cpool.tile([128, 128], BF, tag="identb")
        nc.vector.tensor_copy(identb[:], ident[:])
        ones1 = cpool.tile([1, 128], FP, tag="ones1")
        nc.vector.memset(ones1[:], 1.0)
        ones1b = cpool.tile([1, 128], BF, tag="ones1b")
        nc.vector.memset(ones1b[:], 1.0)
        onesT = cpool.tile([T, 128], FP, tag="onesT")
        nc.vector.memset(onesT[:], 1.0)

        # ---- small bias vectors -> column layout via K=1 matmuls ---------
        def colvec_batch(specs):
            # pipelined: all row-loads first (3 rotating staging slots),
            # then K=1 matmuls into one psum tile, then copies out.
            pdvl = psC.tile([128, 64], FP, tag="pd")
            off = 0
            outs = []
            for idx, (src, n) in enumerate(specs):
                nt = n // 128
                vr = cpool.tile([1, 1024], FP, tag=f"vrow{idx % 2}")
                nc.sync.dma_start(vr[:, :n], src[:][None, :])
                for c in range(nt):
                    nc.tensor.matmul(pdvl[:, off + c:off + c + 1],
                                     vr[0:1, 128 * c:128 * (c + 1)],
                                     ones1[:1, :1], start=True, stop=True,
                                     skip_group_check=True)
                outs.append((off, nt))
                off += nt
            return pdvl, outs

        def colvec_out(pdvl, o_nt, tag):
            o, nt = o_nt
            t_ = cpool.tile([128, nt], FP, tag=tag)
            nc.vector.tensor_copy(t_[:], pdvl[:, o:o + nt])
            return t_

        pdv1, offs1 = colvec_batch([(ga_b[2], D), (ga_b[3], D),
                                    (go_b[2], D), (go_b[3], D),
                                    (la_b[0], D), (la_b[2], D),
                                    (la_b[3], D), (f1_b, 1024),
                                    (f2_b, 512), (f3_b, 1024)])
        b2gaT = colvec_out(pdv1, offs1[0], "b2gaT")
        b3gaT = colvec_out(pdv1, offs1[1], "b3gaT")
        b2goT = colvec_out(pdv1, offs1[2], "b2goT")
        b3goT = colvec_out(pdv1, offs1[3], "b3goT")
        b0laT = colvec_out(pdv1, offs1[4], "b0laT")
        b2laT = colvec_out(pdv1, offs1[5], "b2laT")
        b3laT = colvec_out(pdv1, offs1[6], "b3laT")
        b1fT = colvec_out(pdv1, offs1[7], "b1fT")
        b2fT = colvec_out(pdv1, offs1[8], "b2fT")
        b3fT = colvec_out(pdv1, offs1[9], "b3fT")
        b2laT_bf = cpool.tile([128, ET], BF, tag="b2laT_bf")
        nc.vector.tensor_copy(b2laT_bf[:], b2laT[:])

        lapool_c = cpool.tile([T, 1], FP, tag="lapool_c")
        nc.sync.dma_start(lapool_c[:], la_pool[:][:, None])
        gopool_c = cpool.tile([T, 1], FP, tag="gopool_c")
        nc.sync.dma_start(gopool_c[:], go_pool[:][:, None])
        gapool_c = cpool.tile([1, 1], FP, tag="gapool_c")
        nc.sync.dma_start(gapool_c[:], ga_pool[:][:, None])

        def sum_bcast(vcol, k, tag):
            p = psC.tile([128, 64], FP, tag="pd")
            lhs = onesT if k == T else ones1
            nc.tensor.matmul(p[:, 0:1], lhs[:k, :], vcol[:k, :], start=True,
                             stop=True, skip_group_check=True)
            s = cpool.tile([128, 1], FP, tag=tag)
            nc.vector.tensor_copy(s[:], p[:, 0:1])
            return s

        Sla = sum_bcast(lapool_c, T, "Sla")
        Sgo = sum_bcast(gopool_c, T, "Sgo")
        Sga = sum_bcast(gapool_c, 1, "Sga")

        pmask = cpool.tile([64, 2], FP, tag="pmask")
        nc.vector.memset(pmask[:], 0.0)
        nc.sync.dma_start(pmask[0:T, 0:1], la_pool[:][:, None])
        nc.sync.dma_start(pmask[T:2 * T, 1:2], la_pool[:][:, None])

        # img patch loads (streamed; b0/b1 prefetched early)
        KC = kchunks(NK)
        NKC = len(KC)
        XnMap = {}

        xb2 = [None]

        def load_Xn(b):
            # 3-way buffer rotation: xb holds 2, xb2 (opened for phase D,
            # reusing SBUF freed by the GRU pools) holds the third
            pool = xb2[0] if (b % 3 == 2 and xb2[0] is not None) else xb
            Xn = pool.tile([128, NKC, D], BF, tag="Xn")
            if b < 3:
                # zero the pad rows once per physical buffer (b0,b1 -> xb's
                # two buffers, b2 -> xb2); later b's reuse a buffer and only
                # ever rewrite rows 0..kwl of the last chunk
                nc.vector.memset(Xn[:, NKC - 1, :], 0.0)
            nc.gpsimd.dma_start(
                Xn[:, 0:NKC - 1, :],
                img[b, 1:1 + 128 * (NKC - 1), :].rearrange(
                    "(c p) d -> p c d", p=128))
            k0l, kwl = KC[-1]
            nc.gpsimd.dma_start(Xn[:kwl, NKC - 1, :],
                                img[b, 1 + k0l:1 + k0l + kwl, :])
            XnMap[b] = Xn

        def pgroup(i, ncols=RQ):
            pl, tg = [(psC, "pd"), (psD, "gr"), (psD, "gz"), (psD, "gn")][i % 4]
            pg_t = pl.tile([128, ncols], FP, tag=tg)
            return pg_t

        def pgroup6(i, ncols=RQ):
            # phase E only: wh0/wh1 are also free once D is done
            pl, tg = [(psC, "pd"), (psD, "gr"), (psD, "gz"), (psD, "gn"),
                      (psA, "wh0"), (psA, "wh1")][i % 6]
            pg_t = pl.tile([128, ncols], FP, tag=tg)
            return pg_t

        # persistent outputs of the phases
        qembT = cpool.tile([128, ET, BL, T], BF, tag="qembT")
        wxb = cpool.tile([BL, G], BF, tag="wxb")
        QtT = persist.tile([128, ET, NH * RQ], BF, tag="QtT")
        goutT = cpool.tile([128, ET, BL], BF, tag="goutT")
        aT = cpool.tile([128, ET, BL], GDT, tag="aT")
        pcxT2 = persist.tile([128, ET, NH, BL], BF, tag="pcxT2")

        # ================= phase B: GRU ===================================
        with tc.tile_pool(name="wbig", bufs=1) as wbig, \
             tc.tile_pool(name="wnat", bufs=2) as wnat, \
             tc.tile_pool(name="wst", bufs=3) as wst, \
             tc.tile_pool(name="g1", bufs=2) as g1:
            combr = wbig.tile([1, G], BF, tag="combr")
            nc.gpsimd.dma_start(combr[:], b_ih[:][None, :])
            bhhrow = wbig.tile([1, G], BF, tag="bhhrow")
            nc.gpsimd.dma_start(bhhrow[:], b_hh[:][None, :])
            nc.vector.tensor_add(combr[:, 0:2 * D], combr[:, 0:2 * D],
                                 bhhrow[:, 0:2 * D])
            bhhr_bf = bhhrow[:, 2 * D:3 * D]

            WT = wbig.tile([128, ET, G], GDT, tag="WT")
            tident = identb if GBF else ident

            _rc = [0]

            def build_WT(w_dram, dma_prio=0):
                jts = kchunks(G)
                for g0 in range(0, len(jts), 5):
                    grp = jts[g0:g0 + 5]
                    ng = len(grp)
                    wn = wst.tile([128, 5, D], GDT, tag="wn")
                    src = w_dram[grp[0][0]:grp[-1][0] + grp[-1][1], :]
                    src = src.rearrange("(c p) d -> p c d", p=128)
                    with tc.high_priority(offset=dma_prio if dma_prio else None) \
                            if dma_prio else _nullcm():
                        if GBF:
                            nc.gpsimd.dma_start(wn[:, :ng, :], src)
                        else:
                            nc.sync.dma_start(wn[:, :ng, :], src)
                    sub = 5 if GBF else 3
                    for et in range(ET):
                        for s0 in range(0, ng, sub):
                            sg = min(sub, ng - s0)
                            # rotate staging over 4 banks (gr/gz idle pre-GRU)
                            _rc[0] += 1
                            pl, tg = [(psB, "ptw"), (psD, "gr"),
                                      (psB, "ptw"), (psD, "gz")][_rc[0] % 4]
                            pt = pl.tile([128, 128 * sub], GDT, tag=tg)
                            for i in range(sg):
                                nc.tensor.matmul(pt[:, 128 * i:128 * (i + 1)],
                                                 wn[:, s0 + i, 128 * et:128 * (et + 1)],
                                                 tident[:], is_transpose=True,
                                                 skip_group_check=True)
                            w0 = grp[0][0] + 128 * s0
                            wlen = 128 * sg
                            if (et + s0) % 2 == 0:
                                nc.vector.tensor_copy(_r(WT[:, et, w0:w0 + wlen]),
                                                      pt[:, :wlen])
                            else:
                                nc.scalar.copy(_r(WT[:, et, w0:w0 + wlen]),
                                               pt[:, :wlen])

            build_WT(w_ih)

            # ---- phase A part 1 (cls -> a), interleaved after W_ih ------
            clsn = wbig.tile([BL, D], BF, tag="clsn")
            nc.gpsimd.dma_start(clsn[:], img[0:BL, 0, :])
            wA2 = wnat.tile([128, ET, D], BF, tag="wa")
            nc.gpsimd.dma_start(wA2[:], chunked(ga_w[2], nc))
            wA3 = wnat.tile([128, ET, D], BF, tag="wa")
            nc.gpsimd.dma_start(wA3[:], chunked(ga_w[3], nc))
            ptr = psB.tile([128, 512], BF, tag="ptw")
            for kt in range(ET):
                nc.tensor.matmul(ptr[:, 8 * kt:8 * kt + 8],
                                 clsn[:, 128 * kt:128 * (kt + 1)],
                                 identb[:BL, :BL], is_transpose=True,
                                 skip_group_check=True)
            clsT = wbig.tile([128, ET, BL], BF, tag="clsT")
            nc.vector.tensor_copy(clsT[:].rearrange("p a b -> p (a b)"),
                                  ptr[:, :8 * ET])

            def dense_T(wsb, rhsT, biasT, scaleT, otile, out_r=False):
                for mt in range(ET):
                    p = psC.tile([128, BL], FP, tag="pd")
                    for kt in range(ET):
                        nc.tensor.matmul(p[:], wsb[:, kt, 128 * mt:128 * (mt + 1)],
                                         rhsT[:, kt, :], start=(kt == 0),
                                         stop=(kt == ET - 1))
                    dst = otile[:, mt, :]
                    if out_r:
                        dst = _r(dst)
                    if scaleT is None:
                        nc.vector.tensor_scalar(dst, p[:], biasT[:, mt:mt + 1],
                                                None, OP.add)
                    else:
                        nc.vector.tensor_scalar(dst, p[:], biasT[:, mt:mt + 1],
                                                scaleT[:, 0:1], OP.add, OP.mult)

            A2T = wbig.tile([128, ET, BL], BF, tag="A2T")
            dense_T(wA2, clsT, b2gaT, None, A2T)
            dense_T(wA3, A2T, b3gaT, Sga, aT, out_r=not GBF)

            for (j0, jw) in CH_G:
                p = psA.tile([BL, 512], FP, tag="wh0")
                for kt in range(ET):
                    nc.tensor.matmul(p[:, :jw], aT[:, kt, :] if GBF else _r(aT[:, kt, :]),
                                     _r(WT[:, kt, j0:j0 + jw]),
                                     start=(kt == 0), stop=False)
                nc.tensor.matmul(p[:, :jw], ones1b[:1, :BL],
                                 combr[:, j0:j0 + jw], start=False, stop=True)
                nc.vector.tensor_copy(wxb[:, j0:j0 + jw], p[:, :jw])

            build_WT(w_hh)

            # ---- phase A part 2 (gout path) — loads emitted here, the
            # dense compute happens inside the GRU loop (idle engine slack)
            wG2 = wnat.tile([128, ET, D], BF, tag="wa")
            nc.gpsimd.dma_start(wG2[:], chunked(go_w[2], nc))
            wG3 = wnat.tile([128, ET, D], BF, tag="wa")
            nc.gpsimd.dma_start(wG3[:], chunked(go_w[3], nc))
            G2T = wbig.tile([128, ET, BL], BF, tag="G2T")

            def dense_T_ptw(wsb, rhsT, biasT, scaleT, otile):
                # dense_T variant staged in the ptw banks (free during GRU)
                for mt in range(ET):
                    p = psB.tile([128, BL], FP, tag="ptw")
                    for kt in range(ET):
                        nc.tensor.matmul(p[:], wsb[:, kt, 128 * mt:128 * (mt + 1)],
                                         rhsT[:, kt, :], start=(kt == 0),
                                         stop=(kt == ET - 1))
                    if scaleT is None:
                        nc.vector.tensor_scalar(otile[:, mt, :], p[:],
                                                biasT[:, mt:mt + 1], None, OP.add)
                    else:
                        nc.vector.tensor_scalar(otile[:, mt, :], p[:],
                                                biasT[:, mt:mt + 1],
                                                scaleT[:, 0:1], OP.add, OP.mult)

            # transposed constant wx for the n-gate: [128, ET, BL]
            ptx = psC.tile([128, 64], BF, tag="pd")
            for kt in range(ET):
                nc.tensor.matmul(ptx[:, 8 * kt:8 * kt + 8],
                                 wxb[:, 2 * D + 128 * kt:2 * D + 128 * (kt + 1)],
                                 identb[:BL, :BL], is_transpose=True,
                                 skip_group_check=True)
            wxTn = wbig.tile([128, ET, BL], FP, tag="wxTn")
            nc.vector.tensor_copy(wxTn[:].rearrange("p a b -> p (a b)"),
                                  ptx[:, :8 * ET])

            # initial h0 transposed
            hnat0 = wbig.tile([BL, D], BF, tag="hnat0")
            nc.gpsimd.dma_start(hnat0[:], h0[:, :])
            ptr0 = psC.tile([128, 64], BF, tag="pd")
            for kt in range(ET):
                nc.tensor.matmul(ptr0[:, 8 * kt:8 * kt + 8],
                                 hnat0[:, 128 * kt:128 * (kt + 1)],
                                 identb[:BL, :BL], is_transpose=True,
                                 skip_group_check=True)
            hT = wbig.tile([128, ET, BL], GDT, tag="h0T")
            nc.vector.tensor_copy(_r(hT[:].rearrange("p a b -> p (a b)")),
                                  ptr0[:, :8 * ET])

            # prefetch DMAs for phases C/D/E: deprioritized so they only
            # fill DMA slots the W/A loads are not using
            with tc.high_priority(offset=-100000):
                W0 = persist.tile([128, ET, D], BF, tag="W0")
                nc.gpsimd.dma_start(W0[:], chunked(la_w[0], nc))
                W1 = persist.tile([128, ET, D], BF, tag="W1")
                nc.gpsimd.dma_start(W1[:], chunked(la_w[1], nc))
                W2 = persist.tile([128, ET, D], BF, tag="W2")
                nc.gpsimd.dma_start(W2[:], chunked(la_w[2], nc))
                W3 = persist.tile([128, ET, D], BF, tag="W3")
                nc.gpsimd.dma_start(W3[:], chunked(la_w[3], nc))
                f1 = persist.tile([128, 12, 1024], BF, tag="f1")
                nc.gpsimd.dma_start(f1[:], f1_w.rearrange("(c p) n -> p c n", p=128))
                f2 = persist.tile([128, 8, 512], BF, tag="f2")
                nc.gpsimd.dma_start(f2[:], f2_w.rearrange("(c p) n -> p c n", p=128))
                f3 = persist.tile([128, 4, 1024], BF, tag="f3")
                nc.gpsimd.dma_start(f3[:], f3_w.rearrange("(c p) n -> p c n", p=128))
                load_Xn(0)

            KSTEPS = int(os.environ.get("KSTEPS", str(T)))
            KHALF = os.environ.get("KHALF", "1") == "1"
            HB = BL // 2
            wxTn3 = wxTn[:]
            if not KHALF:
                for t in range(KSTEPS):
                    psR = psD.tile([128, ET * BL], FP, tag="gr")
                    psZ = psD.tile([128, ET * BL], FP, tag="gz")
                    psN = psD.tile([128, ET * BL], FP, tag="gn")

                    def gate_chunk(ps, mi, m):
                        j0 = 128 * m
                        for kt in range(ET):
                            nc.tensor.matmul(ps[:, BL * mi:BL * (mi + 1)],
                                             _r(WT[:, kt, j0:j0 + 128]),
                                             _r(hT[:, kt, :]),
                                             start=(kt == 0), stop=False,
                                             skip_group_check=True)
                        if m < 12:
                            nc.tensor.matmul(ps[:, BL * mi:BL * (mi + 1)],
                                             wxb[:, j0:j0 + 128],
                                             identb[:BL, :BL], start=False,
                                             stop=True, skip_group_check=True)
                        else:
                            nc.tensor.matmul(ps[:, BL * mi:BL * (mi + 1)],
                                             bhhr_bf[:, j0 - 2 * D:j0 - 2 * D + 128],
                                             ones1b[:1, :BL],
                                             start=False, stop=True,
                                             skip_group_check=True)

                    for mi in range(ET):
                        gate_chunk(psR, mi, mi)
                    for mi in range(ET):
                        gate_chunk(psN, mi, 12 + mi)
                    for mi in range(ET):
                        gate_chunk(psZ, mi, 6 + mi)

                    # h_new = (1-z)*n + z*h ; z-products run in tanh's shadow
                    rsig = g1.tile([128, ET * BL], FP, tag="rsig")
                    nc.scalar.activation(rsig[:], psR[:], AF.Sigmoid)
                    zsig = g1.tile([128, ET * BL], FP, tag="zsig")
                    nc.scalar.activation(zsig[:], psZ[:], AF.Sigmoid)
                    rwn = g1.tile([128, ET * BL], FP, tag="rwn")
                    nc.vector.tensor_mul(rwn[:], rsig[:], psN[:])
                    npre = g1.tile([128, ET * BL], FP, tag="npre")
                    nc.vector.tensor_add(npre[:], rwn[:],
                                         wxTn[:].rearrange("p a b -> p (a b)"))
                    nt_ = g1.tile([128, ET * BL], FP, tag="nt")
                    nc.scalar.activation(nt_[:], npre[:], AF.Tanh)
                    zh = g1.tile([128, ET * BL], FP, tag="zh")
                    nc.vector.tensor_mul(zh[:], zsig[:],
                                         hT[:].rearrange("p a b -> p (a b)"))
                    omz = g1.tile([128, ET * BL], FP, tag="omz")
                    nc.vector.tensor_scalar(omz[:], zsig[:], -1.0, 1.0,
                                            OP.mult, OP.add)
                    ozn = g1.tile([128, ET * BL], FP, tag="ozn")
                    nc.vector.tensor_mul(ozn[:], omz[:], nt_[:])
                    hT = g1.tile([128, ET, BL], GDT, tag="hT")
                    nc.vector.tensor_add(_r(hT[:].rearrange("p a b -> p (a b)")),
                                         ozn[:], zh[:])
                    nc.scalar.copy(qembT[:, :, :, t].rearrange("p a b -> p (a b)"),
                                   hT[:].rearrange("p a b -> p (a b)"))
            else:
                # two independent half-batch chains, interleaved so each
                # half's elementwise hides in the other's latency
                hTs = [None, None]
                psmap = [(psD, "gr"), (psD, "gz"), (psA, "wh0"), (psA, "wh1"),
                         (psD, "gn"), (psC, "pd")]
                for t in range(KSTEPS):
                    def hprev(g, kt):
                        if t == 0:
                            return _r(hT[:, kt, HB * g:HB * (g + 1)])
                        return _r(hTs[g][:, kt, :])

                    def ps_half(i):
                        pl, tg = psmap[i]
                        ph_t = pl.tile([128, ET * HB], FP, tag=tg)
                        return ph_t

                    psRs = [ps_half(0), ps_half(1)]
                    psZs = [ps_half(2), ps_half(3)]
                    psNs = [ps_half(4), ps_half(5)]

                    def gate_const(ps, mi, m, g):
                        # constant (wx/bhh) opener: no h dependency, so it
                        # runs during the previous step's elementwise tail
                        j0 = 128 * m
                        if m < 12:
                            nc.tensor.matmul(ps[:, HB * mi:HB * (mi + 1)],
                                             wxb[:, j0:j0 + 128],
                                             identb[:BL, HB * g:HB * (g + 1)],
                                             start=True, stop=False,
                                             skip_group_check=True)
                        else:
                            nc.tensor.matmul(ps[:, HB * mi:HB * (mi + 1)],
                                             bhhr_bf[:, j0 - 2 * D:j0 - 2 * D + 128],
                                             ones1b[:1, :HB],
                                             start=True, stop=False,
                                             skip_group_check=True)

                    def gate_h(ps, mi, m, g):
                        j0 = 128 * m
                        for kt in range(ET):
                            nc.tensor.matmul(ps[:, HB * mi:HB * (mi + 1)],
                                             _r(WT[:, kt, j0:j0 + 128]),
                                             hprev(g, kt),
                                             start=False, stop=(kt == ET - 1),
                                             skip_group_check=True)

                    for ps_list, m0 in ((psRs, 0), (psNs, 12), (psZs, 6)):
                        for g in (0, 1):
                            for mi in range(ET):
                                gate_const(ps_list[g], mi, m0 + mi, g)
                    for ps_list, m0 in ((psRs, 0), (psNs, 12), (psZs, 6)):
                        for g in (0, 1):
                            for mi in range(ET):
                                gate_h(ps_list[g], mi, m0 + mi, g)

                    def tile3(tag):
                        t3 = g1.tile([128, ET, HB], FP, tag=tag)
                        return t3

                    rsig = [tile3("rsig0"), tile3("rsig1")]
                    zsig = [tile3("zsig0"), tile3("zsig1")]
                    rwn = [tile3("rwn0"), tile3("rwn1")]
                    npre = [tile3("npre0"), tile3("npre1")]
                    nt_ = [tile3("nt0"), tile3("nt1")]
                    zh = [tile3("zh0"), tile3("zh1")]
                    omz = [tile3("omz0"), tile3("omz1")]
                    ozn = [tile3("ozn0"), tile3("ozn1")]
                    def tile3g(tag):
                        t3g = g1.tile([128, ET, HB], GDT, tag=tag)
                        return t3g

                    hnew = [tile3g("hTn0"), tile3g("hTn1")]
                    for g in (0, 1):
                        nc.scalar.activation(rsig[g][:].rearrange("p a b -> p (a b)"),
                                             psRs[g][:], AF.Sigmoid)
                    for g in (0, 1):
                        nc.scalar.activation(zsig[g][:].rearrange("p a b -> p (a b)"),
                                             psZs[g][:], AF.Sigmoid)
                    for g in (0, 1):
                        nc.vector.tensor_mul(rwn[g][:].rearrange("p a b -> p (a b)"),
                                             rsig[g][:].rearrange("p a b -> p (a b)"),
                                             psNs[g][:])
                    for g in (0, 1):
                        nc.vector.tensor_add(npre[g][:], rwn[g][:],
                                             wxTn3[:, :, HB * g:HB * (g + 1)])
                    for g in (0, 1):
                        nc.scalar.activation(nt_[g][:].rearrange("p a b -> p (a b)"),
                                             npre[g][:].rearrange("p a b -> p (a b)"),
                                             AF.Tanh)
                    for g in (0, 1):
                        hp = (hT[:, :, HB * g:HB * (g + 1)] if t == 0
                              else hTs[g][:])
                        nc.vector.tensor_mul(zh[g][:], zsig[g][:], hp)
                    for g in (0, 1):
                        nc.vector.tensor_scalar(omz[g][:].rearrange("p a b -> p (a b)"),
                                                zsig[g][:].rearrange("p a b -> p (a b)"),
                                                -1.0, 1.0, OP.mult, OP.add)
                    for g in (0, 1):
                        nc.vector.tensor_mul(ozn[g][:], omz[g][:], nt_[g][:])
                    for g in (0, 1):
                        nc.vector.tensor_add(_r(hnew[g][:]), ozn[g][:], zh[g][:])
                    for g in (0, 1):
                        nc.vector.tensor_copy(qembT[:, :, HB * g:HB * (g + 1), t],
                                              hnew[g][:])
                    hTs = hnew
                    if t == 2:
                        dense_T_ptw(wG2, clsT, b2goT, None, G2T)
                    if t == 4:
                        dense_T_ptw(wG3, G2T, b3goT, Sgo, goutT)
            load_Xn(1)

        # ================= phase C: Q^T, W1^T, Qt^T =======================
        with tc.tile_pool(name="prep", bufs=1) as prep:
            QT = prep.tile([128, ET, RQ], BF, tag="QT")
            qflat = qembT[:].rearrange("p a b t -> p a (b t)")
            for mt in range(ET):
                p = pgroup(mt)
                for kt in range(ET):
                    nc.tensor.matmul(p[:], W0[:, kt, 128 * mt:128 * (mt + 1)],
                                     qflat[:, kt, :], start=(kt == 0),
                                     stop=(kt == ET - 1))
                nc.vector.tensor_scalar(QT[:, mt, :], p[:], b0laT[:, mt:mt + 1],
                                        None, OP.add)
            W1T = prep.tile([128, ET, D], BF, tag="W1T")
            for hd in range(ET):
                for grp in range(2):
                    # rotate staging over ptw x2 + wh0/wh1 (idle during C)
                    pl2, tg2 = [(psB, "ptw"), (psA, "wh0"),
                                (psB, "ptw"), (psA, "wh1")][(2 * hd + grp) % 4]
                    pt2 = pl2.tile([128, 512], BF, tag=tg2)
                    for i in range(3):
                        e2 = grp * 3 + i
                        nc.tensor.matmul(pt2[:, 128 * i:128 * (i + 1)],
                                         W1[:, e2, 128 * hd:128 * (hd + 1)],
                                         identb[:], is_transpose=True,
                                         skip_group_check=True)
                    if grp == 0:
                        nc.vector.tensor_copy(W1T[:, hd, 0:384], pt2[:, 0:384])
                    else:
                        nc.scalar.copy(W1T[:, hd, 384:768], pt2[:, 0:384])
            scl = 1.0 / float(np.sqrt(DK))
            for h in range(NH):
                for mt in range(ET):
                    p = pgroup(h * ET + mt)
                    for i in range(3):
                        kt = h * 3 + i
                        nc.tensor.matmul(p[:], W1T[:, kt, 128 * mt:128 * (mt + 1)],
                                         QT[:, kt, :], start=(i == 0), stop=(i == 2))
                    dst = QtT[:, mt, :].rearrange("p (b h2 t) -> p b h2 t",
                                                  h2=NH, t=T)[:, :, h, :]
                    if (h * ET + mt) % 3 != 2:
                        nc.vector.tensor_scalar(dst, p[:], scl, None, OP.mult)
                    else:
                        nc.scalar.activation(dst, p[:], AF.Copy, scale=scl)

        # ================= phase D: per-b attention =======================
        with tc.tile_pool(name="ab", bufs=2) as ab, \
             tc.tile_pool(name="xbp2", bufs=1) as xb2_pool:
            xb2[0] = xb2_pool
            for b in range(BL):
                for bn in (b + 1, b + 2):
                    if bn < BL and bn not in XnMap:
                        load_Xn(bn)
                Xn = XnMap.pop(b)
                XT = ab.tile([128, ET, NKC * 128], BF, tag="XT")
                for et in range(ET):
                    pl, tg = [(psB, "ptw"), (psD, "gn"), (psB, "ptw"),
                              (psC, "pd")][et % 4]
                    pt = pl.tile([128, 1024], BF, tag=tg)
                    for c in range(NKC):
                        nc.tensor.matmul(pt[:, 128 * c:128 * (c + 1)],
                                         Xn[:, c, 128 * et:128 * (et + 1)],
                                         identb[:], is_transpose=True,
                                         skip_group_check=True)
                    if et != 4:
                        nc.vector.tensor_copy(XT[:, et, :NK], pt[:, :NK])
                    else:
                        nc.scalar.copy(XT[:, et, :NK], pt[:, :NK])
                att = ab.tile([64, NKC * 128], BF, tag="att")
                nc.vector.memset(att[:, NK:], 0.0)
                zacc = ab.tile([64, 2], FP, tag="zacc")
                for ci, (n0, nw) in enumerate(CH_NK):
                    p = psA.tile([64, 512], FP, tag=f"wh{ci}")
                    for kt in range(ET):
                        nc.tensor.matmul(p[:, :nw],
                                         QtT[:, kt, b * 2 * T:(b + 1) * 2 * T],
                                         XT[:, kt, n0:n0 + nw],
                                         start=(kt == 0), stop=(kt == ET - 1))
                    nc.scalar.activation(att[:, n0:n0 + nw], p[:, :nw], AF.Exp,
                                         accum_out=zacc[:, ci:ci + 1])
                zs = ab.tile([64, 1], FP, tag="zs")
                nc.vector.tensor_add(zs[:], zacc[:, 0:1], zacc[:, 1:2])
                rz = ab.tile([64, 1], FP, tag="rz1")
                nc.vector.reciprocal(rz[:], zs[:])
                wm = ab.tile([64, 2], BF, tag="wm")
                nc.vector.tensor_scalar(wm[:], pmask[:], rz[:, 0:1], None, OP.mult)
                # paT[k, i] = sum_q att[q, k] * wm[q, i]  (no transposes!)
                pp = psD.tile([128, 2 * NKC], FP, tag="gr")
                for c in range(NKC):
                    nc.tensor.matmul(pp[:, 2 * c:2 * c + 2],
                                     att[:, 128 * c:128 * (c + 1)], wm[:],
                                     start=True, stop=True,
                                     skip_group_check=True)
                paT = ab.tile([128, NKC, 2], BF, tag="paT")
                nc.vector.tensor_copy(paT[:].rearrange("p a b -> p (a b)"), pp[:])
                # ctxT[d, i] = sum_k Xn[k, d] * paT[k, i]
                pc = psD.tile([128, 2 * ET], FP, tag="gz")
                for dc in range(ET):
                    for c in range(NKC):
                        nc.tensor.matmul(pc[:, 2 * dc:2 * dc + 2],
                                         Xn[:, c, 128 * dc:128 * (dc + 1)],
                                         paT[:, c, :], start=(c == 0),
                                         stop=(c == NKC - 1),
                                         skip_group_check=True)
                nc.vector.tensor_copy(
                    pcxT2[:, :, :, b].rearrange("p a b -> p (a b)"), pc[:])

        # ================= phase E: projections + MLP =====================
        with tc.tile_pool(name="tail", bufs=1) as tail:
            vconT = tail.tile([128, ET], FP, tag="vconT")
            for mt in range(ET):
                p = pgroup6(mt)
                for kt in range(ET):
                    nc.tensor.matmul(p[:, 0:1], W3[:, kt, 128 * mt:128 * (mt + 1)],
                                     b2laT_bf[:, kt:kt + 1], start=(kt == 0),
                                     stop=(kt == ET - 1), skip_group_check=True)
                nc.vector.tensor_scalar(vconT[:, mt:mt + 1], p[:, 0:1],
                                        b3laT[:, mt:mt + 1], Sla[:, 0:1],
                                        OP.add, OP.mult)
            pctxT = tail.tile([128, ET, BL], BF, tag="pctxT")
            for h in range(NH):
                for mi in range(3):
                    mt = h * 3 + mi
                    p = pgroup6(mt)
                    for kt in range(ET):
                        nc.tensor.matmul(p[:, 0:BL],
                                         W2[:, kt, 128 * mt:128 * (mt + 1)],
                                         pcxT2[:, kt, h, :], start=(kt == 0),
                                         stop=(kt == ET - 1),
                                         skip_group_check=True)
                    nc.vector.tensor_copy(pctxT[:, mt, :], p[:, 0:BL])
            loT = tail.tile([128, ET, BL], BF, tag="loT")
            for mt in range(ET):
                p = pgroup6(mt)
                for kt in range(ET):
                    nc.tensor.matmul(p[:, 0:BL], W3[:, kt, 128 * mt:128 * (mt + 1)],
                                     pctxT[:, kt, :], start=(kt == 0),
                                     stop=(kt == ET - 1), skip_group_check=True)
                nc.vector.tensor_scalar(loT[:, mt, :], p[:, 0:BL],
                                        vconT[:, mt:mt + 1], None, OP.add)

            y1T = tail.tile([128, 8, BL], BF, tag="y1T")
            for mt in range(8):
                p = pgroup6(mt)
                for i, kt in enumerate(list(range(ET, 12)) + list(range(ET))):
                    r_ = loT[:, kt, :] if kt < ET else goutT[:, kt - ET, :]
                    nc.tensor.matmul(p[:, 0:BL], f1[:, kt, 128 * mt:128 * (mt + 1)],
                                     r_, start=(i == 0), stop=(i == 11),
                                     skip_group_check=True)
                nc.vector.tensor_scalar(y1T[:, mt, :], p[:, 0:BL],
                                        b1fT[:, mt:mt + 1], None, OP.add)
            y2T = tail.tile([128, 4, BL], BF, tag="y2T")
            for mt in range(4):
                p = pgroup6(mt)
                for kt in range(8):
                    nc.tensor.matmul(p[:, 0:BL], f2[:, kt, 128 * mt:128 * (mt + 1)],
                                     y1T[:, kt, :], start=(kt == 0), stop=(kt == 7),
                                     skip_group_check=True)
                nc.scalar.activation(y2T[:, mt, :], p[:, 0:BL], AF.Relu,
                                     bias=b2fT[:, mt:mt + 1])
            yT = tail.tile([128, 8, BL], FP, tag="yT")
            for mt in range(8):
                p = pgroup6(mt)
                for kt in range(4):
                    nc.tensor.matmul(p[:, 0:BL], f3[:, kt, 128 * mt:128 * (mt + 1)],
                                     y2T[:, kt, :], start=(kt == 0), stop=(kt == 3),
                                     skip_group_check=True)
                nc.vector.tensor_scalar(yT[:, mt, :], p[:, 0:BL],
                                        b3fT[:, mt:mt + 1], None, OP.add)
            ynat = tail.tile([BL, 1024], FP, tag="ynat")
            for g in range(2):
                po = psB.tile([128, 512], FP, tag="ptw")
                for i in range(4):
                    mt = g * 4 + i
                    nc.tensor.matmul(po[:BL, 128 * i:128 * (i + 1)], yT[:, mt, :],
                                     ident[:128, :128], is_transpose=True,
                                     skip_group_check=True)
                nc.vector.tensor_copy(ynat[:, 512 * g:512 * (g + 1)], po[:BL, :])
            nc.sync.dma_start(out_d[:, :], ynat[:])

    nc.compile()
    return nc


_NC = None


def kernel(**inputs):
    global _NC
    if _NC is None:
        _NC = build()
    B = inputs["image_local_embeds"].shape[0]
    per = B // NCORES
    in_maps = []
    for c in range(NCORES):
        sl = slice(c * per, (c + 1) * per)
        m = {
            "img": np.ascontiguousarray(np.asarray(inputs["image_local_embeds"])[sl], dtype=np.float32),
            "h0": np.ascontiguousarray(np.asarray(inputs["h0"])[sl], dtype=np.float32),
        }
        for k in ["gru_w_ih", "gru_w_hh", "gru_b_ih", "gru_b_hh", "ga_w", "ga_b",
                  "ga_pool", "la_w", "la_b", "la_pool", "go_w", "go_b", "go_pool",
                  "f1_w", "f1_b", "f2_w", "f2_b", "f3_w", "f3_b"]:
            m[k] = np.ascontiguousarray(np.asarray(inputs[k], dtype=np.float32))
        in_maps.append(m)
    res = run_bass_kernel_spmd(_NC, in_maps, core_ids=list(range(NCORES)))
    return np.concatenate([res.results[c]["out"] for c in range(NCORES)], axis=0)
